# revision 55
# baseline (speedup 1.0000x reference)
import math
import os
import numpy as np
import ml_dtypes
import contextlib

import concourse.bass as bass
import concourse.tile as tile
from concourse import bacc, mybir, masks
from concourse.bass_utils import run_bass_kernel_spmd

F32 = mybir.dt.float32
F32R = mybir.dt.float32r
BF16 = mybir.dt.bfloat16
ALU = mybir.AluOpType
AF = mybir.ActivationFunctionType
AX = mybir.AxisListType

NCORES = 8
FRAME = 5
NF = FRAME - 1
D = 512
DH = 64
HEADS = 8
B = 20
NPTS = 1024
BE = B // FRAME
NPC = NPTS // NCORES     # 128 points per core
TOK = NF * BE * NPC      # 2048 tokens per core
NLAYER = 12
LNEPS = 1e-5
BNEPS = 1e-5
NBN = 16 * NPTS

CONV_DIMS = [2048, 1536, 1024, 768, 512]


def _pe_table(max_len=16, d=D):
    pos = np.arange(max_len, dtype=np.float32)[:, None]
    div = np.exp(np.arange(0, d, 2, dtype=np.float32) * (-math.log(10000.0) / d))
    pe = np.zeros((max_len, d), np.float32)
    pe[:, 0::2] = np.sin(pos * div)
    pe[:, 1::2] = np.cos(pos * div)
    return pe


def build_kernel():
    nc = bacc.Bacc("TRN2", target_bir_lowering=False, debug=False,
                   num_devices=NCORES)

    xin = nc.dram_tensor("xin", [CONV_DIMS[0], TOK], F32R, kind="ExternalInput").ap()
    convw = [nc.dram_tensor(f"convw{i}", [CONV_DIMS[i], CONV_DIMS[i + 1]], F32R,
                            kind="ExternalInput").ap() for i in range(4)]
    bnconst = [nc.dram_tensor(f"bnconst{i}", [128, 3 * (CONV_DIMS[i + 1] // 128)],
                              F32, kind="ExternalInput").ap() for i in range(3)]
    bias4 = nc.dram_tensor("bias4", [128, 4 * NF], F32, kind="ExternalInput").ap()

    wqkv_d = nc.dram_tensor("wqkv", [NLAYER, D, 3 * D], F32R, kind="ExternalInput").ap()
    wsbq_d = nc.dram_tensor("wsbq", [NLAYER, 2, 3 * D], BF16, kind="ExternalInput").ap()
    wo_d = nc.dram_tensor("wo", [NLAYER, D, D], BF16, kind="ExternalInput").ap()
    w1_d = nc.dram_tensor("w1", [NLAYER, D, D], F32R, kind="ExternalInput").ap()
    w2_d = nc.dram_tensor("w2", [NLAYER, D, D], F32R, kind="ExternalInput").ap()
    tcols_d = nc.dram_tensor("tcols", [NLAYER, 128, 12], F32, kind="ExternalInput").ap()

    projw_d = nc.dram_tensor("projw", [D, D], F32R, kind="ExternalInput").ap()
    rw1_d = nc.dram_tensor("rw1", [D, 256], F32R, kind="ExternalInput").ap()
    rw2_d = nc.dram_tensor("rw2", [256, 128], F32R, kind="ExternalInput").ap()
    rw3_d = nc.dram_tensor("rw3", [128, 8], F32R, kind="ExternalInput").ap()
    tw1_d = nc.dram_tensor("tw1", [D, 256], F32R, kind="ExternalInput").ap()
    tw2_d = nc.dram_tensor("tw2", [256, 128], F32R, kind="ExternalInput").ap()
    tw3_d = nc.dram_tensor("tw3", [128, 8], F32R, kind="ExternalInput").ap()
    hcols_d = nc.dram_tensor("hcols", [128, 10], F32, kind="ExternalInput").ap()
    sb3_d = nc.dram_tensor("sb3", [8, 2], F32, kind="ExternalInput").ap()
    ones_d = nc.dram_tensor("ones_c", [128, 128], F32, kind="ExternalInput").ap()

    d6_o = nc.dram_tensor("d6", [8, TOK], F32, kind="ExternalOutput").ap()
    tr_o = nc.dram_tensor("tr3", [8, TOK], F32, kind="ExternalOutput").ap()

    with tile.TileContext(nc) as tc, contextlib.ExitStack() as ctx:
        const_p = ctx.enter_context(tc.tile_pool(name="consts", bufs=1))
        onescol = const_p.tile([128, 1], F32R)
        onesrow = const_p.tile([1, 128], F32R)
        ident = const_p.tile([128, 128], BF16)
        nc.gpsimd.dma_start(onescol[:], ones_d[:, 0:1])
        nc.gpsimd.dma_start(onesrow[:], ones_d[0:1, :])
        ones_bf = const_p.tile([1, 128], BF16)
        nc.vector.memset(ones_bf[:], 1.0)
        masks.make_identity(nc, ident[:])

        xs_p = ctx.enter_context(tc.tile_pool(name="xstate", bufs=1))
        xA = [xs_p.tile([128, TOK], F32R, tag=f"xA{m}", name=f"xA{m}") for m in range(4)]

        stat_p = ctx.enter_context(tc.tile_pool(name="stats", bufs=1))
        dram_p = ctx.enter_context(tc.tile_pool(name="dramb", bufs=1, space="DRAM"))

        pp_mm = ctx.enter_context(tc.tile_pool(name="ppmm", bufs=4, space="PSUM"))
        pp_row = ctx.enter_context(tc.tile_pool(name="pprow", bufs=1, space="PSUM"))
        pp_bc = ctx.enter_context(tc.tile_pool(name="ppbc", bufs=2, space="PSUM"))

        y_dram = [dram_p.tile([CONV_DIMS[i], TOK], F32R, tag=f"ydram{i}", name=f"ydram{i}")
                  for i in range(1, 4)]

        # ------------------------------------------------------------------
        # conv stack (activations spilled to DRAM, BN applied on load)
        # ------------------------------------------------------------------
        b4sb = stat_p.tile([128, 4 * NF], F32, tag="b4")
        nc.sync.dma_start(b4sb[:], bias4[:])

        bn_s = {}
        bn_t = {}

        def conv_layer(li, wcp, cxp, pp_conv):
            kdim, mdim = CONV_DIMS[li - 1], CONV_DIMS[li]
            KC, MC = kdim // 128, mdim // 128
            src = xin if li == 1 else y_dram[li - 2]
            with_bn = li < 4
            if with_bn:
                sum_acc = stat_p.tile([128, MC * 4], F32, tag=f"sum{li}")
                sq_acc = stat_p.tile([128, MC * 4], F32, tag=f"sq{li}")
            cond_sb = None
            if li == 1:
                # channels 0:1024 repeat frame 0's features for all 4 frames;
                # compute their contribution once and add it at eviction
                KC = 8
                xc = cxp.tile([128, 8 * 512], F32R, tag="convc", name="convc",
                              bufs=1)
                nc.sync.dma_start(
                    xc[:].rearrange("p (k c) -> p k c", k=8),
                    src[0:1024, 0:512].rearrange("(k p) c -> p k c", p=128))
                cond_sb = [cxp.tile([128, 512], F32, tag=f"cond{m}",
                                    name=f"cond{m}") for m in range(MC)]
                for m in range(MC):
                    wslc = wcp.tile([128, 8 * 128], F32R, tag="wslc",
                                    name="wslc", bufs=2)
                    wvc = wslc[:].rearrange("p (k c) -> p k c", k=8)
                    nc.sync.dma_start(
                        wvc, convw[0][0:1024, m * 128:(m + 1) * 128]
                        .rearrange("(k p) c -> p k c", p=128))
                    ps = pp_conv.tile([128, 512], F32, tag="mm", name="ccps")
                    for k in range(8):
                        nc.tensor.matmul(
                            ps[:], wvc[:, k, :], xc[:, k * 512:(k + 1) * 512],
                            start=(k == 0), stop=(k == 7))
                    nc.scalar.copy(cond_sb[m][:], ps[:])
            for pt in range(4):
                xt = cxp.tile([128, KC * 512], F32R, tag="convx", name="convx",
                              bufs=2)
                nc.sync.dma_start(
                    xt[:].rearrange("p (k c) -> p k c", k=KC),
                    src[kdim - KC * 128:, pt * 512:(pt + 1) * 512]
                    .rearrange("(k p) c -> p k c", p=128))
                if li > 1:
                    s_p, t_p = bn_s[li - 1], bn_t[li - 1]
                    for k in range(KC):
                        nc.scalar.activation(
                            xt[:, k * 512:(k + 1) * 512],
                            xt[:, k * 512:(k + 1) * 512],
                            AF.Relu, bias=t_p[:, k:k + 1], scale=s_p[:, k:k + 1])
                for m in range(MC):
                    wsl = wcp.tile([128, KC * 128], F32R, tag="wsl", name="wsl",
                                   bufs=4)
                    wv = wsl[:].rearrange("p (k c) -> p k c", k=KC)
                    nc.sync.dma_start(
                        wv, convw[li - 1][kdim - KC * 128:,
                                          m * 128:(m + 1) * 128]
                        .rearrange("(k p) c -> p k c", p=128))
                    ps = pp_conv.tile([128, 512], F32, tag="mm", name="cps")
                    for k in range(KC):
                        nc.tensor.matmul(
                            ps[:], wv[:, k, :], xt[:, k * 512:(k + 1) * 512],
                            start=(k == 0), stop=(k == KC - 1))
                    if with_bn:
                        ot = cxp.tile([128, 512], F32R, tag="convot", name="cot",
                                      bufs=4)
                        if cond_sb is not None:
                            nc.vector.tensor_tensor(ot[:], ps[:],
                                                    cond_sb[m][:], op=ALU.add)
                            stats_src = ot[:]
                        else:
                            stats_src = ps[:]
                            nc.scalar.activation(
                                ot[:], ps[:], AF.Copy,
                                accum_out=sum_acc[:, m * 4 + pt:m * 4 + pt + 1])
                        sqs = cxp.tile([128, 512], BF16, tag="sqscr", name="sqs",
                                       bufs=4)
                        if cond_sb is not None:
                            nc.scalar.activation(
                                sqs[:], stats_src, AF.Copy,
                                accum_out=sum_acc[:, m * 4 + pt:m * 4 + pt + 1])
                        nc.scalar.activation(
                            sqs[:], stats_src, AF.Square,
                            accum_out=sq_acc[:, m * 4 + pt:m * 4 + pt + 1])
                        nc.sync.dma_start(
                            y_dram[li - 1][m * 128:(m + 1) * 128,
                                           pt * 512:(pt + 1) * 512], ot[:])
                    else:
                        nc.scalar.activation(
                            xA[m][:, pt * 512:(pt + 1) * 512], ps[:], AF.Identity,
                            bias=b4sb[:, m * 4 + pt:m * 4 + pt + 1])
            if not with_bn:
                return
            allin = stat_p.tile([128, 2 * MC], F32, tag=f"ain{li}", name="allin")
            nc.vector.tensor_reduce(
                allin[:, 0:MC], sum_acc[:].rearrange("p (m t) -> p m t", m=MC),
                axis=AX.X, op=ALU.add)
            nc.vector.tensor_reduce(
                allin[:, MC:2 * MC], sq_acc[:].rearrange("p (m t) -> p m t", m=MC),
                axis=AX.X, op=ALU.add)
            bin_ = dram_p.tile([128, 2 * MC], F32, tag=f"arin{li}", name="arin")
            bout = dram_p.tile([128, 2 * MC], F32, tag=f"arout{li}", name="arout")
            nc.sync.dma_start(bin_[:], allin[:])
            nc.gpsimd.collective_compute(
                "AllReduce", ALU.add, replica_groups=[list(range(NCORES))],
                ins=[bin_.opt()], outs=[bout.opt()])
            gl = stat_p.tile([128, 2 * MC], F32, tag=f"gl{li}", name="gl")
            nc.sync.dma_start(gl[:], bout[:])
            cst = stat_p.tile([128, 3 * MC], F32, tag=f"cst{li}", name="cst")
            nc.sync.dma_start(cst[:], bnconst[li - 1][:])
            mu = stat_p.tile([128, MC], F32, tag=f"mu{li}", name="bmu")
            var = stat_p.tile([128, MC], F32, tag=f"va{li}", name="bvar")
            s_t = stat_p.tile([128, MC], F32, tag=f"s{li}", name="bs")
            t_t = stat_p.tile([128, MC], F32, tag=f"t{li}", name="bt")
            nc.scalar.mul(mu[:], gl[:, 0:MC], 1.0 / NBN)
            nc.scalar.mul(var[:], gl[:, MC:2 * MC], 1.0 / NBN)
            msq = stat_p.tile([128, MC], F32, tag=f"ms{li}", name="bmsq")
            nc.vector.tensor_mul(msq[:], mu[:], mu[:])
            nc.vector.tensor_tensor(var[:], var[:], msq[:], op=ALU.subtract)
            nc.vector.tensor_scalar(var[:], var[:], BNEPS, None, op0=ALU.add)
            sd = stat_p.tile([128, MC], F32, tag=f"sd{li}", name="bsd")
            nc.scalar.activation(sd[:], var[:], AF.Sqrt)
            rsd = stat_p.tile([128, MC], F32, tag=f"rs{li}", name="brsd")
            nc.vector.reciprocal(rsd[:], sd[:])
            nc.vector.tensor_mul(s_t[:], rsd[:], cst[:, 0:MC])
            nc.vector.tensor_mul(t_t[:], mu[:], s_t[:])
            nc.vector.tensor_tensor(t_t[:], cst[:, MC:2 * MC], t_t[:],
                                    op=ALU.subtract)
            bn_s[li], bn_t[li] = s_t, t_t

        with tc.tile_pool(name="wcp", bufs=1) as wcp, \
             tc.tile_pool(name="cxp", bufs=1) as cxp:
            for li in (1, 2, 3, 4):
                conv_layer(li, wcp, cxp, pp_mm)

        # ------------------------------------------------------------------
        # transformer
        # ------------------------------------------------------------------
        rows_p = ctx.enter_context(tc.tile_pool(name="rows", bufs=1))
        scr = ctx.enter_context(tc.tile_pool(name="scratch", bufs=2))

        def ln_cols(xt, xview, dst_tiles, dst_cols):
            """LN per token over feature dim (stats + apply on DVE/Pool)."""
            ps_s = pp_row.tile([1, 512], F32, tag="row_s", name="ps_s")
            ps_q = pp_row.tile([1, 512], F32, tag="row_q", name="ps_q")
            for k in range(4):
                nc.tensor.matmul(ps_s[:], onescol[:], xview(k),
                                 start=(k == 0), stop=(k == 3))
            for k in range(4):
                sq = scr.tile([128, 512], F32R, tag="lnsq", name="lnsq")
                eng = nc.vector if k % 2 else nc.gpsimd
                eng.tensor_mul(sq[:], xview(k), xview(k))
                nc.tensor.matmul(ps_q[:], onescol[:], sq[:],
                                 start=(k == 0), stop=(k == 3))
            mu = rows_p.tile([1, 512], F32R, tag="mu", name="lmu", bufs=2)
            e2 = rows_p.tile([1, 512], F32, tag="e2", name="le2", bufs=4)
            r = rows_p.tile([1, 512], F32R, tag="r", name="lr", bufs=2)
            nc.scalar.mul(mu[:], ps_s[:], 1.0 / D)
            nc.scalar.mul(e2[:], ps_q[:], 1.0 / D)
            with nc.allow_low_precision(reason="f32r row math"):
                nc.vector.tensor_mul(r[:], mu[:], mu[:])
                nc.vector.scalar_tensor_tensor(
                    e2[:], e2[:], LNEPS, r[:], op0=ALU.add, op1=ALU.subtract)
                # 1/sqrt(v) = exp(-0.5 ln v): stays inside the exp/ln table
                nc.scalar.activation(e2[:], e2[:], AF.Ln)
                nc.scalar.activation(r[:], e2[:], AF.Exp, scale=-0.5)
            psb_mu = pp_bc.tile([128, 512], F32, tag="bc", name="psbmu")
            psb_r = pp_bc.tile([128, 512], F32, tag="bc", name="psbr")
            nc.tensor.matmul(psb_mu[:], onesrow[:], mu[:], start=True, stop=True)
            nc.tensor.matmul(psb_r[:], onesrow[:], r[:], start=True, stop=True)
            for k in range(4):
                tmp = scr.tile([128, 512], F32, tag="lntmp", name="lntmp")
                nc.vector.tensor_tensor(tmp[:], xview(k), psb_mu[:],
                                        op=ALU.subtract)
                nc.vector.tensor_mul(dst_tiles[k][:, dst_cols], tmp[:], psb_r[:])

        def ln1_rows(x_in, nm, rcol_all):
            """Per-frame LN stats; negmu row (K=1 fold operand) + 1/sd cols."""
            for f in range(4):
                sl = slice(f * 512, (f + 1) * 512)
                ps_s = pp_row.tile([1, 512], F32, tag="row_s", name="ps_s")
                ps_q = pp_row.tile([1, 512], F32, tag="row_q", name="ps_q")
                for k in range(4):
                    nc.tensor.matmul(ps_s[:], onescol[:], x_in[k][:, sl],
                                     start=(k == 0), stop=(k == 3))
                for k in range(4):
                    sq = scr.tile([128, 512], F32R, tag="lnsq", name="lnsq")
                    eng = nc.vector if k % 2 else nc.gpsimd
                    eng.tensor_mul(sq[:], x_in[k][:, sl], x_in[k][:, sl])
                    nc.tensor.matmul(ps_q[:], onescol[:], sq[:],
                                     start=(k == 0), stop=(k == 3))
                e2 = rows_p.tile([1, 512], F32, tag="e2", name="le2", bufs=4)
                rr = rows_p.tile([1, 512], F32, tag="rr", name="lrr", bufs=4)
                nc.scalar.mul(e2[:], ps_q[:], 1.0 / D)
                with nc.allow_low_precision(reason="ln1 rows"):
                    nc.scalar.mul(nm[0:1, sl], ps_s[:], -1.0 / D)
                    msq = rows_p.tile([1, 512], F32, tag="rr", name="lms", bufs=4)
                    nc.vector.tensor_mul(msq[:], nm[0:1, sl], nm[0:1, sl])
                    nc.vector.scalar_tensor_tensor(
                        e2[:], e2[:], LNEPS, msq[:], op0=ALU.add,
                        op1=ALU.subtract)
                    nc.scalar.activation(e2[:], e2[:], AF.Ln)
                    nc.scalar.activation(rr[:], e2[:], AF.Exp, scale=-0.5)
                for st in range(4):
                    nc.sync.dma_start(
                        rcol_all[:, f * 4 + st:f * 4 + st + 1],
                        rr[0:1, st * 128:(st + 1) * 128])

        tr_ctx = ctx.enter_context(contextlib.ExitStack())
        wp = tr_ctx.enter_context(tc.tile_pool(name="wp", bufs=1))
        wqp = tr_ctx.enter_context(tc.tile_pool(name="wqp", bufs=1))
        attn_p = tr_ctx.enter_context(tc.tile_pool(name="attn", bufs=2))
        sl_p = tr_ctx.enter_context(tc.tile_pool(name="slices", bufs=1))
        ot_p = tr_ctx.enter_context(tc.tile_pool(name="otp", bufs=1))
        otb_all = ot_p.tile([128, 4 * TOK], BF16, tag="otall", name="otall")

        def st_view(xt, k, st):
            # scattered columns {f*512 + st*128 + p} as (128, (f,p)=512)
            return xt[k][:].rearrange("p (f s) -> p f s", f=4)[:, :, st * 128:(st + 1) * 128]

        def transformer_layer(li, x_in, x_mid):
            wq = [wqp.tile([128, 3 * D], F32R, tag=f"wqkv{k}", name=f"wq{k}")
                  for k in range(4)]
            for k in range(4):
                nc.sync.dma_start(wq[k][:], wqkv_d[li, k * 128:(k + 1) * 128, :])
            ws_t = rows_p.tile([1, 3 * D], BF16, tag="wsum", name="wst", bufs=1)
            nc.sync.dma_start(ws_t[:], wsbq_d[li, 0:1, :])
            bq_t = rows_p.tile([1, 3 * D], BF16, tag="bqr", name="bqt", bufs=1)
            nc.sync.dma_start(bq_t[:], wsbq_d[li, 1:2, :])
            cols = stat_p.tile([128, 12], F32, tag="tcols", name="tcols")
            nc.sync.dma_start(cols[:], tcols_d[li])

            # q bias broadcast over the token partitions (k-bias is
            # softmax-invariant; v-bias is folded into outb on the host)
            bias_bc = attn_p.tile([128, D], BF16, tag="biasbc",
                                  name="bias_bc", bufs=1)
            psb = pp_bc.tile([128, 512], F32, tag="bc", name="psbb")
            nc.tensor.matmul(psb[:], ones_bf[:], bq_t[:, 0:512],
                             start=True, stop=True)
            nc.scalar.copy(bias_bc[:], psb[:])

            nm = attn_p.tile([1, TOK], BF16, tag="nmsd", name="nm", bufs=1)
            rcol = attn_p.tile([128, 16], F32, tag="rcol", name="rcol", bufs=1)
            ln1_rows(x_in, nm, rcol)

            for st in range(4):
                qt = attn_p.tile([128, TOK], BF16, tag="qst", name="qt", bufs=2)
                kt = attn_p.tile([128, TOK], BF16, tag="kst", name="kt", bufs=2)
                # v stored (j, d, h) so the AV multiply hits the 2x DVE mode
                vt = attn_p.tile([128, TOK], BF16, tag="vst", name="vt", bufs=2)
                for f in range(NF):
                    c0 = f * 512 + st * 128
                    for ns in range(3):
                        ps = pp_mm.tile([128, 512], F32, tag="mm", name="qps")
                        for k in range(4):
                            nc.tensor.matmul(
                                ps[:], x_in[k][:, c0:c0 + 128],
                                wq[k][:, ns * 512:(ns + 1) * 512],
                                start=(k == 0), stop=False)
                        nc.tensor.matmul(ps[:], nm[0:1, c0:c0 + 128],
                                         ws_t[:, ns * 512:(ns + 1) * 512],
                                         start=False, stop=True)
                        rc = rcol[:, f * 4 + st:f * 4 + st + 1]
                        if ns < 2:
                            nc.scalar.activation(
                                (qt if ns == 0 else kt)[:, f * 512:(f + 1) * 512],
                                ps[:], AF.Copy, scale=rc)
                        else:
                            nc.scalar.activation(
                                vt[:, f * 512:(f + 1) * 512]
                                .rearrange("p (d h) -> p h d", h=8),
                                ps[:].rearrange("p (h d) -> p h d", h=8),
                                AF.Copy, scale=rc)
                # q bias, broadcast over frames, one 2x-mode op per st
                nc.vector.tensor_tensor(
                    qt[:].rearrange("p (f c) -> p f c", f=4),
                    qt[:].rearrange("p (f c) -> p f c", f=4),
                    bias_bc[:].unsqueeze(1).broadcast_to([128, 4, 512]),
                    op=ALU.add)

                s_sc = attn_p.tile([128, 128], F32, tag="s_sc", name="s_sc",
                                   bufs=2)
                k4 = kt[:].rearrange("p (j hd) -> p j hd", j=4)
                for i in range(4):
                    pbig = attn_p.tile([128, TOK], BF16, tag="pbig",
                                       name="pbig", bufs=2)
                    qi = qt[:, i * 512:(i + 1) * 512].unsqueeze(1) \
                        .broadcast_to([128, 4, 512])
                    nc.vector.tensor_mul(
                        pbig[:].rearrange("p (j hd) -> p j hd", j=4), qi, k4)
                    st1 = attn_p.tile([128, TOK // 2], BF16, tag="qks1",
                                      name="qks1", bufs=2)
                    pv = pbig[:].rearrange("p (g d) -> p g d", g=32)
                    nc.vector.tensor_tensor(
                        st1[:].rearrange("p (g d) -> p g d", g=32),
                        pv[:, :, 0:32], pv[:, :, 32:64], op=ALU.add)
                    nc.vector.tensor_reduce(
                        s_sc[:, i * 32:(i + 1) * 32],
                        st1[:].rearrange("p (g d) -> p g d", g=32),
                        axis=AX.X, op=ALU.add)
                # softmax over j without max-subtraction (logits bounded)
                # S cols = (i, j, h)
                eexp = attn_p.tile([128, 128], BF16, tag="eexp", name="eexp",
                                   bufs=2)
                nc.scalar.activation(eexp[:], s_sc[:], AF.Exp)
                z = attn_p.tile([128, 32], F32, tag="z", name="zt", bufs=2)
                nc.vector.tensor_reduce(
                    z[:].rearrange("p (i h) -> p i h", i=4),
                    eexp[:].rearrange("p (i j h) -> p i h j", i=4, j=4),
                    axis=AX.X, op=ALU.add)
                zr = attn_p.tile([128, 32], F32, tag="zr", name="zr", bufs=2)
                nc.vector.reciprocal(zr[:], z[:])
                a_t = attn_p.tile([128, 128], BF16, tag="a_t", name="a_t",
                                  bufs=2)
                nc.vector.tensor_mul(
                    a_t[:].rearrange("p (i j h) -> p i j h", i=4, j=4),
                    eexp[:].rearrange("p (i j h) -> p i j h", i=4, j=4),
                    zr[:].rearrange("p (i h) -> p i h", i=4).unsqueeze(2)
                    .broadcast_to([128, 4, 4, 8]))
                for i in range(4):
                    tbig = attn_p.tile([128, TOK], BF16, tag="tbig", name="tbig", bufs=2)
                    ablk = a_t[:, i * 32:(i + 1) * 32] \
                        .rearrange("p (j h) -> p j h", j=4) \
                        .unsqueeze(2).broadcast_to([128, 4, 64, 8])
                    nc.vector.tensor_mul(
                        tbig[:].rearrange("p (j d h) -> p j d h", j=4, d=64),
                        vt[:].rearrange("p (j d h) -> p j d h", j=4, d=64),
                        ablk)
                    av01 = attn_p.tile([128, 512], F32, tag="av01", name="av01",
                                       bufs=2)
                    av23 = attn_p.tile([128, 512], F32, tag="av23", name="av23",
                                       bufs=2)
                    av = attn_p.tile([128, 512], BF16, tag="av", name="av",
                                     bufs=2)
                    nc.gpsimd.tensor_tensor(av01[:], tbig[:, 0:512],
                                            tbig[:, 512:1024], op=ALU.add)
                    nc.gpsimd.tensor_tensor(av23[:], tbig[:, 1024:1536],
                                            tbig[:, 1536:2048], op=ALU.add)
                    nc.gpsimd.tensor_tensor(av[:], av01[:], av23[:],
                                            op=ALU.add)
                    pst = pp_bc.tile([128, 512], BF16, tag="bc", name="pst")
                    for c in range(4):
                        nc.tensor.transpose(pst[:, c * 128:(c + 1) * 128],
                                            av[:, c * 128:(c + 1) * 128],
                                            ident[:])
                    nc.scalar.copy(
                        otb_all[:].rearrange("p (c t) -> p c t", c=4)
                        [:, :, i * 512 + st * 128:i * 512 + st * 128 + 128],
                        pst[:].rearrange("p (c t) -> p c t", c=4))

            wo = [wp.tile([128, D], BF16, tag=f"wo{k}", name=f"wo{k}")
                  for k in range(4)]
            for k in range(4):
                nc.sync.dma_start(wo[k][:], wo_d[li, k * 128:(k + 1) * 128, :])
            for m in range(4):
                for ns in range(4):
                    ps = pp_mm.tile([128, 512], F32, tag="mm", name="ops")
                    for k in range(4):
                        nc.tensor.matmul(
                            ps[:], wo[k][:, m * 128:(m + 1) * 128],
                            otb_all[:, k * TOK + ns * 512:k * TOK + (ns + 1) * 512],
                            start=(k == 0), stop=(k == 3))
                    nc.vector.scalar_tensor_tensor(
                        x_mid[m][:, ns * 512:(ns + 1) * 512], ps[:],
                        cols[:, 4 + m:5 + m], x_in[m][:, ns * 512:(ns + 1) * 512],
                        op0=ALU.add, op1=ALU.add)

            w1 = [wp.tile([128, D], F32R, tag=f"w1_{k}", name=f"w1_{k}")
                  for k in range(4)]
            w2 = [wp.tile([128, D], F32R, tag=f"w2_{k}", name=f"w2_{k}")
                  for k in range(4)]
            for k in range(4):
                nc.sync.dma_start(w1[k][:], w1_d[li, k * 128:(k + 1) * 128, :])
                nc.sync.dma_start(w2[k][:], w2_d[li, k * 128:(k + 1) * 128, :])
            for ns in range(4):
                xh2 = [sl_p.tile([128, 512], F32R, tag=f"xh2_{k}", name=f"xh2_{k}")
                       for k in range(4)]
                ln_cols(x_mid,
                        lambda k: x_mid[k][:, ns * 512:(ns + 1) * 512],
                        xh2, slice(0, 512))
                hsl = [sl_p.tile([128, 512], F32R, tag=f"h_{m}", name=f"hsl{m}")
                       for m in range(4)]
                for m in range(4):
                    ps = pp_mm.tile([128, 512], F32, tag="mm", name="m1ps")
                    for k in range(4):
                        nc.tensor.matmul(
                            ps[:], w1[k][:, m * 128:(m + 1) * 128], xh2[k][:],
                            start=(k == 0), stop=(k == 3))
                    nc.scalar.activation(hsl[m][:], ps[:], AF.Gelu_apprx_tanh,
                                         bias=cols[:, m:m + 1])
                for m in range(4):
                    ps = pp_mm.tile([128, 512], F32, tag="mm", name="m2ps")
                    for k in range(4):
                        nc.tensor.matmul(
                            ps[:], w2[k][:, m * 128:(m + 1) * 128], hsl[k][:],
                            start=(k == 0), stop=(k == 3))
                    nc.vector.scalar_tensor_tensor(
                        x_mid[m][:, ns * 512:(ns + 1) * 512], ps[:],
                        cols[:, 8 + m:9 + m], x_mid[m][:, ns * 512:(ns + 1) * 512],
                        op0=ALU.add, op1=ALU.add)

        cur = xA
        _nl = int(os.environ.get("KNLAYERS", NLAYER))
        for li in range(_nl):
            transformer_layer(li, cur, cur)

        tr_ctx.close()

        # ------------------------------------------------------------------
        # heads
        # ------------------------------------------------------------------
        _skip_heads = os.environ.get("KHEADS", "1") == "0"
        if _skip_heads:
            nc.gpsimd.dma_start(d6_o[:], cur[0][0:8, :])
            nc.gpsimd.dma_start(tr_o[:], cur[1][0:8, :])
        with tc.tile_pool(name="heads", bufs=1) as hp, \
             tc.tile_pool(name="whp", bufs=1) as whp:
          if not _skip_heads:
              hc = stat_p.tile([128, 10], F32, tag="hcols", name="hc")
              nc.sync.dma_start(hc[:], hcols_d[:])
              sb3 = stat_p.tile([8, 2], F32, tag="sb3", name="sb3")
              nc.sync.dma_start(sb3[:], sb3_d[:])

              xhf = [hp.tile([128, TOK], F32R, tag=f"xhf{k}", name=f"xhf{k}")
                     for k in range(4)]
              for st in range(4):
                  ln_cols(cur, lambda k: cur[k][:, st * 512:(st + 1) * 512],
                          xhf, slice(st * 512, (st + 1) * 512))

              xp = [hp.tile([128, TOK], F32R, tag=f"xp{k}", name=f"xp{k}")
                    for k in range(4)]

              def mm_head(src_tiles, wt_dram, kdim, mdim, dst_tiles, evict):
                  KC = kdim // 128
                  MC = max(mdim // 128, 1)
                  wsb = [whp.tile([128, mdim], F32R, tag=f"wh_{kdim}_{mdim}_{k}",
                                  name=f"wh{k}") for k in range(KC)]
                  for k in range(KC):
                      nc.sync.dma_start(wsb[k][:], wt_dram[k * 128:(k + 1) * 128, :])
                  for m in range(MC):
                      for ns in range(4):
                          ps = pp_mm.tile([128, 512], F32, tag="mm", name="hps")
                          for k in range(KC):
                              nc.tensor.matmul(
                                  ps[:], wsb[k][:, m * 128:(m + 1) * 128],
                                  src_tiles[k][:, ns * 512:(ns + 1) * 512],
                                  start=(k == 0), stop=(k == KC - 1))
                          evict(ps, dst_tiles[m], m, ns)

              mm_head(xhf, projw_d, D, D, xp,
                      lambda ps, dst, m, ns: nc.scalar.activation(
                          dst[:, ns * 512:(ns + 1) * 512], ps[:], AF.Identity,
                          bias=hc[:, m:m + 1]))

              def branch(w1d, w2d, w3d, b1ofs, b2ofs, out_dram, b3col, r1, r2, pfx):
                  mm_head(xp, w1d, D, 256, r1,
                          lambda ps, dst, m, ns: nc.scalar.activation(
                              dst[:, ns * 512:(ns + 1) * 512], ps[:], AF.Relu,
                              bias=hc[:, b1ofs + m:b1ofs + m + 1]))
                  mm_head(r1, w2d, 256, 128, r2,
                          lambda ps, dst, m, ns: nc.scalar.activation(
                              dst[:, ns * 512:(ns + 1) * 512], ps[:], AF.Relu,
                              bias=hc[:, b2ofs:b2ofs + 1]))
                  w3 = whp.tile([128, 8], F32R, tag=f"w3{pfx}", name="w3")
                  nc.sync.dma_start(w3[:], w3d[:])
                  out_sb = hp.tile([8, TOK], F32, tag=f"{pfx}out", name=f"{pfx}out")
                  for ns in range(4):
                      ps = pp_mm.tile([8, 512], F32, tag="mm", name="bps")
                      nc.tensor.matmul(ps[:], w3[:], r2[0][:, ns * 512:(ns + 1) * 512],
                                       start=True, stop=True)
                      nc.scalar.activation(out_sb[:, ns * 512:(ns + 1) * 512], ps[:],
                                           AF.Identity, bias=b3col)
                  nc.sync.dma_start(out_dram[:], out_sb[:])

              # reuse dead transformer buffers for intermediates
              branch(rw1_d, rw2_d, rw3_d, 4, 8, d6_o, sb3[:, 0:1],
                     [cur[0], cur[1]], [xhf[0]], "r")
              branch(tw1_d, tw2_d, tw3_d, 6, 9, tr_o, sb3[:, 1:2],
                     [cur[2], cur[3]], [xhf[1]], "t")

    nc.compile()
    return nc


# ----------------------------------------------------------------------------
# host side
# ----------------------------------------------------------------------------

_CACHE = {}


def _normalize_np(v, eps=1e-12):
    return v / np.maximum(np.linalg.norm(v, axis=-1, keepdims=True), eps)


def _rot6d_np(d6):
    a1, a2 = d6[..., :3], d6[..., 3:]
    b1 = _normalize_np(a1)
    b2 = _normalize_np(a2 - np.sum(b1 * a2, -1, keepdims=True) * b1)
    b3 = np.cross(b1, b2)
    return np.stack([b1, b2, b3], axis=-2)


def _prep_weights(inp):
    f32 = np.float32
    wmap = {}
    for i, cw in enumerate(['c1w', 'c2w', 'c3w', 'c4w']):
        wmap[f'convw{i}'] = np.ascontiguousarray(inp[cw].T.astype(f32))
    for i, (g, b2) in enumerate([('bn1g', 'bn1b'), ('bn2g', 'bn2b'),
                                 ('bn3g', 'bn3b')]):
        M = CONV_DIMS[i + 1] // 128
        bn = np.concatenate([
            inp[g].reshape(M, 128).T, inp[b2].reshape(M, 128).T,
            inp[f'c{i + 1}b'].reshape(M, 128).T], axis=1)
        wmap[f'bnconst{i}'] = np.ascontiguousarray(bn.astype(f32))
    pe = _pe_table()[:NF]
    b4 = inp['c4b'][None, :].astype(f32) + pe                   # (4, 512)
    # cols: m*4 + pt ; frame index == pt
    wmap['bias4'] = np.ascontiguousarray(
        b4.reshape(NF, 4, 128).transpose(2, 1, 0).reshape(128, 16).astype(f32))

    qkvw = np.array(inp['qkvw'], f32)
    qkvb = np.array(inp['qkvb'], f32)
    qkvw[:, :, :512] /= math.sqrt(DH)
    qkvb[:, :512] /= math.sqrt(DH)
    g1 = np.array(inp['ln1g'], f32)
    b1 = np.array(inp['ln1b'], f32)
    wq_fold = g1[:, :, None] * qkvw
    bq_fold = qkvb + np.einsum('ld,ldf->lf', b1, qkvw)
    wmap['wqkv'] = np.ascontiguousarray(wq_fold.astype(f32))
    wsbq = np.concatenate([wq_fold.sum(axis=1, keepdims=True),
                           bq_fold[:, None, :]], axis=1)         # (L, 2, 3D)
    wmap['wsbq'] = np.ascontiguousarray(wsbq.astype(ml_dtypes.bfloat16))
    # attention output features are (d, h)-ordered; permute wo rows to match
    wo_ = np.array(inp['outw'], f32)                             # (L, 512, 512)
    d_idx, h_idx = np.meshgrid(np.arange(DH), np.arange(HEADS), indexing='ij')
    perm = (h_idx * DH + d_idx).reshape(512)     # perm[d*8+h] = h*64+d
    wmap['wo'] = np.ascontiguousarray(wo_[:, perm, :]
                                      .astype(ml_dtypes.bfloat16))
    g2 = np.array(inp['ln2g'], f32)
    bl2 = np.array(inp['ln2b'], f32)
    m1w = np.array(inp['m1w'], f32)
    w1_fold = g2[:, :, None] * m1w
    b1_fold = np.array(inp['m1b'], f32) + np.einsum('ld,ldf->lf', bl2, m1w)
    wmap['w1'] = np.ascontiguousarray(w1_fold.astype(f32))
    wmap['w2'] = np.ascontiguousarray(np.array(inp['m2w'], f32))
    cols = np.zeros((NLAYER, 128, 12), f32)
    cols[:, :, 0:4] = b1_fold.reshape(NLAYER, 4, 128).transpose(0, 2, 1)
    # v-bias is dropped at the v eviction; fold bv @ Wo into outb instead
    bv = bq_fold[:, 2 * 512:3 * 512]                             # (L, 512)
    outb_fold = np.array(inp['outb'], f32) + np.einsum('lk,lko->lo', bv, wo_)
    cols[:, :, 4:8] = outb_fold.reshape(NLAYER, 4, 128).transpose(0, 2, 1)
    cols[:, :, 8:12] = np.array(inp['m2b'], f32).reshape(NLAYER, 4, 128) \
        .transpose(0, 2, 1)
    wmap['tcols'] = cols

    gf_ = np.array(inp['lnfg'], f32)
    bf_ = np.array(inp['lnfb'], f32)
    projw = np.array(inp['projw'], f32)
    wmap['projw'] = np.ascontiguousarray(gf_[:, None] * projw)
    projb_fold = np.array(inp['projb'], f32) + bf_ @ projw
    wmap['rw1'] = np.ascontiguousarray(np.array(inp['rw1'], f32))
    wmap['rw2'] = np.ascontiguousarray(np.array(inp['rw2'], f32))
    rw3 = np.zeros((128, 8), f32)
    rw3[:, :6] = np.array(inp['rw3'], f32)
    wmap['rw3'] = rw3
    wmap['tw1'] = np.ascontiguousarray(np.array(inp['tw1'], f32))
    wmap['tw2'] = np.ascontiguousarray(np.array(inp['tw2'], f32))
    tw3 = np.zeros((128, 8), f32)
    tw3[:, :3] = np.array(inp['tw3'], f32)
    wmap['tw3'] = tw3
    hcols = np.zeros((128, 10), f32)
    hcols[:, 0:4] = projb_fold.reshape(4, 128).T
    hcols[:, 4:6] = np.array(inp['rb1'], f32).reshape(2, 128).T
    hcols[:, 6:8] = np.array(inp['tb1'], f32).reshape(2, 128).T
    hcols[:, 8] = np.array(inp['rb2'], f32)
    hcols[:, 9] = np.array(inp['tb2'], f32)
    wmap['hcols'] = hcols
    sb3 = np.zeros((8, 2), f32)
    sb3[0:6, 0] = np.array(inp['rb3'], f32)
    sb3[0:3, 1] = np.array(inp['tb3'], f32)
    wmap['sb3'] = sb3
    wmap['ones_c'] = np.ones((128, 128), f32)
    return wmap


def kernel(**inputs):
    inp = {k: np.asarray(v) for k, v in inputs.items()}

    idx = inp['seed_idxs'].reshape(B, -1).astype(np.int64)      # (B, N)
    sel_seed = np.take_along_axis(np.asarray(inp['fp2_features'], np.float32),
                                  idx[:, None, :], axis=2)
    sel_grasp = np.take_along_axis(np.asarray(inp['local_grasp_features'], np.float32),
                                   idx[:, None, :], axis=2)
    sel_color = np.take_along_axis(np.asarray(inp['local_color_features'], np.float32),
                                   idx[:, None, :], axis=2)
    sel_pose = np.take_along_axis(np.asarray(inp['grasp_pose_feature'], np.float32),
                                  idx[:, None, :], axis=2)
    gsf = np.asarray(inp['sa4_features'], np.float32).max(axis=-1)
    gsf = np.broadcast_to(gsf[:, :, None], (B, 256, NPTS))
    fused = sel_pose + np.concatenate([sel_grasp, sel_color, sel_seed, gsf], axis=1)
    gf = fused.reshape(BE, FRAME, 1024, NPTS)
    cond = np.broadcast_to(gf[:, :1], (BE, NF, 1024, NPTS))
    X = np.concatenate([cond, gf[:, 1:]], axis=2)               # (e, f, 2048, N)

    if 'nc' not in _CACHE:
        _CACHE['nc'] = build_kernel()
    nc = _CACHE['nc']
    wmap = _prep_weights(inp)

    in_maps = []
    for k in range(NCORES):
        xc = X[:, :, :, k * NPC:(k + 1) * NPC]                  # (e, f, c, n)
        xc = xc.transpose(2, 1, 0, 3).reshape(2048, TOK)        # (c, (f,e,n))
        m = dict(wmap)
        m['xin'] = np.ascontiguousarray(xc, dtype=np.float32)
        in_maps.append(m)

    res = run_bass_kernel_spmd(nc, in_maps, core_ids=list(range(NCORES)))

    out = np.zeros((BE * NPTS, NF, 12), np.float32)
    for k in range(NCORES):
        d6 = res.results[k]['d6'][:6]
        tr = res.results[k]['tr3'][:3]
        d6 = d6.reshape(6, NF, BE, NPC).transpose(2, 3, 1, 0)   # (e, n, f, 6)
        tr = tr.reshape(3, NF, BE, NPC).transpose(2, 3, 1, 0)
        rot = _rot6d_np(d6).reshape(BE, NPC, NF, 9)
        for e in range(BE):
            rows = slice(e * NPTS + k * NPC, e * NPTS + (k + 1) * NPC)
            out[rows, :, 0:3] = tr[e]
            out[rows, :, 3:12] = rot[e]
    return out


if __name__ == "__main__":
    build_kernel()
    print("built ok")



# revision 64
# speedup vs baseline: 1.0096x; 1.0096x over previous
import math
import os
import numpy as np
import ml_dtypes
import contextlib

import concourse.bass as bass
import concourse.tile as tile
from concourse import bacc, mybir, masks
from concourse.bass_utils import run_bass_kernel_spmd

F32 = mybir.dt.float32
F32R = mybir.dt.float32r
BF16 = mybir.dt.bfloat16
ALU = mybir.AluOpType
AF = mybir.ActivationFunctionType
AX = mybir.AxisListType

NCORES = 8
FRAME = 5
NF = FRAME - 1
D = 512
DH = 64
HEADS = 8
B = 20
NPTS = 1024
BE = B // FRAME
NPC = NPTS // NCORES     # 128 points per core
TOK = NF * BE * NPC      # 2048 tokens per core
NLAYER = 12
LNEPS = 1e-5
BNEPS = 1e-5
NBN = 16 * NPTS

CONV_DIMS = [2048, 1536, 1024, 768, 512]


def _pe_table(max_len=16, d=D):
    pos = np.arange(max_len, dtype=np.float32)[:, None]
    div = np.exp(np.arange(0, d, 2, dtype=np.float32) * (-math.log(10000.0) / d))
    pe = np.zeros((max_len, d), np.float32)
    pe[:, 0::2] = np.sin(pos * div)
    pe[:, 1::2] = np.cos(pos * div)
    return pe


def build_kernel():
    nc = bacc.Bacc("TRN2", target_bir_lowering=False, debug=False,
                   num_devices=NCORES)

    xin = nc.dram_tensor("xin", [CONV_DIMS[0], TOK], F32R, kind="ExternalInput").ap()
    convw = [nc.dram_tensor(f"convw{i}", [CONV_DIMS[i], CONV_DIMS[i + 1]], F32R,
                            kind="ExternalInput").ap() for i in range(4)]
    bnconst = [nc.dram_tensor(f"bnconst{i}", [128, 3 * (CONV_DIMS[i + 1] // 128)],
                              F32, kind="ExternalInput").ap() for i in range(3)]
    bias4 = nc.dram_tensor("bias4", [128, 4 * NF], F32, kind="ExternalInput").ap()

    wqkv_d = nc.dram_tensor("wqkv", [NLAYER, D, 3 * D], F32R, kind="ExternalInput").ap()
    wsbq_d = nc.dram_tensor("wsbq", [NLAYER, 2, 3 * D], BF16, kind="ExternalInput").ap()
    wo_d = nc.dram_tensor("wo", [NLAYER, D, D], BF16, kind="ExternalInput").ap()
    w1_d = nc.dram_tensor("w1", [NLAYER, D, D], F32R, kind="ExternalInput").ap()
    w2_d = nc.dram_tensor("w2", [NLAYER, D, D], F32R, kind="ExternalInput").ap()
    tcols_d = nc.dram_tensor("tcols", [NLAYER, 128, 12], F32, kind="ExternalInput").ap()

    projw_d = nc.dram_tensor("projw", [D, D], F32R, kind="ExternalInput").ap()
    rw1_d = nc.dram_tensor("rw1", [D, 256], F32R, kind="ExternalInput").ap()
    rw2_d = nc.dram_tensor("rw2", [256, 128], F32R, kind="ExternalInput").ap()
    rw3_d = nc.dram_tensor("rw3", [128, 8], F32R, kind="ExternalInput").ap()
    tw1_d = nc.dram_tensor("tw1", [D, 256], F32R, kind="ExternalInput").ap()
    tw2_d = nc.dram_tensor("tw2", [256, 128], F32R, kind="ExternalInput").ap()
    tw3_d = nc.dram_tensor("tw3", [128, 8], F32R, kind="ExternalInput").ap()
    hcols_d = nc.dram_tensor("hcols", [128, 10], F32, kind="ExternalInput").ap()
    sb3_d = nc.dram_tensor("sb3", [8, 2], F32, kind="ExternalInput").ap()
    ones_d = nc.dram_tensor("ones_c", [128, 128], F32, kind="ExternalInput").ap()

    d6_o = nc.dram_tensor("d6", [8, TOK], F32, kind="ExternalOutput").ap()
    tr_o = nc.dram_tensor("tr3", [8, TOK], F32, kind="ExternalOutput").ap()

    with tile.TileContext(nc) as tc, contextlib.ExitStack() as ctx:
        const_p = ctx.enter_context(tc.tile_pool(name="consts", bufs=1))
        onescol = const_p.tile([128, 1], F32R)
        onesrow = const_p.tile([1, 128], F32R)
        ident = const_p.tile([128, 128], BF16)
        nc.gpsimd.dma_start(onescol[:], ones_d[:, 0:1])
        nc.gpsimd.dma_start(onesrow[:], ones_d[0:1, :])
        ones_bf = const_p.tile([1, 128], BF16)
        nc.vector.memset(ones_bf[:], 1.0)
        masks.make_identity(nc, ident[:])

        xs_p = ctx.enter_context(tc.tile_pool(name="xstate", bufs=1))
        xA = [xs_p.tile([128, TOK], F32R, tag=f"xA{m}", name=f"xA{m}") for m in range(4)]

        stat_p = ctx.enter_context(tc.tile_pool(name="stats", bufs=1))
        dram_p = ctx.enter_context(tc.tile_pool(name="dramb", bufs=1, space="DRAM"))

        pp_mm = ctx.enter_context(tc.tile_pool(name="ppmm", bufs=4, space="PSUM"))
        pp_row = ctx.enter_context(tc.tile_pool(name="pprow", bufs=1, space="PSUM"))
        pp_bc = ctx.enter_context(tc.tile_pool(name="ppbc", bufs=2, space="PSUM"))

        y_dram = [dram_p.tile([CONV_DIMS[i], TOK], F32R, tag=f"ydram{i}", name=f"ydram{i}")
                  for i in range(1, 4)]

        # ------------------------------------------------------------------
        # conv stack (activations spilled to DRAM, BN applied on load)
        # ------------------------------------------------------------------
        b4sb = stat_p.tile([128, 4 * NF], F32, tag="b4")
        nc.sync.dma_start(b4sb[:], bias4[:])

        bn_s = {}
        bn_t = {}

        def conv_layer(li, wcp, cxp, pp_conv):
            kdim, mdim = CONV_DIMS[li - 1], CONV_DIMS[li]
            KC, MC = kdim // 128, mdim // 128
            src = xin if li == 1 else y_dram[li - 2]
            with_bn = li < 4
            if with_bn:
                sum_acc = stat_p.tile([128, MC * 4], F32, tag=f"sum{li}")
                sq_acc = stat_p.tile([128, MC * 4], F32, tag=f"sq{li}")
            cond_sb = None
            if li == 1:
                # channels 0:1024 repeat frame 0's features for all 4 frames;
                # compute their contribution once and add it at eviction
                KC = 8
                xc = cxp.tile([128, 8 * 512], F32R, tag="convc", name="convc",
                              bufs=1)
                nc.sync.dma_start(
                    xc[:].rearrange("p (k c) -> p k c", k=8),
                    src[0:1024, 0:512].rearrange("(k p) c -> p k c", p=128))
                cond_sb = [cxp.tile([128, 512], F32, tag=f"cond{m}",
                                    name=f"cond{m}") for m in range(MC)]
                for m in range(MC):
                    wslc = wcp.tile([128, 8 * 128], F32R, tag="wslc",
                                    name="wslc", bufs=2)
                    wvc = wslc[:].rearrange("p (k c) -> p k c", k=8)
                    nc.sync.dma_start(
                        wvc, convw[0][0:1024, m * 128:(m + 1) * 128]
                        .rearrange("(k p) c -> p k c", p=128))
                    ps = pp_conv.tile([128, 512], F32, tag="mm", name="ccps")
                    for k in range(8):
                        nc.tensor.matmul(
                            ps[:], wvc[:, k, :], xc[:, k * 512:(k + 1) * 512],
                            start=(k == 0), stop=(k == 7))
                    nc.scalar.copy(cond_sb[m][:], ps[:])
            for pt in range(4):
                xt = cxp.tile([128, KC * 512], F32R, tag="convx", name="convx",
                              bufs=3)
                nc.sync.dma_start(
                    xt[:].rearrange("p (k c) -> p k c", k=KC),
                    src[kdim - KC * 128:, pt * 512:(pt + 1) * 512]
                    .rearrange("(k p) c -> p k c", p=128))
                if li > 1:
                    s_p, t_p = bn_s[li - 1], bn_t[li - 1]
                    for k in range(KC):
                        nc.scalar.activation(
                            xt[:, k * 512:(k + 1) * 512],
                            xt[:, k * 512:(k + 1) * 512],
                            AF.Relu, bias=t_p[:, k:k + 1], scale=s_p[:, k:k + 1])
                for m in range(MC):
                    wsl = wcp.tile([128, KC * 128], F32R, tag="wsl", name="wsl",
                                   bufs=4)
                    wv = wsl[:].rearrange("p (k c) -> p k c", k=KC)
                    nc.sync.dma_start(
                        wv, convw[li - 1][kdim - KC * 128:,
                                          m * 128:(m + 1) * 128]
                        .rearrange("(k p) c -> p k c", p=128))
                    ps = pp_conv.tile([128, 512], F32, tag="mm", name="cps")
                    for k in range(KC):
                        nc.tensor.matmul(
                            ps[:], wv[:, k, :], xt[:, k * 512:(k + 1) * 512],
                            start=(k == 0), stop=(k == KC - 1))
                    if with_bn:
                        ot = cxp.tile([128, 512], F32R, tag="convot", name="cot",
                                      bufs=4)
                        if cond_sb is not None:
                            nc.vector.tensor_tensor(ot[:], ps[:],
                                                    cond_sb[m][:], op=ALU.add)
                            stats_src = ot[:]
                        else:
                            stats_src = ps[:]
                            nc.scalar.activation(
                                ot[:], ps[:], AF.Copy,
                                accum_out=sum_acc[:, m * 4 + pt:m * 4 + pt + 1])
                        sqs = cxp.tile([128, 512], BF16, tag="sqscr", name="sqs",
                                       bufs=4)
                        if cond_sb is not None:
                            nc.scalar.activation(
                                sqs[:], stats_src, AF.Copy,
                                accum_out=sum_acc[:, m * 4 + pt:m * 4 + pt + 1])
                        nc.scalar.activation(
                            sqs[:], stats_src, AF.Square,
                            accum_out=sq_acc[:, m * 4 + pt:m * 4 + pt + 1])
                        nc.sync.dma_start(
                            y_dram[li - 1][m * 128:(m + 1) * 128,
                                           pt * 512:(pt + 1) * 512], ot[:])
                    else:
                        nc.scalar.activation(
                            xA[m][:, pt * 512:(pt + 1) * 512], ps[:], AF.Identity,
                            bias=b4sb[:, m * 4 + pt:m * 4 + pt + 1])
            if not with_bn:
                return
            allin = stat_p.tile([128, 2 * MC], F32, tag=f"ain{li}", name="allin")
            nc.vector.tensor_reduce(
                allin[:, 0:MC], sum_acc[:].rearrange("p (m t) -> p m t", m=MC),
                axis=AX.X, op=ALU.add)
            nc.vector.tensor_reduce(
                allin[:, MC:2 * MC], sq_acc[:].rearrange("p (m t) -> p m t", m=MC),
                axis=AX.X, op=ALU.add)
            bin_ = dram_p.tile([128, 2 * MC], F32, tag=f"arin{li}", name="arin")
            bout = dram_p.tile([128, 2 * MC], F32, tag=f"arout{li}", name="arout")
            nc.sync.dma_start(bin_[:], allin[:])
            nc.gpsimd.collective_compute(
                "AllReduce", ALU.add, replica_groups=[list(range(NCORES))],
                ins=[bin_.opt()], outs=[bout.opt()])
            gl = stat_p.tile([128, 2 * MC], F32, tag=f"gl{li}", name="gl")
            nc.sync.dma_start(gl[:], bout[:])
            cst = stat_p.tile([128, 3 * MC], F32, tag=f"cst{li}", name="cst")
            nc.sync.dma_start(cst[:], bnconst[li - 1][:])
            mu = stat_p.tile([128, MC], F32, tag=f"mu{li}", name="bmu")
            var = stat_p.tile([128, MC], F32, tag=f"va{li}", name="bvar")
            s_t = stat_p.tile([128, MC], F32, tag=f"s{li}", name="bs")
            t_t = stat_p.tile([128, MC], F32, tag=f"t{li}", name="bt")
            nc.scalar.mul(mu[:], gl[:, 0:MC], 1.0 / NBN)
            nc.scalar.mul(var[:], gl[:, MC:2 * MC], 1.0 / NBN)
            msq = stat_p.tile([128, MC], F32, tag=f"ms{li}", name="bmsq")
            nc.vector.tensor_mul(msq[:], mu[:], mu[:])
            nc.vector.tensor_tensor(var[:], var[:], msq[:], op=ALU.subtract)
            nc.vector.tensor_scalar(var[:], var[:], BNEPS, None, op0=ALU.add)
            sd = stat_p.tile([128, MC], F32, tag=f"sd{li}", name="bsd")
            nc.scalar.activation(sd[:], var[:], AF.Sqrt)
            rsd = stat_p.tile([128, MC], F32, tag=f"rs{li}", name="brsd")
            nc.vector.reciprocal(rsd[:], sd[:])
            nc.vector.tensor_mul(s_t[:], rsd[:], cst[:, 0:MC])
            nc.vector.tensor_mul(t_t[:], mu[:], s_t[:])
            nc.vector.tensor_tensor(t_t[:], cst[:, MC:2 * MC], t_t[:],
                                    op=ALU.subtract)
            bn_s[li], bn_t[li] = s_t, t_t

        with tc.tile_pool(name="wcp", bufs=1) as wcp, \
             tc.tile_pool(name="cxp", bufs=1) as cxp:
            for li in (1, 2, 3, 4):
                conv_layer(li, wcp, cxp, pp_mm)

        # ------------------------------------------------------------------
        # transformer
        # ------------------------------------------------------------------
        rows_p = ctx.enter_context(tc.tile_pool(name="rows", bufs=1))
        scr = ctx.enter_context(tc.tile_pool(name="scratch", bufs=2))

        def ln_cols(xt, xview, dst_tiles, dst_cols):
            """LN per token over feature dim (stats + apply on DVE/Pool)."""
            ps_s = pp_row.tile([1, 512], F32, tag="row_s", name="ps_s")
            ps_q = pp_row.tile([1, 512], F32, tag="row_q", name="ps_q")
            for k in range(4):
                nc.tensor.matmul(ps_s[:], onescol[:], xview(k),
                                 start=(k == 0), stop=(k == 3))
            for k in range(4):
                sq = scr.tile([128, 512], F32R, tag="lnsq", name="lnsq")
                eng = nc.vector if k % 2 else nc.gpsimd
                eng.tensor_mul(sq[:], xview(k), xview(k))
                nc.tensor.matmul(ps_q[:], onescol[:], sq[:],
                                 start=(k == 0), stop=(k == 3))
            mu = rows_p.tile([1, 512], F32R, tag="mu", name="lmu", bufs=2)
            e2 = rows_p.tile([1, 512], F32, tag="e2", name="le2", bufs=2)
            r = rows_p.tile([1, 512], F32R, tag="r", name="lr", bufs=2)
            nc.scalar.mul(mu[:], ps_s[:], 1.0 / D)
            nc.scalar.mul(e2[:], ps_q[:], 1.0 / D)
            with nc.allow_low_precision(reason="f32r row math"):
                nc.vector.tensor_mul(r[:], mu[:], mu[:])
                nc.vector.scalar_tensor_tensor(
                    e2[:], e2[:], LNEPS, r[:], op0=ALU.add, op1=ALU.subtract)
                # 1/sqrt(v) = exp(-0.5 ln v): stays inside the exp/ln table
                nc.scalar.activation(e2[:], e2[:], AF.Ln)
                nc.scalar.activation(r[:], e2[:], AF.Exp, scale=-0.5)
            psb_mu = pp_bc.tile([128, 512], F32, tag="bc", name="psbmu")
            psb_r = pp_bc.tile([128, 512], F32, tag="bc", name="psbr")
            nc.tensor.matmul(psb_mu[:], onesrow[:], mu[:], start=True, stop=True)
            nc.tensor.matmul(psb_r[:], onesrow[:], r[:], start=True, stop=True)
            for k in range(4):
                tmp = scr.tile([128, 512], F32, tag="lntmp", name="lntmp")
                nc.vector.tensor_tensor(tmp[:], xview(k), psb_mu[:],
                                        op=ALU.subtract)
                nc.vector.tensor_mul(dst_tiles[k][:, dst_cols], tmp[:], psb_r[:])

        def ln1_rows(x_in, nm, rcol_all):
            """Per-frame LN stats; negmu row (K=1 fold operand) + 1/sd cols."""
            for f in range(4):
                sl = slice(f * 512, (f + 1) * 512)
                ps_s = pp_row.tile([1, 512], F32, tag="row_s", name="ps_s")
                ps_q = pp_row.tile([1, 512], F32, tag="row_q", name="ps_q")
                for k in range(4):
                    nc.tensor.matmul(ps_s[:], onescol[:], x_in[k][:, sl],
                                     start=(k == 0), stop=(k == 3))
                for k in range(4):
                    sq = scr.tile([128, 512], F32R, tag="lnsq", name="lnsq")
                    eng = nc.vector if k % 2 else nc.gpsimd
                    eng.tensor_mul(sq[:], x_in[k][:, sl], x_in[k][:, sl])
                    nc.tensor.matmul(ps_q[:], onescol[:], sq[:],
                                     start=(k == 0), stop=(k == 3))
                e2 = rows_p.tile([1, 512], F32, tag="e2", name="le2", bufs=2)
                rr = rows_p.tile([1, 512], F32, tag="rr", name="lrr", bufs=4)
                nc.scalar.mul(e2[:], ps_q[:], 1.0 / D)
                with nc.allow_low_precision(reason="ln1 rows"):
                    nc.scalar.mul(nm[0:1, sl], ps_s[:], -1.0 / D)
                    msq = rows_p.tile([1, 512], F32, tag="rr", name="lms", bufs=4)
                    nc.vector.tensor_mul(msq[:], nm[0:1, sl], nm[0:1, sl])
                    nc.vector.scalar_tensor_tensor(
                        e2[:], e2[:], LNEPS, msq[:], op0=ALU.add,
                        op1=ALU.subtract)
                    nc.scalar.activation(e2[:], e2[:], AF.Ln)
                    nc.scalar.activation(rr[:], e2[:], AF.Exp, scale=-0.5)
                for st in range(4):
                    nc.sync.dma_start(
                        rcol_all[:, f * 4 + st:f * 4 + st + 1],
                        rr[0:1, st * 128:(st + 1) * 128])

        tr_ctx = ctx.enter_context(contextlib.ExitStack())
        wp = tr_ctx.enter_context(tc.tile_pool(name="wp", bufs=1))
        wqp = tr_ctx.enter_context(tc.tile_pool(name="wqp", bufs=1))
        attn_p = tr_ctx.enter_context(tc.tile_pool(name="attn", bufs=2))
        sl_p = tr_ctx.enter_context(tc.tile_pool(name="slices", bufs=1))
        ot_p = tr_ctx.enter_context(tc.tile_pool(name="otp", bufs=1))
        otb_all = ot_p.tile([128, 4 * TOK], BF16, tag="otall", name="otall")

        def st_view(xt, k, st):
            # scattered columns {f*512 + st*128 + p} as (128, (f,p)=512)
            return xt[k][:].rearrange("p (f s) -> p f s", f=4)[:, :, st * 128:(st + 1) * 128]

        def transformer_layer(li, x_in, x_mid):
            wq = [wqp.tile([128, 3 * D], F32R, tag=f"wqkv{k}", name=f"wq{k}")
                  for k in range(4)]
            for k in range(4):
                nc.sync.dma_start(wq[k][:], wqkv_d[li, k * 128:(k + 1) * 128, :])
            ws_t = rows_p.tile([1, 3 * D], BF16, tag="wsum", name="wst", bufs=1)
            nc.sync.dma_start(ws_t[:], wsbq_d[li, 0:1, :])
            bq_t = rows_p.tile([1, 3 * D], BF16, tag="bqr", name="bqt", bufs=1)
            nc.sync.dma_start(bq_t[:], wsbq_d[li, 1:2, :])
            cols = stat_p.tile([128, 12], F32, tag="tcols", name="tcols")
            nc.sync.dma_start(cols[:], tcols_d[li])

            # q bias broadcast over the token partitions (k-bias is
            # softmax-invariant; v-bias is folded into outb on the host)
            bias_bc = attn_p.tile([128, D], BF16, tag="biasbc",
                                  name="bias_bc", bufs=1)
            psb = pp_bc.tile([128, 512], F32, tag="bc", name="psbb")
            nc.tensor.matmul(psb[:], ones_bf[:], bq_t[:, 0:512],
                             start=True, stop=True)
            nc.scalar.copy(bias_bc[:], psb[:])

            nm = attn_p.tile([1, TOK], BF16, tag="nmsd", name="nm", bufs=1)
            rcol = attn_p.tile([128, 16], F32, tag="rcol", name="rcol", bufs=1)
            ln1_rows(x_in, nm, rcol)

            for st in range(4):
                qt = attn_p.tile([128, TOK], BF16, tag="qst", name="qt", bufs=2)
                kt = attn_p.tile([128, TOK], BF16, tag="kst", name="kt", bufs=2)
                # v stored (j, d, h) so the AV multiply hits the 2x DVE mode
                vt = attn_p.tile([128, TOK], BF16, tag="vst", name="vt", bufs=2)
                for f in range(NF):
                    c0 = f * 512 + st * 128
                    for ns in range(3):
                        ps = pp_mm.tile([128, 512], F32, tag="mm", name="qps")
                        for k in range(4):
                            nc.tensor.matmul(
                                ps[:], x_in[k][:, c0:c0 + 128],
                                wq[k][:, ns * 512:(ns + 1) * 512],
                                start=(k == 0), stop=False)
                        nc.tensor.matmul(ps[:], nm[0:1, c0:c0 + 128],
                                         ws_t[:, ns * 512:(ns + 1) * 512],
                                         start=False, stop=True)
                        rc = rcol[:, f * 4 + st:f * 4 + st + 1]
                        if ns < 2:
                            nc.scalar.activation(
                                (qt if ns == 0 else kt)[:, f * 512:(f + 1) * 512],
                                ps[:], AF.Copy, scale=rc)
                        else:
                            nc.scalar.activation(
                                vt[:, f * 512:(f + 1) * 512]
                                .rearrange("p (d h) -> p h d", h=8),
                                ps[:].rearrange("p (h d) -> p h d", h=8),
                                AF.Copy, scale=rc)
                # q bias, broadcast over frames, one 2x-mode op per st
                nc.vector.tensor_tensor(
                    qt[:].rearrange("p (f c) -> p f c", f=4),
                    qt[:].rearrange("p (f c) -> p f c", f=4),
                    bias_bc[:].unsqueeze(1).broadcast_to([128, 4, 512]),
                    op=ALU.add)

                s_sc = attn_p.tile([128, 128], F32, tag="s_sc", name="s_sc",
                                   bufs=2)
                k4 = kt[:].rearrange("p (j hd) -> p j hd", j=4)
                for i in range(4):
                    pbig = attn_p.tile([128, TOK], BF16, tag="pbig",
                                       name="pbig", bufs=2)
                    qi = qt[:, i * 512:(i + 1) * 512].unsqueeze(1) \
                        .broadcast_to([128, 4, 512])
                    nc.vector.tensor_mul(
                        pbig[:].rearrange("p (j hd) -> p j hd", j=4), qi, k4)
                    st1 = attn_p.tile([128, TOK // 2], BF16, tag="qks1",
                                      name="qks1", bufs=2)
                    pv = pbig[:].rearrange("p (g d) -> p g d", g=32)
                    nc.vector.tensor_tensor(
                        st1[:].rearrange("p (g d) -> p g d", g=32),
                        pv[:, :, 0:32], pv[:, :, 32:64], op=ALU.add)
                    nc.vector.tensor_reduce(
                        s_sc[:, i * 32:(i + 1) * 32],
                        st1[:].rearrange("p (g d) -> p g d", g=32),
                        axis=AX.X, op=ALU.add)
                # softmax over j without max-subtraction (logits bounded)
                # S cols = (i, j, h)
                eexp = attn_p.tile([128, 128], BF16, tag="eexp", name="eexp",
                                   bufs=2)
                nc.scalar.activation(eexp[:], s_sc[:], AF.Exp)
                z = attn_p.tile([128, 32], F32, tag="z", name="zt", bufs=2)
                nc.vector.tensor_reduce(
                    z[:].rearrange("p (i h) -> p i h", i=4),
                    eexp[:].rearrange("p (i j h) -> p i h j", i=4, j=4),
                    axis=AX.X, op=ALU.add)
                zr = attn_p.tile([128, 32], F32, tag="zr", name="zr", bufs=2)
                nc.vector.reciprocal(zr[:], z[:])
                a_t = attn_p.tile([128, 128], BF16, tag="a_t", name="a_t",
                                  bufs=2)
                nc.vector.tensor_mul(
                    a_t[:].rearrange("p (i j h) -> p i j h", i=4, j=4),
                    eexp[:].rearrange("p (i j h) -> p i j h", i=4, j=4),
                    zr[:].rearrange("p (i h) -> p i h", i=4).unsqueeze(2)
                    .broadcast_to([128, 4, 4, 8]))
                for i in range(4):
                    tbig = attn_p.tile([128, TOK], BF16, tag="tbig", name="tbig", bufs=2)
                    ablk = a_t[:, i * 32:(i + 1) * 32] \
                        .rearrange("p (j h) -> p j h", j=4) \
                        .unsqueeze(2).broadcast_to([128, 4, 64, 8])
                    nc.vector.tensor_mul(
                        tbig[:].rearrange("p (j d h) -> p j d h", j=4, d=64),
                        vt[:].rearrange("p (j d h) -> p j d h", j=4, d=64),
                        ablk)
                    av01 = attn_p.tile([128, 512], F32, tag="av01", name="av01",
                                       bufs=2)
                    av23 = attn_p.tile([128, 512], F32, tag="av23", name="av23",
                                       bufs=2)
                    av = attn_p.tile([128, 512], BF16, tag="av", name="av",
                                     bufs=2)
                    nc.gpsimd.tensor_tensor(av01[:], tbig[:, 0:512],
                                            tbig[:, 512:1024], op=ALU.add)
                    nc.gpsimd.tensor_tensor(av23[:], tbig[:, 1024:1536],
                                            tbig[:, 1536:2048], op=ALU.add)
                    nc.gpsimd.tensor_tensor(av[:], av01[:], av23[:],
                                            op=ALU.add)
                    pst = pp_bc.tile([128, 512], BF16, tag="bc", name="pst")
                    for c in range(4):
                        nc.tensor.transpose(pst[:, c * 128:(c + 1) * 128],
                                            av[:, c * 128:(c + 1) * 128],
                                            ident[:])
                    nc.scalar.copy(
                        otb_all[:].rearrange("p (c t) -> p c t", c=4)
                        [:, :, i * 512 + st * 128:i * 512 + st * 128 + 128],
                        pst[:].rearrange("p (c t) -> p c t", c=4))

            wo = [wp.tile([128, D], BF16, tag=f"wo{k}", name=f"wo{k}")
                  for k in range(4)]
            for k in range(4):
                nc.sync.dma_start(wo[k][:], wo_d[li, k * 128:(k + 1) * 128, :])
            for m in range(4):
                for ns in range(4):
                    ps = pp_mm.tile([128, 512], F32, tag="mm", name="ops")
                    for k in range(4):
                        nc.tensor.matmul(
                            ps[:], wo[k][:, m * 128:(m + 1) * 128],
                            otb_all[:, k * TOK + ns * 512:k * TOK + (ns + 1) * 512],
                            start=(k == 0), stop=(k == 3))
                    nc.vector.scalar_tensor_tensor(
                        x_mid[m][:, ns * 512:(ns + 1) * 512], ps[:],
                        cols[:, 4 + m:5 + m], x_in[m][:, ns * 512:(ns + 1) * 512],
                        op0=ALU.add, op1=ALU.add)

            w1 = [wp.tile([128, D], F32R, tag=f"w1_{k}", name=f"w1_{k}")
                  for k in range(4)]
            w2 = [wp.tile([128, D], F32R, tag=f"w2_{k}", name=f"w2_{k}")
                  for k in range(4)]
            for k in range(4):
                nc.sync.dma_start(w1[k][:], w1_d[li, k * 128:(k + 1) * 128, :])
                nc.sync.dma_start(w2[k][:], w2_d[li, k * 128:(k + 1) * 128, :])
            for ns in range(4):
                xh2 = [sl_p.tile([128, 512], F32R, tag=f"xh2_{k}", name=f"xh2_{k}")
                       for k in range(4)]
                ln_cols(x_mid,
                        lambda k: x_mid[k][:, ns * 512:(ns + 1) * 512],
                        xh2, slice(0, 512))
                hsl = [sl_p.tile([128, 512], F32R, tag=f"h_{m}", name=f"hsl{m}")
                       for m in range(4)]
                for m in range(4):
                    ps = pp_mm.tile([128, 512], F32, tag="mm", name="m1ps")
                    for k in range(4):
                        nc.tensor.matmul(
                            ps[:], w1[k][:, m * 128:(m + 1) * 128], xh2[k][:],
                            start=(k == 0), stop=(k == 3))
                    nc.scalar.activation(hsl[m][:], ps[:], AF.Gelu_apprx_tanh,
                                         bias=cols[:, m:m + 1])
                for m in range(4):
                    ps = pp_mm.tile([128, 512], F32, tag="mm", name="m2ps")
                    for k in range(4):
                        nc.tensor.matmul(
                            ps[:], w2[k][:, m * 128:(m + 1) * 128], hsl[k][:],
                            start=(k == 0), stop=(k == 3))
                    nc.vector.scalar_tensor_tensor(
                        x_mid[m][:, ns * 512:(ns + 1) * 512], ps[:],
                        cols[:, 8 + m:9 + m], x_mid[m][:, ns * 512:(ns + 1) * 512],
                        op0=ALU.add, op1=ALU.add)

        cur = xA
        _nl = int(os.environ.get("KNLAYERS", NLAYER))
        for li in range(_nl):
            transformer_layer(li, cur, cur)

        tr_ctx.close()

        # ------------------------------------------------------------------
        # heads
        # ------------------------------------------------------------------
        _skip_heads = os.environ.get("KHEADS", "1") == "0"
        if _skip_heads:
            nc.gpsimd.dma_start(d6_o[:], cur[0][0:8, :])
            nc.gpsimd.dma_start(tr_o[:], cur[1][0:8, :])
        with tc.tile_pool(name="heads", bufs=1) as hp, \
             tc.tile_pool(name="whp", bufs=1) as whp:
          if not _skip_heads:
              hc = stat_p.tile([128, 10], F32, tag="hcols", name="hc")
              nc.sync.dma_start(hc[:], hcols_d[:])
              sb3 = stat_p.tile([8, 2], F32, tag="sb3", name="sb3")
              nc.sync.dma_start(sb3[:], sb3_d[:])

              xhf = [hp.tile([128, TOK], F32R, tag=f"xhf{k}", name=f"xhf{k}")
                     for k in range(4)]
              for st in range(4):
                  ln_cols(cur, lambda k: cur[k][:, st * 512:(st + 1) * 512],
                          xhf, slice(st * 512, (st + 1) * 512))

              xp = [hp.tile([128, TOK], F32R, tag=f"xp{k}", name=f"xp{k}")
                    for k in range(4)]

              def mm_head(src_tiles, wt_dram, kdim, mdim, dst_tiles, evict):
                  KC = kdim // 128
                  MC = max(mdim // 128, 1)
                  wsb = [whp.tile([128, mdim], F32R, tag=f"wh_{kdim}_{mdim}_{k}",
                                  name=f"wh{k}") for k in range(KC)]
                  for k in range(KC):
                      nc.sync.dma_start(wsb[k][:], wt_dram[k * 128:(k + 1) * 128, :])
                  for m in range(MC):
                      for ns in range(4):
                          ps = pp_mm.tile([128, 512], F32, tag="mm", name="hps")
                          for k in range(KC):
                              nc.tensor.matmul(
                                  ps[:], wsb[k][:, m * 128:(m + 1) * 128],
                                  src_tiles[k][:, ns * 512:(ns + 1) * 512],
                                  start=(k == 0), stop=(k == KC - 1))
                          evict(ps, dst_tiles[m], m, ns)

              mm_head(xhf, projw_d, D, D, xp,
                      lambda ps, dst, m, ns: nc.scalar.activation(
                          dst[:, ns * 512:(ns + 1) * 512], ps[:], AF.Identity,
                          bias=hc[:, m:m + 1]))

              def branch(w1d, w2d, w3d, b1ofs, b2ofs, out_dram, b3col, r1, r2, pfx):
                  mm_head(xp, w1d, D, 256, r1,
                          lambda ps, dst, m, ns: nc.scalar.activation(
                              dst[:, ns * 512:(ns + 1) * 512], ps[:], AF.Relu,
                              bias=hc[:, b1ofs + m:b1ofs + m + 1]))
                  mm_head(r1, w2d, 256, 128, r2,
                          lambda ps, dst, m, ns: nc.scalar.activation(
                              dst[:, ns * 512:(ns + 1) * 512], ps[:], AF.Relu,
                              bias=hc[:, b2ofs:b2ofs + 1]))
                  w3 = whp.tile([128, 8], F32R, tag=f"w3{pfx}", name="w3")
                  nc.sync.dma_start(w3[:], w3d[:])
                  out_sb = hp.tile([8, TOK], F32, tag=f"{pfx}out", name=f"{pfx}out")
                  for ns in range(4):
                      ps = pp_mm.tile([8, 512], F32, tag="mm", name="bps")
                      nc.tensor.matmul(ps[:], w3[:], r2[0][:, ns * 512:(ns + 1) * 512],
                                       start=True, stop=True)
                      nc.scalar.activation(out_sb[:, ns * 512:(ns + 1) * 512], ps[:],
                                           AF.Identity, bias=b3col)
                  nc.sync.dma_start(out_dram[:], out_sb[:])

              # reuse dead transformer buffers for intermediates
              branch(rw1_d, rw2_d, rw3_d, 4, 8, d6_o, sb3[:, 0:1],
                     [cur[0], cur[1]], [xhf[0]], "r")
              branch(tw1_d, tw2_d, tw3_d, 6, 9, tr_o, sb3[:, 1:2],
                     [cur[2], cur[3]], [xhf[1]], "t")

    nc.compile()
    return nc


# ----------------------------------------------------------------------------
# host side
# ----------------------------------------------------------------------------

_CACHE = {}


def _normalize_np(v, eps=1e-12):
    return v / np.maximum(np.linalg.norm(v, axis=-1, keepdims=True), eps)


def _rot6d_np(d6):
    a1, a2 = d6[..., :3], d6[..., 3:]
    b1 = _normalize_np(a1)
    b2 = _normalize_np(a2 - np.sum(b1 * a2, -1, keepdims=True) * b1)
    b3 = np.cross(b1, b2)
    return np.stack([b1, b2, b3], axis=-2)


def _prep_weights(inp):
    f32 = np.float32
    wmap = {}
    for i, cw in enumerate(['c1w', 'c2w', 'c3w', 'c4w']):
        wmap[f'convw{i}'] = np.ascontiguousarray(inp[cw].T.astype(f32))
    for i, (g, b2) in enumerate([('bn1g', 'bn1b'), ('bn2g', 'bn2b'),
                                 ('bn3g', 'bn3b')]):
        M = CONV_DIMS[i + 1] // 128
        bn = np.concatenate([
            inp[g].reshape(M, 128).T, inp[b2].reshape(M, 128).T,
            inp[f'c{i + 1}b'].reshape(M, 128).T], axis=1)
        wmap[f'bnconst{i}'] = np.ascontiguousarray(bn.astype(f32))
    pe = _pe_table()[:NF]
    b4 = inp['c4b'][None, :].astype(f32) + pe                   # (4, 512)
    # cols: m*4 + pt ; frame index == pt
    wmap['bias4'] = np.ascontiguousarray(
        b4.reshape(NF, 4, 128).transpose(2, 1, 0).reshape(128, 16).astype(f32))

    qkvw = np.array(inp['qkvw'], f32)
    qkvb = np.array(inp['qkvb'], f32)
    qkvw[:, :, :512] /= math.sqrt(DH)
    qkvb[:, :512] /= math.sqrt(DH)
    g1 = np.array(inp['ln1g'], f32)
    b1 = np.array(inp['ln1b'], f32)
    wq_fold = g1[:, :, None] * qkvw
    bq_fold = qkvb + np.einsum('ld,ldf->lf', b1, qkvw)
    wmap['wqkv'] = np.ascontiguousarray(wq_fold.astype(f32))
    wsbq = np.concatenate([wq_fold.sum(axis=1, keepdims=True),
                           bq_fold[:, None, :]], axis=1)         # (L, 2, 3D)
    wmap['wsbq'] = np.ascontiguousarray(wsbq.astype(ml_dtypes.bfloat16))
    # attention output features are (d, h)-ordered; permute wo rows to match
    wo_ = np.array(inp['outw'], f32)                             # (L, 512, 512)
    d_idx, h_idx = np.meshgrid(np.arange(DH), np.arange(HEADS), indexing='ij')
    perm = (h_idx * DH + d_idx).reshape(512)     # perm[d*8+h] = h*64+d
    wmap['wo'] = np.ascontiguousarray(wo_[:, perm, :]
                                      .astype(ml_dtypes.bfloat16))
    g2 = np.array(inp['ln2g'], f32)
    bl2 = np.array(inp['ln2b'], f32)
    m1w = np.array(inp['m1w'], f32)
    w1_fold = g2[:, :, None] * m1w
    b1_fold = np.array(inp['m1b'], f32) + np.einsum('ld,ldf->lf', bl2, m1w)
    wmap['w1'] = np.ascontiguousarray(w1_fold.astype(f32))
    wmap['w2'] = np.ascontiguousarray(np.array(inp['m2w'], f32))
    cols = np.zeros((NLAYER, 128, 12), f32)
    cols[:, :, 0:4] = b1_fold.reshape(NLAYER, 4, 128).transpose(0, 2, 1)
    # v-bias is dropped at the v eviction; fold bv @ Wo into outb instead
    bv = bq_fold[:, 2 * 512:3 * 512]                             # (L, 512)
    outb_fold = np.array(inp['outb'], f32) + np.einsum('lk,lko->lo', bv, wo_)
    cols[:, :, 4:8] = outb_fold.reshape(NLAYER, 4, 128).transpose(0, 2, 1)
    cols[:, :, 8:12] = np.array(inp['m2b'], f32).reshape(NLAYER, 4, 128) \
        .transpose(0, 2, 1)
    wmap['tcols'] = cols

    gf_ = np.array(inp['lnfg'], f32)
    bf_ = np.array(inp['lnfb'], f32)
    projw = np.array(inp['projw'], f32)
    wmap['projw'] = np.ascontiguousarray(gf_[:, None] * projw)
    projb_fold = np.array(inp['projb'], f32) + bf_ @ projw
    wmap['rw1'] = np.ascontiguousarray(np.array(inp['rw1'], f32))
    wmap['rw2'] = np.ascontiguousarray(np.array(inp['rw2'], f32))
    rw3 = np.zeros((128, 8), f32)
    rw3[:, :6] = np.array(inp['rw3'], f32)
    wmap['rw3'] = rw3
    wmap['tw1'] = np.ascontiguousarray(np.array(inp['tw1'], f32))
    wmap['tw2'] = np.ascontiguousarray(np.array(inp['tw2'], f32))
    tw3 = np.zeros((128, 8), f32)
    tw3[:, :3] = np.array(inp['tw3'], f32)
    wmap['tw3'] = tw3
    hcols = np.zeros((128, 10), f32)
    hcols[:, 0:4] = projb_fold.reshape(4, 128).T
    hcols[:, 4:6] = np.array(inp['rb1'], f32).reshape(2, 128).T
    hcols[:, 6:8] = np.array(inp['tb1'], f32).reshape(2, 128).T
    hcols[:, 8] = np.array(inp['rb2'], f32)
    hcols[:, 9] = np.array(inp['tb2'], f32)
    wmap['hcols'] = hcols
    sb3 = np.zeros((8, 2), f32)
    sb3[0:6, 0] = np.array(inp['rb3'], f32)
    sb3[0:3, 1] = np.array(inp['tb3'], f32)
    wmap['sb3'] = sb3
    wmap['ones_c'] = np.ones((128, 128), f32)
    return wmap


def kernel(**inputs):
    inp = {k: np.asarray(v) for k, v in inputs.items()}

    idx = inp['seed_idxs'].reshape(B, -1).astype(np.int64)      # (B, N)
    sel_seed = np.take_along_axis(np.asarray(inp['fp2_features'], np.float32),
                                  idx[:, None, :], axis=2)
    sel_grasp = np.take_along_axis(np.asarray(inp['local_grasp_features'], np.float32),
                                   idx[:, None, :], axis=2)
    sel_color = np.take_along_axis(np.asarray(inp['local_color_features'], np.float32),
                                   idx[:, None, :], axis=2)
    sel_pose = np.take_along_axis(np.asarray(inp['grasp_pose_feature'], np.float32),
                                  idx[:, None, :], axis=2)
    gsf = np.asarray(inp['sa4_features'], np.float32).max(axis=-1)
    gsf = np.broadcast_to(gsf[:, :, None], (B, 256, NPTS))
    fused = sel_pose + np.concatenate([sel_grasp, sel_color, sel_seed, gsf], axis=1)
    gf = fused.reshape(BE, FRAME, 1024, NPTS)
    cond = np.broadcast_to(gf[:, :1], (BE, NF, 1024, NPTS))
    X = np.concatenate([cond, gf[:, 1:]], axis=2)               # (e, f, 2048, N)

    if 'nc' not in _CACHE:
        _CACHE['nc'] = build_kernel()
    nc = _CACHE['nc']
    wmap = _prep_weights(inp)

    in_maps = []
    for k in range(NCORES):
        xc = X[:, :, :, k * NPC:(k + 1) * NPC]                  # (e, f, c, n)
        xc = xc.transpose(2, 1, 0, 3).reshape(2048, TOK)        # (c, (f,e,n))
        m = dict(wmap)
        m['xin'] = np.ascontiguousarray(xc, dtype=np.float32)
        in_maps.append(m)

    res = run_bass_kernel_spmd(nc, in_maps, core_ids=list(range(NCORES)))

    out = np.zeros((BE * NPTS, NF, 12), np.float32)
    for k in range(NCORES):
        d6 = res.results[k]['d6'][:6]
        tr = res.results[k]['tr3'][:3]
        d6 = d6.reshape(6, NF, BE, NPC).transpose(2, 3, 1, 0)   # (e, n, f, 6)
        tr = tr.reshape(3, NF, BE, NPC).transpose(2, 3, 1, 0)
        rot = _rot6d_np(d6).reshape(BE, NPC, NF, 9)
        for e in range(BE):
            rows = slice(e * NPTS + k * NPC, e * NPTS + (k + 1) * NPC)
            out[rows, :, 0:3] = tr[e]
            out[rows, :, 3:12] = rot[e]
    return out


if __name__ == "__main__":
    build_kernel()
    print("built ok")



# revision 65
# speedup vs baseline: 1.0245x; 1.0147x over previous
import math
import os
import numpy as np
import ml_dtypes
import contextlib

import concourse.bass as bass
import concourse.tile as tile
from concourse import bacc, mybir, masks
from concourse.bass_utils import run_bass_kernel_spmd

F32 = mybir.dt.float32
F32R = mybir.dt.float32r
BF16 = mybir.dt.bfloat16
ALU = mybir.AluOpType
AF = mybir.ActivationFunctionType
AX = mybir.AxisListType

NCORES = 8
FRAME = 5
NF = FRAME - 1
D = 512
DH = 64
HEADS = 8
B = 20
NPTS = 1024
BE = B // FRAME
NPC = NPTS // NCORES     # 128 points per core
TOK = NF * BE * NPC      # 2048 tokens per core
NLAYER = 12
LNEPS = 1e-5
BNEPS = 1e-5
NBN = 16 * NPTS

CONV_DIMS = [2048, 1536, 1024, 768, 512]


def _pe_table(max_len=16, d=D):
    pos = np.arange(max_len, dtype=np.float32)[:, None]
    div = np.exp(np.arange(0, d, 2, dtype=np.float32) * (-math.log(10000.0) / d))
    pe = np.zeros((max_len, d), np.float32)
    pe[:, 0::2] = np.sin(pos * div)
    pe[:, 1::2] = np.cos(pos * div)
    return pe


def build_kernel():
    nc = bacc.Bacc("TRN2", target_bir_lowering=False, debug=False,
                   num_devices=NCORES)

    xin = nc.dram_tensor("xin", [CONV_DIMS[0], TOK], F32R, kind="ExternalInput").ap()
    convw = [nc.dram_tensor(f"convw{i}", [CONV_DIMS[i], CONV_DIMS[i + 1]], F32R,
                            kind="ExternalInput").ap() for i in range(4)]
    bnconst = [nc.dram_tensor(f"bnconst{i}", [128, 3 * (CONV_DIMS[i + 1] // 128)],
                              F32, kind="ExternalInput").ap() for i in range(3)]
    bias4 = nc.dram_tensor("bias4", [128, 4 * NF], F32, kind="ExternalInput").ap()

    wqkv_d = nc.dram_tensor("wqkv", [NLAYER, D, 3 * D], F32R, kind="ExternalInput").ap()
    wsbq_d = nc.dram_tensor("wsbq", [NLAYER, 2, 3 * D], BF16, kind="ExternalInput").ap()
    wo_d = nc.dram_tensor("wo", [NLAYER, D, D], BF16, kind="ExternalInput").ap()
    w1_d = nc.dram_tensor("w1", [NLAYER, D, D], F32R, kind="ExternalInput").ap()
    w2_d = nc.dram_tensor("w2", [NLAYER, D, D], F32R, kind="ExternalInput").ap()
    tcols_d = nc.dram_tensor("tcols", [NLAYER, 128, 12], F32, kind="ExternalInput").ap()

    projw_d = nc.dram_tensor("projw", [D, D], F32R, kind="ExternalInput").ap()
    rw1_d = nc.dram_tensor("rw1", [D, 256], F32R, kind="ExternalInput").ap()
    rw2_d = nc.dram_tensor("rw2", [256, 128], F32R, kind="ExternalInput").ap()
    rw3_d = nc.dram_tensor("rw3", [128, 8], F32R, kind="ExternalInput").ap()
    tw1_d = nc.dram_tensor("tw1", [D, 256], F32R, kind="ExternalInput").ap()
    tw2_d = nc.dram_tensor("tw2", [256, 128], F32R, kind="ExternalInput").ap()
    tw3_d = nc.dram_tensor("tw3", [128, 8], F32R, kind="ExternalInput").ap()
    hcols_d = nc.dram_tensor("hcols", [128, 10], F32, kind="ExternalInput").ap()
    sb3_d = nc.dram_tensor("sb3", [8, 2], F32, kind="ExternalInput").ap()
    ones_d = nc.dram_tensor("ones_c", [128, 128], F32, kind="ExternalInput").ap()

    d6_o = nc.dram_tensor("d6", [8, TOK], F32, kind="ExternalOutput").ap()
    tr_o = nc.dram_tensor("tr3", [8, TOK], F32, kind="ExternalOutput").ap()

    with tile.TileContext(nc) as tc, contextlib.ExitStack() as ctx:
        const_p = ctx.enter_context(tc.tile_pool(name="consts", bufs=1))
        onescol = const_p.tile([128, 1], F32R)
        onesrow = const_p.tile([1, 128], F32R)
        ident = const_p.tile([128, 128], BF16)
        nc.gpsimd.dma_start(onescol[:], ones_d[:, 0:1])
        nc.gpsimd.dma_start(onesrow[:], ones_d[0:1, :])
        ones_bf = const_p.tile([1, 128], BF16)
        nc.vector.memset(ones_bf[:], 1.0)
        masks.make_identity(nc, ident[:])

        xs_p = ctx.enter_context(tc.tile_pool(name="xstate", bufs=1))
        xA = [xs_p.tile([128, TOK], F32R, tag=f"xA{m}", name=f"xA{m}") for m in range(4)]

        stat_p = ctx.enter_context(tc.tile_pool(name="stats", bufs=1))
        dram_p = ctx.enter_context(tc.tile_pool(name="dramb", bufs=1, space="DRAM"))

        pp_mm = ctx.enter_context(tc.tile_pool(name="ppmm", bufs=4, space="PSUM"))
        pp_row = ctx.enter_context(tc.tile_pool(name="pprow", bufs=1, space="PSUM"))
        pp_bc = ctx.enter_context(tc.tile_pool(name="ppbc", bufs=2, space="PSUM"))

        y_dram = [dram_p.tile([CONV_DIMS[i], TOK], F32R, tag=f"ydram{i}", name=f"ydram{i}")
                  for i in range(1, 4)]

        # ------------------------------------------------------------------
        # conv stack (activations spilled to DRAM, BN applied on load)
        # ------------------------------------------------------------------
        b4sb = stat_p.tile([128, 4 * NF], F32, tag="b4")
        nc.sync.dma_start(b4sb[:], bias4[:])

        bn_s = {}
        bn_t = {}

        def conv_layer(li, wcp, cxp, pp_conv):
            kdim, mdim = CONV_DIMS[li - 1], CONV_DIMS[li]
            KC, MC = kdim // 128, mdim // 128
            src = xin if li == 1 else y_dram[li - 2]
            with_bn = li < 4
            if with_bn:
                sum_acc = stat_p.tile([128, MC * 4], F32, tag=f"sum{li}")
                sq_acc = stat_p.tile([128, MC * 4], F32, tag=f"sq{li}")
            cond_sb = None
            if li == 1:
                # channels 0:1024 repeat frame 0's features for all 4 frames;
                # compute their contribution once and add it at eviction
                KC = 8
                xc = cxp.tile([128, 8 * 512], F32R, tag="convc", name="convc",
                              bufs=1)
                nc.sync.dma_start(
                    xc[:].rearrange("p (k c) -> p k c", k=8),
                    src[0:1024, 0:512].rearrange("(k p) c -> p k c", p=128))
                cond_sb = [cxp.tile([128, 512], F32, tag=f"cond{m}",
                                    name=f"cond{m}") for m in range(MC)]
                for m in range(MC):
                    wslc = wcp.tile([128, 8 * 128], F32R, tag="wslc",
                                    name="wslc", bufs=3)
                    wvc = wslc[:].rearrange("p (k c) -> p k c", k=8)
                    nc.sync.dma_start(
                        wvc, convw[0][0:1024, m * 128:(m + 1) * 128]
                        .rearrange("(k p) c -> p k c", p=128))
                    ps = pp_conv.tile([128, 512], F32, tag="mm", name="ccps")
                    for k in range(8):
                        nc.tensor.matmul(
                            ps[:], wvc[:, k, :], xc[:, k * 512:(k + 1) * 512],
                            start=(k == 0), stop=(k == 7))
                    nc.scalar.copy(cond_sb[m][:], ps[:])
            for pt in range(4):
                xt = cxp.tile([128, KC * 512], F32R, tag="convx", name="convx",
                              bufs=3)
                nc.sync.dma_start(
                    xt[:].rearrange("p (k c) -> p k c", k=KC),
                    src[kdim - KC * 128:, pt * 512:(pt + 1) * 512]
                    .rearrange("(k p) c -> p k c", p=128))
                if li > 1:
                    s_p, t_p = bn_s[li - 1], bn_t[li - 1]
                    for k in range(KC):
                        nc.scalar.activation(
                            xt[:, k * 512:(k + 1) * 512],
                            xt[:, k * 512:(k + 1) * 512],
                            AF.Relu, bias=t_p[:, k:k + 1], scale=s_p[:, k:k + 1])
                for m in range(MC):
                    wsl = wcp.tile([128, KC * 128], F32R, tag="wsl", name="wsl",
                                   bufs=6)
                    wv = wsl[:].rearrange("p (k c) -> p k c", k=KC)
                    nc.sync.dma_start(
                        wv, convw[li - 1][kdim - KC * 128:,
                                          m * 128:(m + 1) * 128]
                        .rearrange("(k p) c -> p k c", p=128))
                    ps = pp_conv.tile([128, 512], F32, tag="mm", name="cps")
                    for k in range(KC):
                        nc.tensor.matmul(
                            ps[:], wv[:, k, :], xt[:, k * 512:(k + 1) * 512],
                            start=(k == 0), stop=(k == KC - 1))
                    if with_bn:
                        ot = cxp.tile([128, 512], F32R, tag="convot", name="cot",
                                      bufs=4)
                        if cond_sb is not None:
                            nc.vector.tensor_tensor(ot[:], ps[:],
                                                    cond_sb[m][:], op=ALU.add)
                            stats_src = ot[:]
                        else:
                            stats_src = ps[:]
                            nc.scalar.activation(
                                ot[:], ps[:], AF.Copy,
                                accum_out=sum_acc[:, m * 4 + pt:m * 4 + pt + 1])
                        sqs = cxp.tile([128, 512], BF16, tag="sqscr", name="sqs",
                                       bufs=4)
                        if cond_sb is not None:
                            nc.scalar.activation(
                                sqs[:], stats_src, AF.Copy,
                                accum_out=sum_acc[:, m * 4 + pt:m * 4 + pt + 1])
                        nc.scalar.activation(
                            sqs[:], stats_src, AF.Square,
                            accum_out=sq_acc[:, m * 4 + pt:m * 4 + pt + 1])
                        nc.sync.dma_start(
                            y_dram[li - 1][m * 128:(m + 1) * 128,
                                           pt * 512:(pt + 1) * 512], ot[:])
                    else:
                        nc.scalar.activation(
                            xA[m][:, pt * 512:(pt + 1) * 512], ps[:], AF.Identity,
                            bias=b4sb[:, m * 4 + pt:m * 4 + pt + 1])
            if not with_bn:
                return
            allin = stat_p.tile([128, 2 * MC], F32, tag=f"ain{li}", name="allin")
            nc.vector.tensor_reduce(
                allin[:, 0:MC], sum_acc[:].rearrange("p (m t) -> p m t", m=MC),
                axis=AX.X, op=ALU.add)
            nc.vector.tensor_reduce(
                allin[:, MC:2 * MC], sq_acc[:].rearrange("p (m t) -> p m t", m=MC),
                axis=AX.X, op=ALU.add)
            bin_ = dram_p.tile([128, 2 * MC], F32, tag=f"arin{li}", name="arin")
            bout = dram_p.tile([128, 2 * MC], F32, tag=f"arout{li}", name="arout")
            nc.sync.dma_start(bin_[:], allin[:])
            nc.gpsimd.collective_compute(
                "AllReduce", ALU.add, replica_groups=[list(range(NCORES))],
                ins=[bin_.opt()], outs=[bout.opt()])
            gl = stat_p.tile([128, 2 * MC], F32, tag=f"gl{li}", name="gl")
            nc.sync.dma_start(gl[:], bout[:])
            cst = stat_p.tile([128, 3 * MC], F32, tag=f"cst{li}", name="cst")
            nc.sync.dma_start(cst[:], bnconst[li - 1][:])
            mu = stat_p.tile([128, MC], F32, tag=f"mu{li}", name="bmu")
            var = stat_p.tile([128, MC], F32, tag=f"va{li}", name="bvar")
            s_t = stat_p.tile([128, MC], F32, tag=f"s{li}", name="bs")
            t_t = stat_p.tile([128, MC], F32, tag=f"t{li}", name="bt")
            nc.scalar.mul(mu[:], gl[:, 0:MC], 1.0 / NBN)
            nc.scalar.mul(var[:], gl[:, MC:2 * MC], 1.0 / NBN)
            msq = stat_p.tile([128, MC], F32, tag=f"ms{li}", name="bmsq")
            nc.vector.tensor_mul(msq[:], mu[:], mu[:])
            nc.vector.tensor_tensor(var[:], var[:], msq[:], op=ALU.subtract)
            nc.vector.tensor_scalar(var[:], var[:], BNEPS, None, op0=ALU.add)
            sd = stat_p.tile([128, MC], F32, tag=f"sd{li}", name="bsd")
            nc.scalar.activation(sd[:], var[:], AF.Sqrt)
            rsd = stat_p.tile([128, MC], F32, tag=f"rs{li}", name="brsd")
            nc.vector.reciprocal(rsd[:], sd[:])
            nc.vector.tensor_mul(s_t[:], rsd[:], cst[:, 0:MC])
            nc.vector.tensor_mul(t_t[:], mu[:], s_t[:])
            nc.vector.tensor_tensor(t_t[:], cst[:, MC:2 * MC], t_t[:],
                                    op=ALU.subtract)
            bn_s[li], bn_t[li] = s_t, t_t

        with tc.tile_pool(name="wcp", bufs=1) as wcp, \
             tc.tile_pool(name="cxp", bufs=1) as cxp:
            for li in (1, 2, 3, 4):
                conv_layer(li, wcp, cxp, pp_mm)

        # ------------------------------------------------------------------
        # transformer
        # ------------------------------------------------------------------
        rows_p = ctx.enter_context(tc.tile_pool(name="rows", bufs=1))
        scr = ctx.enter_context(tc.tile_pool(name="scratch", bufs=2))

        def ln_cols(xt, xview, dst_tiles, dst_cols):
            """LN per token over feature dim (stats + apply on DVE/Pool)."""
            ps_s = pp_row.tile([1, 512], F32, tag="row_s", name="ps_s")
            ps_q = pp_row.tile([1, 512], F32, tag="row_q", name="ps_q")
            for k in range(4):
                nc.tensor.matmul(ps_s[:], onescol[:], xview(k),
                                 start=(k == 0), stop=(k == 3))
            for k in range(4):
                sq = scr.tile([128, 512], F32R, tag="lnsq", name="lnsq")
                eng = nc.vector if k % 2 else nc.gpsimd
                eng.tensor_mul(sq[:], xview(k), xview(k))
                nc.tensor.matmul(ps_q[:], onescol[:], sq[:],
                                 start=(k == 0), stop=(k == 3))
            mu = rows_p.tile([1, 512], F32R, tag="mu", name="lmu", bufs=2)
            e2 = rows_p.tile([1, 512], F32, tag="e2", name="le2", bufs=2)
            r = rows_p.tile([1, 512], F32R, tag="r", name="lr", bufs=2)
            nc.scalar.mul(mu[:], ps_s[:], 1.0 / D)
            nc.scalar.mul(e2[:], ps_q[:], 1.0 / D)
            with nc.allow_low_precision(reason="f32r row math"):
                nc.vector.tensor_mul(r[:], mu[:], mu[:])
                nc.vector.scalar_tensor_tensor(
                    e2[:], e2[:], LNEPS, r[:], op0=ALU.add, op1=ALU.subtract)
                # 1/sqrt(v) = exp(-0.5 ln v): stays inside the exp/ln table
                nc.scalar.activation(e2[:], e2[:], AF.Ln)
                nc.scalar.activation(r[:], e2[:], AF.Exp, scale=-0.5)
            psb_mu = pp_bc.tile([128, 512], F32, tag="bc", name="psbmu")
            psb_r = pp_bc.tile([128, 512], F32, tag="bc", name="psbr")
            nc.tensor.matmul(psb_mu[:], onesrow[:], mu[:], start=True, stop=True)
            nc.tensor.matmul(psb_r[:], onesrow[:], r[:], start=True, stop=True)
            for k in range(4):
                tmp = scr.tile([128, 512], F32, tag="lntmp", name="lntmp")
                nc.vector.tensor_tensor(tmp[:], xview(k), psb_mu[:],
                                        op=ALU.subtract)
                nc.vector.tensor_mul(dst_tiles[k][:, dst_cols], tmp[:], psb_r[:])

        def ln1_rows(x_in, nm, rcol_all):
            """Per-frame LN stats; negmu row (K=1 fold operand) + 1/sd cols."""
            for f in range(4):
                sl = slice(f * 512, (f + 1) * 512)
                ps_s = pp_row.tile([1, 512], F32, tag="row_s", name="ps_s")
                ps_q = pp_row.tile([1, 512], F32, tag="row_q", name="ps_q")
                for k in range(4):
                    nc.tensor.matmul(ps_s[:], onescol[:], x_in[k][:, sl],
                                     start=(k == 0), stop=(k == 3))
                for k in range(4):
                    sq = scr.tile([128, 512], F32R, tag="lnsq", name="lnsq")
                    eng = nc.vector if k % 2 else nc.gpsimd
                    eng.tensor_mul(sq[:], x_in[k][:, sl], x_in[k][:, sl])
                    nc.tensor.matmul(ps_q[:], onescol[:], sq[:],
                                     start=(k == 0), stop=(k == 3))
                e2 = rows_p.tile([1, 512], F32, tag="e2", name="le2", bufs=2)
                rr = rows_p.tile([1, 512], F32, tag="rr", name="lrr", bufs=4)
                nc.scalar.mul(e2[:], ps_q[:], 1.0 / D)
                with nc.allow_low_precision(reason="ln1 rows"):
                    nc.scalar.mul(nm[0:1, sl], ps_s[:], -1.0 / D)
                    msq = rows_p.tile([1, 512], F32, tag="rr", name="lms", bufs=4)
                    nc.vector.tensor_mul(msq[:], nm[0:1, sl], nm[0:1, sl])
                    nc.vector.scalar_tensor_tensor(
                        e2[:], e2[:], LNEPS, msq[:], op0=ALU.add,
                        op1=ALU.subtract)
                    nc.scalar.activation(e2[:], e2[:], AF.Ln)
                    nc.scalar.activation(rr[:], e2[:], AF.Exp, scale=-0.5)
                for st in range(4):
                    nc.sync.dma_start(
                        rcol_all[:, f * 4 + st:f * 4 + st + 1],
                        rr[0:1, st * 128:(st + 1) * 128])

        tr_ctx = ctx.enter_context(contextlib.ExitStack())
        wp = tr_ctx.enter_context(tc.tile_pool(name="wp", bufs=1))
        wqp = tr_ctx.enter_context(tc.tile_pool(name="wqp", bufs=1))
        attn_p = tr_ctx.enter_context(tc.tile_pool(name="attn", bufs=2))
        sl_p = tr_ctx.enter_context(tc.tile_pool(name="slices", bufs=1))
        ot_p = tr_ctx.enter_context(tc.tile_pool(name="otp", bufs=1))
        otb_all = ot_p.tile([128, 4 * TOK], BF16, tag="otall", name="otall")

        def st_view(xt, k, st):
            # scattered columns {f*512 + st*128 + p} as (128, (f,p)=512)
            return xt[k][:].rearrange("p (f s) -> p f s", f=4)[:, :, st * 128:(st + 1) * 128]

        def transformer_layer(li, x_in, x_mid):
            wq = [wqp.tile([128, 3 * D], F32R, tag=f"wqkv{k}", name=f"wq{k}")
                  for k in range(4)]
            for k in range(4):
                nc.sync.dma_start(wq[k][:], wqkv_d[li, k * 128:(k + 1) * 128, :])
            ws_t = rows_p.tile([1, 3 * D], BF16, tag="wsum", name="wst", bufs=1)
            nc.sync.dma_start(ws_t[:], wsbq_d[li, 0:1, :])
            bq_t = rows_p.tile([1, 3 * D], BF16, tag="bqr", name="bqt", bufs=1)
            nc.sync.dma_start(bq_t[:], wsbq_d[li, 1:2, :])
            cols = stat_p.tile([128, 12], F32, tag="tcols", name="tcols")
            nc.sync.dma_start(cols[:], tcols_d[li])

            # q bias broadcast over the token partitions (k-bias is
            # softmax-invariant; v-bias is folded into outb on the host)
            bias_bc = attn_p.tile([128, D], BF16, tag="biasbc",
                                  name="bias_bc", bufs=1)
            psb = pp_bc.tile([128, 512], F32, tag="bc", name="psbb")
            nc.tensor.matmul(psb[:], ones_bf[:], bq_t[:, 0:512],
                             start=True, stop=True)
            nc.scalar.copy(bias_bc[:], psb[:])

            nm = attn_p.tile([1, TOK], BF16, tag="nmsd", name="nm", bufs=1)
            rcol = attn_p.tile([128, 16], F32, tag="rcol", name="rcol", bufs=1)
            ln1_rows(x_in, nm, rcol)

            for st in range(4):
                qt = attn_p.tile([128, TOK], BF16, tag="qst", name="qt", bufs=2)
                kt = attn_p.tile([128, TOK], BF16, tag="kst", name="kt", bufs=2)
                # v stored (j, d, h) so the AV multiply hits the 2x DVE mode
                vt = attn_p.tile([128, TOK], BF16, tag="vst", name="vt", bufs=2)
                for f in range(NF):
                    c0 = f * 512 + st * 128
                    for ns in range(3):
                        ps = pp_mm.tile([128, 512], F32, tag="mm", name="qps")
                        for k in range(4):
                            nc.tensor.matmul(
                                ps[:], x_in[k][:, c0:c0 + 128],
                                wq[k][:, ns * 512:(ns + 1) * 512],
                                start=(k == 0), stop=False)
                        nc.tensor.matmul(ps[:], nm[0:1, c0:c0 + 128],
                                         ws_t[:, ns * 512:(ns + 1) * 512],
                                         start=False, stop=True)
                        rc = rcol[:, f * 4 + st:f * 4 + st + 1]
                        if ns < 2:
                            nc.scalar.activation(
                                (qt if ns == 0 else kt)[:, f * 512:(f + 1) * 512],
                                ps[:], AF.Copy, scale=rc)
                        else:
                            nc.scalar.activation(
                                vt[:, f * 512:(f + 1) * 512]
                                .rearrange("p (d h) -> p h d", h=8),
                                ps[:].rearrange("p (h d) -> p h d", h=8),
                                AF.Copy, scale=rc)
                # q bias, broadcast over frames, one 2x-mode op per st
                nc.vector.tensor_tensor(
                    qt[:].rearrange("p (f c) -> p f c", f=4),
                    qt[:].rearrange("p (f c) -> p f c", f=4),
                    bias_bc[:].unsqueeze(1).broadcast_to([128, 4, 512]),
                    op=ALU.add)

                s_sc = attn_p.tile([128, 128], F32, tag="s_sc", name="s_sc",
                                   bufs=2)
                k4 = kt[:].rearrange("p (j hd) -> p j hd", j=4)
                for i in range(4):
                    pbig = attn_p.tile([128, TOK], BF16, tag="pbig",
                                       name="pbig", bufs=2)
                    qi = qt[:, i * 512:(i + 1) * 512].unsqueeze(1) \
                        .broadcast_to([128, 4, 512])
                    nc.vector.tensor_mul(
                        pbig[:].rearrange("p (j hd) -> p j hd", j=4), qi, k4)
                    st1 = attn_p.tile([128, TOK // 2], BF16, tag="qks1",
                                      name="qks1", bufs=2)
                    pv = pbig[:].rearrange("p (g d) -> p g d", g=32)
                    nc.vector.tensor_tensor(
                        st1[:].rearrange("p (g d) -> p g d", g=32),
                        pv[:, :, 0:32], pv[:, :, 32:64], op=ALU.add)
                    nc.vector.tensor_reduce(
                        s_sc[:, i * 32:(i + 1) * 32],
                        st1[:].rearrange("p (g d) -> p g d", g=32),
                        axis=AX.X, op=ALU.add)
                # softmax over j without max-subtraction (logits bounded)
                # S cols = (i, j, h)
                eexp = attn_p.tile([128, 128], BF16, tag="eexp", name="eexp",
                                   bufs=2)
                nc.scalar.activation(eexp[:], s_sc[:], AF.Exp)
                z = attn_p.tile([128, 32], F32, tag="z", name="zt", bufs=2)
                nc.vector.tensor_reduce(
                    z[:].rearrange("p (i h) -> p i h", i=4),
                    eexp[:].rearrange("p (i j h) -> p i h j", i=4, j=4),
                    axis=AX.X, op=ALU.add)
                zr = attn_p.tile([128, 32], F32, tag="zr", name="zr", bufs=2)
                nc.vector.reciprocal(zr[:], z[:])
                a_t = attn_p.tile([128, 128], BF16, tag="a_t", name="a_t",
                                  bufs=2)
                nc.vector.tensor_mul(
                    a_t[:].rearrange("p (i j h) -> p i j h", i=4, j=4),
                    eexp[:].rearrange("p (i j h) -> p i j h", i=4, j=4),
                    zr[:].rearrange("p (i h) -> p i h", i=4).unsqueeze(2)
                    .broadcast_to([128, 4, 4, 8]))
                for i in range(4):
                    tbig = attn_p.tile([128, TOK], BF16, tag="tbig", name="tbig", bufs=2)
                    ablk = a_t[:, i * 32:(i + 1) * 32] \
                        .rearrange("p (j h) -> p j h", j=4) \
                        .unsqueeze(2).broadcast_to([128, 4, 64, 8])
                    nc.vector.tensor_mul(
                        tbig[:].rearrange("p (j d h) -> p j d h", j=4, d=64),
                        vt[:].rearrange("p (j d h) -> p j d h", j=4, d=64),
                        ablk)
                    av01 = attn_p.tile([128, 512], F32, tag="av01", name="av01",
                                       bufs=2)
                    av23 = attn_p.tile([128, 512], F32, tag="av23", name="av23",
                                       bufs=2)
                    av = attn_p.tile([128, 512], BF16, tag="av", name="av",
                                     bufs=2)
                    nc.gpsimd.tensor_tensor(av01[:], tbig[:, 0:512],
                                            tbig[:, 512:1024], op=ALU.add)
                    nc.gpsimd.tensor_tensor(av23[:], tbig[:, 1024:1536],
                                            tbig[:, 1536:2048], op=ALU.add)
                    nc.gpsimd.tensor_tensor(av[:], av01[:], av23[:],
                                            op=ALU.add)
                    pst = pp_bc.tile([128, 512], BF16, tag="bc", name="pst")
                    for c in range(4):
                        nc.tensor.transpose(pst[:, c * 128:(c + 1) * 128],
                                            av[:, c * 128:(c + 1) * 128],
                                            ident[:])
                    nc.scalar.copy(
                        otb_all[:].rearrange("p (c t) -> p c t", c=4)
                        [:, :, i * 512 + st * 128:i * 512 + st * 128 + 128],
                        pst[:].rearrange("p (c t) -> p c t", c=4))

            wo = [wp.tile([128, D], BF16, tag=f"wo{k}", name=f"wo{k}")
                  for k in range(4)]
            for k in range(4):
                nc.sync.dma_start(wo[k][:], wo_d[li, k * 128:(k + 1) * 128, :])
            for m in range(4):
                for ns in range(4):
                    ps = pp_mm.tile([128, 512], F32, tag="mm", name="ops")
                    for k in range(4):
                        nc.tensor.matmul(
                            ps[:], wo[k][:, m * 128:(m + 1) * 128],
                            otb_all[:, k * TOK + ns * 512:k * TOK + (ns + 1) * 512],
                            start=(k == 0), stop=(k == 3))
                    nc.vector.scalar_tensor_tensor(
                        x_mid[m][:, ns * 512:(ns + 1) * 512], ps[:],
                        cols[:, 4 + m:5 + m], x_in[m][:, ns * 512:(ns + 1) * 512],
                        op0=ALU.add, op1=ALU.add)

            w1 = [wp.tile([128, D], F32R, tag=f"w1_{k}", name=f"w1_{k}")
                  for k in range(4)]
            w2 = [wp.tile([128, D], F32R, tag=f"w2_{k}", name=f"w2_{k}")
                  for k in range(4)]
            for k in range(4):
                nc.sync.dma_start(w1[k][:], w1_d[li, k * 128:(k + 1) * 128, :])
                nc.sync.dma_start(w2[k][:], w2_d[li, k * 128:(k + 1) * 128, :])
            for ns in range(4):
                xh2 = [sl_p.tile([128, 512], F32R, tag=f"xh2_{k}", name=f"xh2_{k}")
                       for k in range(4)]
                ln_cols(x_mid,
                        lambda k: x_mid[k][:, ns * 512:(ns + 1) * 512],
                        xh2, slice(0, 512))
                hsl = [sl_p.tile([128, 512], F32R, tag=f"h_{m}", name=f"hsl{m}")
                       for m in range(4)]
                for m in range(4):
                    ps = pp_mm.tile([128, 512], F32, tag="mm", name="m1ps")
                    for k in range(4):
                        nc.tensor.matmul(
                            ps[:], w1[k][:, m * 128:(m + 1) * 128], xh2[k][:],
                            start=(k == 0), stop=(k == 3))
                    nc.scalar.activation(hsl[m][:], ps[:], AF.Gelu_apprx_tanh,
                                         bias=cols[:, m:m + 1])
                for m in range(4):
                    ps = pp_mm.tile([128, 512], F32, tag="mm", name="m2ps")
                    for k in range(4):
                        nc.tensor.matmul(
                            ps[:], w2[k][:, m * 128:(m + 1) * 128], hsl[k][:],
                            start=(k == 0), stop=(k == 3))
                    nc.vector.scalar_tensor_tensor(
                        x_mid[m][:, ns * 512:(ns + 1) * 512], ps[:],
                        cols[:, 8 + m:9 + m], x_mid[m][:, ns * 512:(ns + 1) * 512],
                        op0=ALU.add, op1=ALU.add)

        cur = xA
        _nl = int(os.environ.get("KNLAYERS", NLAYER))
        for li in range(_nl):
            transformer_layer(li, cur, cur)

        tr_ctx.close()

        # ------------------------------------------------------------------
        # heads
        # ------------------------------------------------------------------
        _skip_heads = os.environ.get("KHEADS", "1") == "0"
        if _skip_heads:
            nc.gpsimd.dma_start(d6_o[:], cur[0][0:8, :])
            nc.gpsimd.dma_start(tr_o[:], cur[1][0:8, :])
        with tc.tile_pool(name="heads", bufs=1) as hp, \
             tc.tile_pool(name="whp", bufs=1) as whp:
          if not _skip_heads:
              hc = stat_p.tile([128, 10], F32, tag="hcols", name="hc")
              nc.sync.dma_start(hc[:], hcols_d[:])
              sb3 = stat_p.tile([8, 2], F32, tag="sb3", name="sb3")
              nc.sync.dma_start(sb3[:], sb3_d[:])

              xhf = [hp.tile([128, TOK], F32R, tag=f"xhf{k}", name=f"xhf{k}")
                     for k in range(4)]
              for st in range(4):
                  ln_cols(cur, lambda k: cur[k][:, st * 512:(st + 1) * 512],
                          xhf, slice(st * 512, (st + 1) * 512))

              xp = [hp.tile([128, TOK], F32R, tag=f"xp{k}", name=f"xp{k}")
                    for k in range(4)]

              def mm_head(src_tiles, wt_dram, kdim, mdim, dst_tiles, evict):
                  KC = kdim // 128
                  MC = max(mdim // 128, 1)
                  wsb = [whp.tile([128, mdim], F32R, tag=f"wh_{kdim}_{mdim}_{k}",
                                  name=f"wh{k}") for k in range(KC)]
                  for k in range(KC):
                      nc.sync.dma_start(wsb[k][:], wt_dram[k * 128:(k + 1) * 128, :])
                  for m in range(MC):
                      for ns in range(4):
                          ps = pp_mm.tile([128, 512], F32, tag="mm", name="hps")
                          for k in range(KC):
                              nc.tensor.matmul(
                                  ps[:], wsb[k][:, m * 128:(m + 1) * 128],
                                  src_tiles[k][:, ns * 512:(ns + 1) * 512],
                                  start=(k == 0), stop=(k == KC - 1))
                          evict(ps, dst_tiles[m], m, ns)

              mm_head(xhf, projw_d, D, D, xp,
                      lambda ps, dst, m, ns: nc.scalar.activation(
                          dst[:, ns * 512:(ns + 1) * 512], ps[:], AF.Identity,
                          bias=hc[:, m:m + 1]))

              def branch(w1d, w2d, w3d, b1ofs, b2ofs, out_dram, b3col, r1, r2, pfx):
                  mm_head(xp, w1d, D, 256, r1,
                          lambda ps, dst, m, ns: nc.scalar.activation(
                              dst[:, ns * 512:(ns + 1) * 512], ps[:], AF.Relu,
                              bias=hc[:, b1ofs + m:b1ofs + m + 1]))
                  mm_head(r1, w2d, 256, 128, r2,
                          lambda ps, dst, m, ns: nc.scalar.activation(
                              dst[:, ns * 512:(ns + 1) * 512], ps[:], AF.Relu,
                              bias=hc[:, b2ofs:b2ofs + 1]))
                  w3 = whp.tile([128, 8], F32R, tag=f"w3{pfx}", name="w3")
                  nc.sync.dma_start(w3[:], w3d[:])
                  out_sb = hp.tile([8, TOK], F32, tag=f"{pfx}out", name=f"{pfx}out")
                  for ns in range(4):
                      ps = pp_mm.tile([8, 512], F32, tag="mm", name="bps")
                      nc.tensor.matmul(ps[:], w3[:], r2[0][:, ns * 512:(ns + 1) * 512],
                                       start=True, stop=True)
                      nc.scalar.activation(out_sb[:, ns * 512:(ns + 1) * 512], ps[:],
                                           AF.Identity, bias=b3col)
                  nc.sync.dma_start(out_dram[:], out_sb[:])

              # reuse dead transformer buffers for intermediates
              branch(rw1_d, rw2_d, rw3_d, 4, 8, d6_o, sb3[:, 0:1],
                     [cur[0], cur[1]], [xhf[0]], "r")
              branch(tw1_d, tw2_d, tw3_d, 6, 9, tr_o, sb3[:, 1:2],
                     [cur[2], cur[3]], [xhf[1]], "t")

    nc.compile()
    return nc


# ----------------------------------------------------------------------------
# host side
# ----------------------------------------------------------------------------

_CACHE = {}


def _normalize_np(v, eps=1e-12):
    return v / np.maximum(np.linalg.norm(v, axis=-1, keepdims=True), eps)


def _rot6d_np(d6):
    a1, a2 = d6[..., :3], d6[..., 3:]
    b1 = _normalize_np(a1)
    b2 = _normalize_np(a2 - np.sum(b1 * a2, -1, keepdims=True) * b1)
    b3 = np.cross(b1, b2)
    return np.stack([b1, b2, b3], axis=-2)


def _prep_weights(inp):
    f32 = np.float32
    wmap = {}
    for i, cw in enumerate(['c1w', 'c2w', 'c3w', 'c4w']):
        wmap[f'convw{i}'] = np.ascontiguousarray(inp[cw].T.astype(f32))
    for i, (g, b2) in enumerate([('bn1g', 'bn1b'), ('bn2g', 'bn2b'),
                                 ('bn3g', 'bn3b')]):
        M = CONV_DIMS[i + 1] // 128
        bn = np.concatenate([
            inp[g].reshape(M, 128).T, inp[b2].reshape(M, 128).T,
            inp[f'c{i + 1}b'].reshape(M, 128).T], axis=1)
        wmap[f'bnconst{i}'] = np.ascontiguousarray(bn.astype(f32))
    pe = _pe_table()[:NF]
    b4 = inp['c4b'][None, :].astype(f32) + pe                   # (4, 512)
    # cols: m*4 + pt ; frame index == pt
    wmap['bias4'] = np.ascontiguousarray(
        b4.reshape(NF, 4, 128).transpose(2, 1, 0).reshape(128, 16).astype(f32))

    qkvw = np.array(inp['qkvw'], f32)
    qkvb = np.array(inp['qkvb'], f32)
    qkvw[:, :, :512] /= math.sqrt(DH)
    qkvb[:, :512] /= math.sqrt(DH)
    g1 = np.array(inp['ln1g'], f32)
    b1 = np.array(inp['ln1b'], f32)
    wq_fold = g1[:, :, None] * qkvw
    bq_fold = qkvb + np.einsum('ld,ldf->lf', b1, qkvw)
    wmap['wqkv'] = np.ascontiguousarray(wq_fold.astype(f32))
    wsbq = np.concatenate([wq_fold.sum(axis=1, keepdims=True),
                           bq_fold[:, None, :]], axis=1)         # (L, 2, 3D)
    wmap['wsbq'] = np.ascontiguousarray(wsbq.astype(ml_dtypes.bfloat16))
    # attention output features are (d, h)-ordered; permute wo rows to match
    wo_ = np.array(inp['outw'], f32)                             # (L, 512, 512)
    d_idx, h_idx = np.meshgrid(np.arange(DH), np.arange(HEADS), indexing='ij')
    perm = (h_idx * DH + d_idx).reshape(512)     # perm[d*8+h] = h*64+d
    wmap['wo'] = np.ascontiguousarray(wo_[:, perm, :]
                                      .astype(ml_dtypes.bfloat16))
    g2 = np.array(inp['ln2g'], f32)
    bl2 = np.array(inp['ln2b'], f32)
    m1w = np.array(inp['m1w'], f32)
    w1_fold = g2[:, :, None] * m1w
    b1_fold = np.array(inp['m1b'], f32) + np.einsum('ld,ldf->lf', bl2, m1w)
    wmap['w1'] = np.ascontiguousarray(w1_fold.astype(f32))
    wmap['w2'] = np.ascontiguousarray(np.array(inp['m2w'], f32))
    cols = np.zeros((NLAYER, 128, 12), f32)
    cols[:, :, 0:4] = b1_fold.reshape(NLAYER, 4, 128).transpose(0, 2, 1)
    # v-bias is dropped at the v eviction; fold bv @ Wo into outb instead
    bv = bq_fold[:, 2 * 512:3 * 512]                             # (L, 512)
    outb_fold = np.array(inp['outb'], f32) + np.einsum('lk,lko->lo', bv, wo_)
    cols[:, :, 4:8] = outb_fold.reshape(NLAYER, 4, 128).transpose(0, 2, 1)
    cols[:, :, 8:12] = np.array(inp['m2b'], f32).reshape(NLAYER, 4, 128) \
        .transpose(0, 2, 1)
    wmap['tcols'] = cols

    gf_ = np.array(inp['lnfg'], f32)
    bf_ = np.array(inp['lnfb'], f32)
    projw = np.array(inp['projw'], f32)
    wmap['projw'] = np.ascontiguousarray(gf_[:, None] * projw)
    projb_fold = np.array(inp['projb'], f32) + bf_ @ projw
    wmap['rw1'] = np.ascontiguousarray(np.array(inp['rw1'], f32))
    wmap['rw2'] = np.ascontiguousarray(np.array(inp['rw2'], f32))
    rw3 = np.zeros((128, 8), f32)
    rw3[:, :6] = np.array(inp['rw3'], f32)
    wmap['rw3'] = rw3
    wmap['tw1'] = np.ascontiguousarray(np.array(inp['tw1'], f32))
    wmap['tw2'] = np.ascontiguousarray(np.array(inp['tw2'], f32))
    tw3 = np.zeros((128, 8), f32)
    tw3[:, :3] = np.array(inp['tw3'], f32)
    wmap['tw3'] = tw3
    hcols = np.zeros((128, 10), f32)
    hcols[:, 0:4] = projb_fold.reshape(4, 128).T
    hcols[:, 4:6] = np.array(inp['rb1'], f32).reshape(2, 128).T
    hcols[:, 6:8] = np.array(inp['tb1'], f32).reshape(2, 128).T
    hcols[:, 8] = np.array(inp['rb2'], f32)
    hcols[:, 9] = np.array(inp['tb2'], f32)
    wmap['hcols'] = hcols
    sb3 = np.zeros((8, 2), f32)
    sb3[0:6, 0] = np.array(inp['rb3'], f32)
    sb3[0:3, 1] = np.array(inp['tb3'], f32)
    wmap['sb3'] = sb3
    wmap['ones_c'] = np.ones((128, 128), f32)
    return wmap


def kernel(**inputs):
    inp = {k: np.asarray(v) for k, v in inputs.items()}

    idx = inp['seed_idxs'].reshape(B, -1).astype(np.int64)      # (B, N)
    sel_seed = np.take_along_axis(np.asarray(inp['fp2_features'], np.float32),
                                  idx[:, None, :], axis=2)
    sel_grasp = np.take_along_axis(np.asarray(inp['local_grasp_features'], np.float32),
                                   idx[:, None, :], axis=2)
    sel_color = np.take_along_axis(np.asarray(inp['local_color_features'], np.float32),
                                   idx[:, None, :], axis=2)
    sel_pose = np.take_along_axis(np.asarray(inp['grasp_pose_feature'], np.float32),
                                  idx[:, None, :], axis=2)
    gsf = np.asarray(inp['sa4_features'], np.float32).max(axis=-1)
    gsf = np.broadcast_to(gsf[:, :, None], (B, 256, NPTS))
    fused = sel_pose + np.concatenate([sel_grasp, sel_color, sel_seed, gsf], axis=1)
    gf = fused.reshape(BE, FRAME, 1024, NPTS)
    cond = np.broadcast_to(gf[:, :1], (BE, NF, 1024, NPTS))
    X = np.concatenate([cond, gf[:, 1:]], axis=2)               # (e, f, 2048, N)

    if 'nc' not in _CACHE:
        _CACHE['nc'] = build_kernel()
    nc = _CACHE['nc']
    wmap = _prep_weights(inp)

    in_maps = []
    for k in range(NCORES):
        xc = X[:, :, :, k * NPC:(k + 1) * NPC]                  # (e, f, c, n)
        xc = xc.transpose(2, 1, 0, 3).reshape(2048, TOK)        # (c, (f,e,n))
        m = dict(wmap)
        m['xin'] = np.ascontiguousarray(xc, dtype=np.float32)
        in_maps.append(m)

    res = run_bass_kernel_spmd(nc, in_maps, core_ids=list(range(NCORES)))

    out = np.zeros((BE * NPTS, NF, 12), np.float32)
    for k in range(NCORES):
        d6 = res.results[k]['d6'][:6]
        tr = res.results[k]['tr3'][:3]
        d6 = d6.reshape(6, NF, BE, NPC).transpose(2, 3, 1, 0)   # (e, n, f, 6)
        tr = tr.reshape(3, NF, BE, NPC).transpose(2, 3, 1, 0)
        rot = _rot6d_np(d6).reshape(BE, NPC, NF, 9)
        for e in range(BE):
            rows = slice(e * NPTS + k * NPC, e * NPTS + (k + 1) * NPC)
            out[rows, :, 0:3] = tr[e]
            out[rows, :, 3:12] = rot[e]
    return out


if __name__ == "__main__":
    build_kernel()
    print("built ok")



# revision 66
# speedup vs baseline: 1.0703x; 1.0448x over previous
import math
import os
import numpy as np
import ml_dtypes
import contextlib

import concourse.bass as bass
import concourse.tile as tile
from concourse import bacc, mybir, masks
from concourse.bass_utils import run_bass_kernel_spmd

F32 = mybir.dt.float32
F32R = mybir.dt.float32r
BF16 = mybir.dt.bfloat16
ALU = mybir.AluOpType
AF = mybir.ActivationFunctionType
AX = mybir.AxisListType

NCORES = 8
FRAME = 5
NF = FRAME - 1
D = 512
DH = 64
HEADS = 8
B = 20
NPTS = 1024
BE = B // FRAME
NPC = NPTS // NCORES     # 128 points per core
TOK = NF * BE * NPC      # 2048 tokens per core
NLAYER = 12
LNEPS = 1e-5
BNEPS = 1e-5
NBN = 16 * NPTS

CONV_DIMS = [2048, 1536, 1024, 768, 512]


def _pe_table(max_len=16, d=D):
    pos = np.arange(max_len, dtype=np.float32)[:, None]
    div = np.exp(np.arange(0, d, 2, dtype=np.float32) * (-math.log(10000.0) / d))
    pe = np.zeros((max_len, d), np.float32)
    pe[:, 0::2] = np.sin(pos * div)
    pe[:, 1::2] = np.cos(pos * div)
    return pe


def build_kernel():
    nc = bacc.Bacc("TRN2", target_bir_lowering=False, debug=False,
                   num_devices=NCORES)

    xin = nc.dram_tensor("xin", [CONV_DIMS[0], TOK], F32R, kind="ExternalInput").ap()
    convw = [nc.dram_tensor(f"convw{i}", [CONV_DIMS[i], CONV_DIMS[i + 1]], F32R,
                            kind="ExternalInput").ap() for i in range(4)]
    bnconst = [nc.dram_tensor(f"bnconst{i}", [128, 3 * (CONV_DIMS[i + 1] // 128)],
                              F32, kind="ExternalInput").ap() for i in range(3)]
    bias4 = nc.dram_tensor("bias4", [128, 4 * NF], F32, kind="ExternalInput").ap()

    wqkv_d = nc.dram_tensor("wqkv", [NLAYER, D, 3 * D], F32R, kind="ExternalInput").ap()
    wsbq_d = nc.dram_tensor("wsbq", [NLAYER, 2, 3 * D], BF16, kind="ExternalInput").ap()
    wo_d = nc.dram_tensor("wo", [NLAYER, D, D], BF16, kind="ExternalInput").ap()
    w1_d = nc.dram_tensor("w1", [NLAYER, D, D], F32R, kind="ExternalInput").ap()
    w2_d = nc.dram_tensor("w2", [NLAYER, D, D], F32R, kind="ExternalInput").ap()
    tcols_d = nc.dram_tensor("tcols", [NLAYER, 128, 12], F32, kind="ExternalInput").ap()

    projw_d = nc.dram_tensor("projw", [D, D], F32R, kind="ExternalInput").ap()
    rw1_d = nc.dram_tensor("rw1", [D, 256], F32R, kind="ExternalInput").ap()
    rw2_d = nc.dram_tensor("rw2", [256, 128], F32R, kind="ExternalInput").ap()
    rw3_d = nc.dram_tensor("rw3", [128, 8], F32R, kind="ExternalInput").ap()
    tw1_d = nc.dram_tensor("tw1", [D, 256], F32R, kind="ExternalInput").ap()
    tw2_d = nc.dram_tensor("tw2", [256, 128], F32R, kind="ExternalInput").ap()
    tw3_d = nc.dram_tensor("tw3", [128, 8], F32R, kind="ExternalInput").ap()
    hcols_d = nc.dram_tensor("hcols", [128, 10], F32, kind="ExternalInput").ap()
    sb3_d = nc.dram_tensor("sb3", [8, 2], F32, kind="ExternalInput").ap()
    ones_d = nc.dram_tensor("ones_c", [128, 128], F32, kind="ExternalInput").ap()

    d6_o = nc.dram_tensor("d6", [8, TOK], F32, kind="ExternalOutput").ap()
    tr_o = nc.dram_tensor("tr3", [8, TOK], F32, kind="ExternalOutput").ap()

    with tile.TileContext(nc) as tc, contextlib.ExitStack() as ctx:
        const_p = ctx.enter_context(tc.tile_pool(name="consts", bufs=1))
        onescol = const_p.tile([128, 1], F32R)
        onesrow = const_p.tile([1, 128], F32R)
        ident = const_p.tile([128, 128], BF16)
        nc.gpsimd.dma_start(onescol[:], ones_d[:, 0:1])
        nc.gpsimd.dma_start(onesrow[:], ones_d[0:1, :])
        ones_bf = const_p.tile([1, 128], BF16)
        nc.vector.memset(ones_bf[:], 1.0)
        masks.make_identity(nc, ident[:])

        xs_p = ctx.enter_context(tc.tile_pool(name="xstate", bufs=1))
        xA = [xs_p.tile([128, TOK], F32R, tag=f"xA{m}", name=f"xA{m}") for m in range(4)]

        stat_p = ctx.enter_context(tc.tile_pool(name="stats", bufs=1))
        dram_p = ctx.enter_context(tc.tile_pool(name="dramb", bufs=1, space="DRAM"))

        pp_mm = ctx.enter_context(tc.tile_pool(name="ppmm", bufs=4, space="PSUM"))
        pp_row = ctx.enter_context(tc.tile_pool(name="pprow", bufs=1, space="PSUM"))
        pp_bc = ctx.enter_context(tc.tile_pool(name="ppbc", bufs=2, space="PSUM"))

        y_dram = [dram_p.tile([CONV_DIMS[i], TOK], F32R, tag=f"ydram{i}", name=f"ydram{i}")
                  for i in range(1, 4)]

        # ------------------------------------------------------------------
        # conv stack (activations spilled to DRAM, BN applied on load)
        # ------------------------------------------------------------------
        b4sb = stat_p.tile([128, 4 * NF], F32, tag="b4")
        nc.sync.dma_start(b4sb[:], bias4[:])

        bn_s = {}
        bn_t = {}

        def conv_layer(li, wcp, cxp, pp_conv):
            kdim, mdim = CONV_DIMS[li - 1], CONV_DIMS[li]
            KC, MC = kdim // 128, mdim // 128
            src = xin if li == 1 else y_dram[li - 2]
            with_bn = li < 4
            if with_bn:
                sum_acc = stat_p.tile([128, MC * 4], F32, tag=f"sum{li}")
                sq_acc = stat_p.tile([128, MC * 4], F32, tag=f"sq{li}")
            cond_sb = None
            if li == 1:
                # channels 0:1024 repeat frame 0's features for all 4 frames;
                # compute their contribution once and add it at eviction
                KC = 8
                xc = cxp.tile([128, 8 * 512], F32R, tag="convc", name="convc",
                              bufs=1)
                nc.sync.dma_start(
                    xc[:].rearrange("p (k c) -> p k c", k=8),
                    src[0:1024, 0:512].rearrange("(k p) c -> p k c", p=128))
                cond_sb = [cxp.tile([128, 512], F32, tag=f"cond{m}",
                                    name=f"cond{m}") for m in range(MC)]
                for m in range(MC):
                    wslc = wcp.tile([128, 8 * 128], F32R, tag="wslc",
                                    name="wslc", bufs=3)
                    wvc = wslc[:].rearrange("p (k c) -> p k c", k=8)
                    nc.sync.dma_start(
                        wvc, convw[0][0:1024, m * 128:(m + 1) * 128]
                        .rearrange("(k p) c -> p k c", p=128))
                    ps = pp_conv.tile([128, 512], F32, tag="mm", name="ccps")
                    for k in range(8):
                        nc.tensor.matmul(
                            ps[:], wvc[:, k, :], xc[:, k * 512:(k + 1) * 512],
                            start=(k == 0), stop=(k == 7))
                    nc.scalar.copy(cond_sb[m][:], ps[:])
            for pt in range(4):
                xt = cxp.tile([128, KC * 512], F32R, tag="convx", name="convx",
                              bufs=3)
                nc.sync.dma_start(
                    xt[:].rearrange("p (k c) -> p k c", k=KC),
                    src[kdim - KC * 128:, pt * 512:(pt + 1) * 512]
                    .rearrange("(k p) c -> p k c", p=128))
                if li > 1:
                    s_p, t_p = bn_s[li - 1], bn_t[li - 1]
                    for k in range(KC):
                        nc.scalar.activation(
                            xt[:, k * 512:(k + 1) * 512],
                            xt[:, k * 512:(k + 1) * 512],
                            AF.Relu, bias=t_p[:, k:k + 1], scale=s_p[:, k:k + 1])
                for m in range(MC):
                    wsl = wcp.tile([128, KC * 128], F32R, tag="wsl", name="wsl",
                                   bufs=6)
                    wv = wsl[:].rearrange("p (k c) -> p k c", k=KC)
                    nc.sync.dma_start(
                        wv, convw[li - 1][kdim - KC * 128:,
                                          m * 128:(m + 1) * 128]
                        .rearrange("(k p) c -> p k c", p=128))
                    ps = pp_conv.tile([128, 512], F32, tag="mm", name="cps")
                    for k in range(KC):
                        nc.tensor.matmul(
                            ps[:], wv[:, k, :], xt[:, k * 512:(k + 1) * 512],
                            start=(k == 0), stop=(k == KC - 1))
                    if with_bn:
                        ot = cxp.tile([128, 512], F32R, tag="convot", name="cot",
                                      bufs=4)
                        if cond_sb is not None:
                            nc.vector.tensor_tensor(ot[:], ps[:],
                                                    cond_sb[m][:], op=ALU.add)
                            stats_src = ot[:]
                        else:
                            stats_src = ps[:]
                            nc.scalar.activation(
                                ot[:], ps[:], AF.Copy,
                                accum_out=sum_acc[:, m * 4 + pt:m * 4 + pt + 1])
                        sqs = cxp.tile([128, 512], BF16, tag="sqscr", name="sqs",
                                       bufs=4)
                        if cond_sb is not None:
                            nc.scalar.activation(
                                sqs[:], stats_src, AF.Copy,
                                accum_out=sum_acc[:, m * 4 + pt:m * 4 + pt + 1])
                        nc.scalar.activation(
                            sqs[:], stats_src, AF.Square,
                            accum_out=sq_acc[:, m * 4 + pt:m * 4 + pt + 1])
                        nc.sync.dma_start(
                            y_dram[li - 1][m * 128:(m + 1) * 128,
                                           pt * 512:(pt + 1) * 512], ot[:])
                    else:
                        nc.scalar.activation(
                            xA[m][:, pt * 512:(pt + 1) * 512], ps[:], AF.Identity,
                            bias=b4sb[:, m * 4 + pt:m * 4 + pt + 1])
            if not with_bn:
                return
            allin = stat_p.tile([128, 2 * MC], F32, tag=f"ain{li}", name="allin")
            nc.vector.tensor_reduce(
                allin[:, 0:MC], sum_acc[:].rearrange("p (m t) -> p m t", m=MC),
                axis=AX.X, op=ALU.add)
            nc.vector.tensor_reduce(
                allin[:, MC:2 * MC], sq_acc[:].rearrange("p (m t) -> p m t", m=MC),
                axis=AX.X, op=ALU.add)
            bin_ = dram_p.tile([128, 2 * MC], F32, tag=f"arin{li}", name="arin")
            bout = dram_p.tile([128, 2 * MC], F32, tag=f"arout{li}", name="arout")
            nc.sync.dma_start(bin_[:], allin[:])
            nc.gpsimd.collective_compute(
                "AllReduce", ALU.add, replica_groups=[list(range(NCORES))],
                ins=[bin_.opt()], outs=[bout.opt()])
            gl = stat_p.tile([128, 2 * MC], F32, tag=f"gl{li}", name="gl")
            nc.sync.dma_start(gl[:], bout[:])
            cst = stat_p.tile([128, 3 * MC], F32, tag=f"cst{li}", name="cst")
            nc.sync.dma_start(cst[:], bnconst[li - 1][:])
            mu = stat_p.tile([128, MC], F32, tag=f"mu{li}", name="bmu")
            var = stat_p.tile([128, MC], F32, tag=f"va{li}", name="bvar")
            s_t = stat_p.tile([128, MC], F32, tag=f"s{li}", name="bs")
            t_t = stat_p.tile([128, MC], F32, tag=f"t{li}", name="bt")
            nc.scalar.mul(mu[:], gl[:, 0:MC], 1.0 / NBN)
            nc.scalar.mul(var[:], gl[:, MC:2 * MC], 1.0 / NBN)
            msq = stat_p.tile([128, MC], F32, tag=f"ms{li}", name="bmsq")
            nc.vector.tensor_mul(msq[:], mu[:], mu[:])
            nc.vector.tensor_tensor(var[:], var[:], msq[:], op=ALU.subtract)
            nc.vector.tensor_scalar(var[:], var[:], BNEPS, None, op0=ALU.add)
            sd = stat_p.tile([128, MC], F32, tag=f"sd{li}", name="bsd")
            nc.scalar.activation(sd[:], var[:], AF.Sqrt)
            rsd = stat_p.tile([128, MC], F32, tag=f"rs{li}", name="brsd")
            nc.vector.reciprocal(rsd[:], sd[:])
            nc.vector.tensor_mul(s_t[:], rsd[:], cst[:, 0:MC])
            nc.vector.tensor_mul(t_t[:], mu[:], s_t[:])
            nc.vector.tensor_tensor(t_t[:], cst[:, MC:2 * MC], t_t[:],
                                    op=ALU.subtract)
            bn_s[li], bn_t[li] = s_t, t_t

        with tc.tile_pool(name="wcp", bufs=1) as wcp, \
             tc.tile_pool(name="cxp", bufs=1) as cxp:
            for li in (1, 2, 3, 4):
                conv_layer(li, wcp, cxp, pp_mm)

        # ------------------------------------------------------------------
        # transformer
        # ------------------------------------------------------------------
        rows_p = ctx.enter_context(tc.tile_pool(name="rows", bufs=1))
        scr = ctx.enter_context(tc.tile_pool(name="scratch", bufs=2))

        def ln_cols(xt, xview, dst_tiles, dst_cols):
            """LN per token over feature dim (stats + apply on DVE/Pool)."""
            ps_s = pp_row.tile([1, 512], F32, tag="row_s", name="ps_s")
            ps_q = pp_row.tile([1, 512], F32, tag="row_q", name="ps_q")
            for k in range(4):
                nc.tensor.matmul(ps_s[:], onescol[:], xview(k),
                                 start=(k == 0), stop=(k == 3))
            for k in range(4):
                sq = scr.tile([128, 512], F32R, tag="lnsq", name="lnsq")
                eng = nc.vector if k % 2 else nc.gpsimd
                eng.tensor_mul(sq[:], xview(k), xview(k))
                nc.tensor.matmul(ps_q[:], onescol[:], sq[:],
                                 start=(k == 0), stop=(k == 3))
            mu = rows_p.tile([1, 512], F32R, tag="mu", name="lmu", bufs=2)
            e2 = rows_p.tile([1, 512], F32, tag="e2", name="le2", bufs=2)
            r = rows_p.tile([1, 512], F32R, tag="r", name="lr", bufs=2)
            nc.scalar.mul(mu[:], ps_s[:], 1.0 / D)
            nc.scalar.mul(e2[:], ps_q[:], 1.0 / D)
            with nc.allow_low_precision(reason="f32r row math"):
                nc.vector.tensor_mul(r[:], mu[:], mu[:])
                nc.vector.scalar_tensor_tensor(
                    e2[:], e2[:], LNEPS, r[:], op0=ALU.add, op1=ALU.subtract)
                # 1/sqrt(v) = exp(-0.5 ln v): stays inside the exp/ln table
                nc.scalar.activation(e2[:], e2[:], AF.Ln)
                nc.scalar.activation(r[:], e2[:], AF.Exp, scale=-0.5)
            psb_mu = pp_bc.tile([128, 512], F32, tag="bc", name="psbmu")
            psb_r = pp_bc.tile([128, 512], F32, tag="bc", name="psbr")
            nc.tensor.matmul(psb_mu[:], onesrow[:], mu[:], start=True, stop=True)
            nc.tensor.matmul(psb_r[:], onesrow[:], r[:], start=True, stop=True)
            for k in range(4):
                tmp = scr.tile([128, 512], F32, tag="lntmp", name="lntmp")
                nc.vector.tensor_tensor(tmp[:], xview(k), psb_mu[:],
                                        op=ALU.subtract)
                nc.vector.tensor_mul(dst_tiles[k][:, dst_cols], tmp[:], psb_r[:])

        def ln1_rows(x_in, nm, rcol_all):
            """Per-frame LN stats; negmu row (K=1 fold operand) + 1/sd cols."""
            for f in range(4):
                sl = slice(f * 512, (f + 1) * 512)
                ps_s = pp_row.tile([1, 512], F32, tag="row_s", name="ps_s")
                ps_q = pp_row.tile([1, 512], F32, tag="row_q", name="ps_q")
                for k in range(4):
                    nc.tensor.matmul(ps_s[:], onescol[:], x_in[k][:, sl],
                                     start=(k == 0), stop=(k == 3))
                for k in range(4):
                    sq = scr.tile([128, 512], F32R, tag="lnsq", name="lnsq")
                    eng = nc.vector if k % 2 else nc.gpsimd
                    eng.tensor_mul(sq[:], x_in[k][:, sl], x_in[k][:, sl])
                    nc.tensor.matmul(ps_q[:], onescol[:], sq[:],
                                     start=(k == 0), stop=(k == 3))
                e2 = rows_p.tile([1, 512], F32, tag="e2", name="le2", bufs=2)
                rr = rows_p.tile([1, 512], F32, tag="rr", name="lrr", bufs=4)
                nc.scalar.mul(e2[:], ps_q[:], 1.0 / D)
                with nc.allow_low_precision(reason="ln1 rows"):
                    nc.scalar.mul(nm[0:1, sl], ps_s[:], -1.0 / D)
                    msq = rows_p.tile([1, 512], F32, tag="rr", name="lms", bufs=4)
                    nc.vector.tensor_mul(msq[:], nm[0:1, sl], nm[0:1, sl])
                    nc.vector.scalar_tensor_tensor(
                        e2[:], e2[:], LNEPS, msq[:], op0=ALU.add,
                        op1=ALU.subtract)
                    nc.scalar.activation(e2[:], e2[:], AF.Ln)
                    nc.scalar.activation(rr[:], e2[:], AF.Exp, scale=-0.5)
                for st in range(4):
                    nc.sync.dma_start(
                        rcol_all[:, f * 4 + st:f * 4 + st + 1],
                        rr[0:1, st * 128:(st + 1) * 128])

        tr_ctx = ctx.enter_context(contextlib.ExitStack())
        wp = tr_ctx.enter_context(tc.tile_pool(name="wp", bufs=1))
        wqp = tr_ctx.enter_context(tc.tile_pool(name="wqp", bufs=1))
        attn_p = tr_ctx.enter_context(tc.tile_pool(name="attn", bufs=2))
        sl_p = tr_ctx.enter_context(tc.tile_pool(name="slices", bufs=1))
        ot_p = tr_ctx.enter_context(tc.tile_pool(name="otp", bufs=1))
        otb_all = ot_p.tile([128, 4 * TOK], BF16, tag="otall", name="otall")

        def st_view(xt, k, st):
            # scattered columns {f*512 + st*128 + p} as (128, (f,p)=512)
            return xt[k][:].rearrange("p (f s) -> p f s", f=4)[:, :, st * 128:(st + 1) * 128]

        def transformer_layer(li, x_in, x_mid):
            wq = [wqp.tile([128, 3 * D], F32R, tag=f"wqkv{k}", name=f"wq{k}")
                  for k in range(4)]
            for k in range(4):
                nc.sync.dma_start(wq[k][:], wqkv_d[li, k * 128:(k + 1) * 128, :])
            ws_t = rows_p.tile([1, 3 * D], BF16, tag="wsum", name="wst", bufs=1)
            nc.sync.dma_start(ws_t[:], wsbq_d[li, 0:1, :])
            bq_t = rows_p.tile([1, 3 * D], BF16, tag="bqr", name="bqt", bufs=1)
            nc.sync.dma_start(bq_t[:], wsbq_d[li, 1:2, :])
            cols = stat_p.tile([128, 12], F32, tag="tcols", name="tcols")
            nc.sync.dma_start(cols[:], tcols_d[li])

            # q bias broadcast over the token partitions (k-bias is
            # softmax-invariant; v-bias is folded into outb on the host)
            bias_bc = attn_p.tile([128, D], BF16, tag="biasbc",
                                  name="bias_bc", bufs=1)
            psb = pp_bc.tile([128, 512], F32, tag="bc", name="psbb")
            nc.tensor.matmul(psb[:], ones_bf[:], bq_t[:, 0:512],
                             start=True, stop=True)
            nc.scalar.copy(bias_bc[:], psb[:])

            nm = attn_p.tile([1, TOK], BF16, tag="nmsd", name="nm", bufs=1)
            rcol = attn_p.tile([128, 16], F32, tag="rcol", name="rcol", bufs=1)
            ln1_rows(x_in, nm, rcol)

            for st in range(4):
                qt = attn_p.tile([128, TOK], BF16, tag="qst", name="qt", bufs=2)
                kt = attn_p.tile([128, TOK], BF16, tag="kst", name="kt", bufs=2)
                # v stored (j, d, h) so the AV multiply hits the 2x DVE mode
                vt = attn_p.tile([128, TOK], BF16, tag="vst", name="vt", bufs=2)
                for f in range(NF):
                    c0 = f * 512 + st * 128
                    for ns in range(3):
                        ps = pp_mm.tile([128, 512], F32, tag="mm", name="qps")
                        for k in range(4):
                            nc.tensor.matmul(
                                ps[:], x_in[k][:, c0:c0 + 128],
                                wq[k][:, ns * 512:(ns + 1) * 512],
                                start=(k == 0), stop=False)
                        nc.tensor.matmul(ps[:], nm[0:1, c0:c0 + 128],
                                         ws_t[:, ns * 512:(ns + 1) * 512],
                                         start=False, stop=True)
                        rc = rcol[:, f * 4 + st:f * 4 + st + 1]
                        if ns < 2:
                            nc.scalar.activation(
                                (qt if ns == 0 else kt)[:, f * 512:(f + 1) * 512],
                                ps[:], AF.Copy, scale=rc)
                        else:
                            nc.scalar.activation(
                                vt[:, f * 512:(f + 1) * 512]
                                .rearrange("p (d h) -> p h d", h=8),
                                ps[:].rearrange("p (h d) -> p h d", h=8),
                                AF.Copy, scale=rc)
                # q bias, broadcast over frames, one 2x-mode op per st
                nc.vector.tensor_tensor(
                    qt[:].rearrange("p (f c) -> p f c", f=4),
                    qt[:].rearrange("p (f c) -> p f c", f=4),
                    bias_bc[:].unsqueeze(1).broadcast_to([128, 4, 512]),
                    op=ALU.add)

                s_sc = attn_p.tile([128, 128], F32, tag="s_sc", name="s_sc",
                                   bufs=2)
                k4 = kt[:].rearrange("p (j hd) -> p j hd", j=4)
                for i in range(4):
                    pbig = attn_p.tile([128, TOK], BF16, tag="pbig",
                                       name="pbig", bufs=2)
                    qi = qt[:, i * 512:(i + 1) * 512].unsqueeze(1) \
                        .broadcast_to([128, 4, 512])
                    nc.vector.tensor_mul(
                        pbig[:].rearrange("p (j hd) -> p j hd", j=4), qi, k4)
                    st1 = attn_p.tile([128, TOK // 2], BF16, tag="qks1",
                                      name="qks1", bufs=2)
                    pv = pbig[:].rearrange("p (g d) -> p g d", g=32)
                    nc.vector.tensor_tensor(
                        st1[:].rearrange("p (g d) -> p g d", g=32),
                        pv[:, :, 0:32], pv[:, :, 32:64], op=ALU.add)
                    nc.vector.tensor_reduce(
                        s_sc[:, i * 32:(i + 1) * 32],
                        st1[:].rearrange("p (g d) -> p g d", g=32),
                        axis=AX.X, op=ALU.add)
                # softmax over j without max-subtraction (logits bounded)
                # S cols = (i, j, h)
                eexp = attn_p.tile([128, 128], BF16, tag="eexp", name="eexp",
                                   bufs=2)
                nc.scalar.activation(eexp[:], s_sc[:], AF.Exp)
                z = attn_p.tile([128, 32], F32, tag="z", name="zt", bufs=2)
                nc.vector.tensor_reduce(
                    z[:].rearrange("p (i h) -> p i h", i=4),
                    eexp[:].rearrange("p (i j h) -> p i h j", i=4, j=4),
                    axis=AX.X, op=ALU.add)
                zr = attn_p.tile([128, 32], F32, tag="zr", name="zr", bufs=2)
                nc.vector.reciprocal(zr[:], z[:])
                a_t = attn_p.tile([128, 128], BF16, tag="a_t", name="a_t",
                                  bufs=2)
                nc.vector.tensor_mul(
                    a_t[:].rearrange("p (i j h) -> p i j h", i=4, j=4),
                    eexp[:].rearrange("p (i j h) -> p i j h", i=4, j=4),
                    zr[:].rearrange("p (i h) -> p i h", i=4).unsqueeze(2)
                    .broadcast_to([128, 4, 4, 8]))
                for i in range(4):
                    tbig = attn_p.tile([128, TOK], BF16, tag="tbig", name="tbig", bufs=2)
                    ablk = a_t[:, i * 32:(i + 1) * 32] \
                        .rearrange("p (j h) -> p j h", j=4) \
                        .unsqueeze(2).broadcast_to([128, 4, 64, 8])
                    nc.vector.tensor_mul(
                        tbig[:].rearrange("p (j d h) -> p j d h", j=4, d=64),
                        vt[:].rearrange("p (j d h) -> p j d h", j=4, d=64),
                        ablk)
                    av01 = attn_p.tile([128, 512], BF16, tag="av01", name="av01",
                                       bufs=2)
                    av = attn_p.tile([128, 512], BF16, tag="av", name="av",
                                     bufs=2)
                    nc.vector.tensor_tensor(av01[:], tbig[:, 0:512],
                                            tbig[:, 512:1024], op=ALU.add)
                    nc.gpsimd.tensor_tensor(av[:], tbig[:, 1024:1536],
                                            tbig[:, 1536:2048], op=ALU.add)
                    nc.vector.tensor_tensor(av[:], av01[:], av[:],
                                            op=ALU.add)
                    pst = pp_bc.tile([128, 512], BF16, tag="bc", name="pst")
                    for c in range(4):
                        nc.tensor.transpose(pst[:, c * 128:(c + 1) * 128],
                                            av[:, c * 128:(c + 1) * 128],
                                            ident[:])
                    nc.scalar.copy(
                        otb_all[:].rearrange("p (c t) -> p c t", c=4)
                        [:, :, i * 512 + st * 128:i * 512 + st * 128 + 128],
                        pst[:].rearrange("p (c t) -> p c t", c=4))

            wo = [wp.tile([128, D], BF16, tag=f"wo{k}", name=f"wo{k}")
                  for k in range(4)]
            for k in range(4):
                nc.sync.dma_start(wo[k][:], wo_d[li, k * 128:(k + 1) * 128, :])
            for m in range(4):
                for ns in range(4):
                    ps = pp_mm.tile([128, 512], F32, tag="mm", name="ops")
                    for k in range(4):
                        nc.tensor.matmul(
                            ps[:], wo[k][:, m * 128:(m + 1) * 128],
                            otb_all[:, k * TOK + ns * 512:k * TOK + (ns + 1) * 512],
                            start=(k == 0), stop=(k == 3))
                    nc.vector.scalar_tensor_tensor(
                        x_mid[m][:, ns * 512:(ns + 1) * 512], ps[:],
                        cols[:, 4 + m:5 + m], x_in[m][:, ns * 512:(ns + 1) * 512],
                        op0=ALU.add, op1=ALU.add)

            w1 = [wp.tile([128, D], F32R, tag=f"w1_{k}", name=f"w1_{k}")
                  for k in range(4)]
            w2 = [wp.tile([128, D], F32R, tag=f"w2_{k}", name=f"w2_{k}")
                  for k in range(4)]
            for k in range(4):
                nc.sync.dma_start(w1[k][:], w1_d[li, k * 128:(k + 1) * 128, :])
                nc.sync.dma_start(w2[k][:], w2_d[li, k * 128:(k + 1) * 128, :])
            for ns in range(4):
                xh2 = [sl_p.tile([128, 512], F32R, tag=f"xh2_{k}", name=f"xh2_{k}")
                       for k in range(4)]
                ln_cols(x_mid,
                        lambda k: x_mid[k][:, ns * 512:(ns + 1) * 512],
                        xh2, slice(0, 512))
                hsl = [sl_p.tile([128, 512], F32R, tag=f"h_{m}", name=f"hsl{m}")
                       for m in range(4)]
                for m in range(4):
                    ps = pp_mm.tile([128, 512], F32, tag="mm", name="m1ps")
                    for k in range(4):
                        nc.tensor.matmul(
                            ps[:], w1[k][:, m * 128:(m + 1) * 128], xh2[k][:],
                            start=(k == 0), stop=(k == 3))
                    nc.scalar.activation(hsl[m][:], ps[:], AF.Gelu_apprx_tanh,
                                         bias=cols[:, m:m + 1])
                for m in range(4):
                    ps = pp_mm.tile([128, 512], F32, tag="mm", name="m2ps")
                    for k in range(4):
                        nc.tensor.matmul(
                            ps[:], w2[k][:, m * 128:(m + 1) * 128], hsl[k][:],
                            start=(k == 0), stop=(k == 3))
                    nc.vector.scalar_tensor_tensor(
                        x_mid[m][:, ns * 512:(ns + 1) * 512], ps[:],
                        cols[:, 8 + m:9 + m], x_mid[m][:, ns * 512:(ns + 1) * 512],
                        op0=ALU.add, op1=ALU.add)

        cur = xA
        _nl = int(os.environ.get("KNLAYERS", NLAYER))
        for li in range(_nl):
            transformer_layer(li, cur, cur)

        tr_ctx.close()

        # ------------------------------------------------------------------
        # heads
        # ------------------------------------------------------------------
        _skip_heads = os.environ.get("KHEADS", "1") == "0"
        if _skip_heads:
            nc.gpsimd.dma_start(d6_o[:], cur[0][0:8, :])
            nc.gpsimd.dma_start(tr_o[:], cur[1][0:8, :])
        with tc.tile_pool(name="heads", bufs=1) as hp, \
             tc.tile_pool(name="whp", bufs=1) as whp:
          if not _skip_heads:
              hc = stat_p.tile([128, 10], F32, tag="hcols", name="hc")
              nc.sync.dma_start(hc[:], hcols_d[:])
              sb3 = stat_p.tile([8, 2], F32, tag="sb3", name="sb3")
              nc.sync.dma_start(sb3[:], sb3_d[:])

              xhf = [hp.tile([128, TOK], F32R, tag=f"xhf{k}", name=f"xhf{k}")
                     for k in range(4)]
              for st in range(4):
                  ln_cols(cur, lambda k: cur[k][:, st * 512:(st + 1) * 512],
                          xhf, slice(st * 512, (st + 1) * 512))

              xp = [hp.tile([128, TOK], F32R, tag=f"xp{k}", name=f"xp{k}")
                    for k in range(4)]

              def mm_head(src_tiles, wt_dram, kdim, mdim, dst_tiles, evict):
                  KC = kdim // 128
                  MC = max(mdim // 128, 1)
                  wsb = [whp.tile([128, mdim], F32R, tag=f"wh_{kdim}_{mdim}_{k}",
                                  name=f"wh{k}") for k in range(KC)]
                  for k in range(KC):
                      nc.sync.dma_start(wsb[k][:], wt_dram[k * 128:(k + 1) * 128, :])
                  for m in range(MC):
                      for ns in range(4):
                          ps = pp_mm.tile([128, 512], F32, tag="mm", name="hps")
                          for k in range(KC):
                              nc.tensor.matmul(
                                  ps[:], wsb[k][:, m * 128:(m + 1) * 128],
                                  src_tiles[k][:, ns * 512:(ns + 1) * 512],
                                  start=(k == 0), stop=(k == KC - 1))
                          evict(ps, dst_tiles[m], m, ns)

              mm_head(xhf, projw_d, D, D, xp,
                      lambda ps, dst, m, ns: nc.scalar.activation(
                          dst[:, ns * 512:(ns + 1) * 512], ps[:], AF.Identity,
                          bias=hc[:, m:m + 1]))

              def branch(w1d, w2d, w3d, b1ofs, b2ofs, out_dram, b3col, r1, r2, pfx):
                  mm_head(xp, w1d, D, 256, r1,
                          lambda ps, dst, m, ns: nc.scalar.activation(
                              dst[:, ns * 512:(ns + 1) * 512], ps[:], AF.Relu,
                              bias=hc[:, b1ofs + m:b1ofs + m + 1]))
                  mm_head(r1, w2d, 256, 128, r2,
                          lambda ps, dst, m, ns: nc.scalar.activation(
                              dst[:, ns * 512:(ns + 1) * 512], ps[:], AF.Relu,
                              bias=hc[:, b2ofs:b2ofs + 1]))
                  w3 = whp.tile([128, 8], F32R, tag=f"w3{pfx}", name="w3")
                  nc.sync.dma_start(w3[:], w3d[:])
                  out_sb = hp.tile([8, TOK], F32, tag=f"{pfx}out", name=f"{pfx}out")
                  for ns in range(4):
                      ps = pp_mm.tile([8, 512], F32, tag="mm", name="bps")
                      nc.tensor.matmul(ps[:], w3[:], r2[0][:, ns * 512:(ns + 1) * 512],
                                       start=True, stop=True)
                      nc.scalar.activation(out_sb[:, ns * 512:(ns + 1) * 512], ps[:],
                                           AF.Identity, bias=b3col)
                  nc.sync.dma_start(out_dram[:], out_sb[:])

              # reuse dead transformer buffers for intermediates
              branch(rw1_d, rw2_d, rw3_d, 4, 8, d6_o, sb3[:, 0:1],
                     [cur[0], cur[1]], [xhf[0]], "r")
              branch(tw1_d, tw2_d, tw3_d, 6, 9, tr_o, sb3[:, 1:2],
                     [cur[2], cur[3]], [xhf[1]], "t")

    nc.compile()
    return nc


# ----------------------------------------------------------------------------
# host side
# ----------------------------------------------------------------------------

_CACHE = {}


def _normalize_np(v, eps=1e-12):
    return v / np.maximum(np.linalg.norm(v, axis=-1, keepdims=True), eps)


def _rot6d_np(d6):
    a1, a2 = d6[..., :3], d6[..., 3:]
    b1 = _normalize_np(a1)
    b2 = _normalize_np(a2 - np.sum(b1 * a2, -1, keepdims=True) * b1)
    b3 = np.cross(b1, b2)
    return np.stack([b1, b2, b3], axis=-2)


def _prep_weights(inp):
    f32 = np.float32
    wmap = {}
    for i, cw in enumerate(['c1w', 'c2w', 'c3w', 'c4w']):
        wmap[f'convw{i}'] = np.ascontiguousarray(inp[cw].T.astype(f32))
    for i, (g, b2) in enumerate([('bn1g', 'bn1b'), ('bn2g', 'bn2b'),
                                 ('bn3g', 'bn3b')]):
        M = CONV_DIMS[i + 1] // 128
        bn = np.concatenate([
            inp[g].reshape(M, 128).T, inp[b2].reshape(M, 128).T,
            inp[f'c{i + 1}b'].reshape(M, 128).T], axis=1)
        wmap[f'bnconst{i}'] = np.ascontiguousarray(bn.astype(f32))
    pe = _pe_table()[:NF]
    b4 = inp['c4b'][None, :].astype(f32) + pe                   # (4, 512)
    # cols: m*4 + pt ; frame index == pt
    wmap['bias4'] = np.ascontiguousarray(
        b4.reshape(NF, 4, 128).transpose(2, 1, 0).reshape(128, 16).astype(f32))

    qkvw = np.array(inp['qkvw'], f32)
    qkvb = np.array(inp['qkvb'], f32)
    qkvw[:, :, :512] /= math.sqrt(DH)
    qkvb[:, :512] /= math.sqrt(DH)
    g1 = np.array(inp['ln1g'], f32)
    b1 = np.array(inp['ln1b'], f32)
    wq_fold = g1[:, :, None] * qkvw
    bq_fold = qkvb + np.einsum('ld,ldf->lf', b1, qkvw)
    wmap['wqkv'] = np.ascontiguousarray(wq_fold.astype(f32))
    wsbq = np.concatenate([wq_fold.sum(axis=1, keepdims=True),
                           bq_fold[:, None, :]], axis=1)         # (L, 2, 3D)
    wmap['wsbq'] = np.ascontiguousarray(wsbq.astype(ml_dtypes.bfloat16))
    # attention output features are (d, h)-ordered; permute wo rows to match
    wo_ = np.array(inp['outw'], f32)                             # (L, 512, 512)
    d_idx, h_idx = np.meshgrid(np.arange(DH), np.arange(HEADS), indexing='ij')
    perm = (h_idx * DH + d_idx).reshape(512)     # perm[d*8+h] = h*64+d
    wmap['wo'] = np.ascontiguousarray(wo_[:, perm, :]
                                      .astype(ml_dtypes.bfloat16))
    g2 = np.array(inp['ln2g'], f32)
    bl2 = np.array(inp['ln2b'], f32)
    m1w = np.array(inp['m1w'], f32)
    w1_fold = g2[:, :, None] * m1w
    b1_fold = np.array(inp['m1b'], f32) + np.einsum('ld,ldf->lf', bl2, m1w)
    wmap['w1'] = np.ascontiguousarray(w1_fold.astype(f32))
    wmap['w2'] = np.ascontiguousarray(np.array(inp['m2w'], f32))
    cols = np.zeros((NLAYER, 128, 12), f32)
    cols[:, :, 0:4] = b1_fold.reshape(NLAYER, 4, 128).transpose(0, 2, 1)
    # v-bias is dropped at the v eviction; fold bv @ Wo into outb instead
    bv = bq_fold[:, 2 * 512:3 * 512]                             # (L, 512)
    outb_fold = np.array(inp['outb'], f32) + np.einsum('lk,lko->lo', bv, wo_)
    cols[:, :, 4:8] = outb_fold.reshape(NLAYER, 4, 128).transpose(0, 2, 1)
    cols[:, :, 8:12] = np.array(inp['m2b'], f32).reshape(NLAYER, 4, 128) \
        .transpose(0, 2, 1)
    wmap['tcols'] = cols

    gf_ = np.array(inp['lnfg'], f32)
    bf_ = np.array(inp['lnfb'], f32)
    projw = np.array(inp['projw'], f32)
    wmap['projw'] = np.ascontiguousarray(gf_[:, None] * projw)
    projb_fold = np.array(inp['projb'], f32) + bf_ @ projw
    wmap['rw1'] = np.ascontiguousarray(np.array(inp['rw1'], f32))
    wmap['rw2'] = np.ascontiguousarray(np.array(inp['rw2'], f32))
    rw3 = np.zeros((128, 8), f32)
    rw3[:, :6] = np.array(inp['rw3'], f32)
    wmap['rw3'] = rw3
    wmap['tw1'] = np.ascontiguousarray(np.array(inp['tw1'], f32))
    wmap['tw2'] = np.ascontiguousarray(np.array(inp['tw2'], f32))
    tw3 = np.zeros((128, 8), f32)
    tw3[:, :3] = np.array(inp['tw3'], f32)
    wmap['tw3'] = tw3
    hcols = np.zeros((128, 10), f32)
    hcols[:, 0:4] = projb_fold.reshape(4, 128).T
    hcols[:, 4:6] = np.array(inp['rb1'], f32).reshape(2, 128).T
    hcols[:, 6:8] = np.array(inp['tb1'], f32).reshape(2, 128).T
    hcols[:, 8] = np.array(inp['rb2'], f32)
    hcols[:, 9] = np.array(inp['tb2'], f32)
    wmap['hcols'] = hcols
    sb3 = np.zeros((8, 2), f32)
    sb3[0:6, 0] = np.array(inp['rb3'], f32)
    sb3[0:3, 1] = np.array(inp['tb3'], f32)
    wmap['sb3'] = sb3
    wmap['ones_c'] = np.ones((128, 128), f32)
    return wmap


def kernel(**inputs):
    inp = {k: np.asarray(v) for k, v in inputs.items()}

    idx = inp['seed_idxs'].reshape(B, -1).astype(np.int64)      # (B, N)
    sel_seed = np.take_along_axis(np.asarray(inp['fp2_features'], np.float32),
                                  idx[:, None, :], axis=2)
    sel_grasp = np.take_along_axis(np.asarray(inp['local_grasp_features'], np.float32),
                                   idx[:, None, :], axis=2)
    sel_color = np.take_along_axis(np.asarray(inp['local_color_features'], np.float32),
                                   idx[:, None, :], axis=2)
    sel_pose = np.take_along_axis(np.asarray(inp['grasp_pose_feature'], np.float32),
                                  idx[:, None, :], axis=2)
    gsf = np.asarray(inp['sa4_features'], np.float32).max(axis=-1)
    gsf = np.broadcast_to(gsf[:, :, None], (B, 256, NPTS))
    fused = sel_pose + np.concatenate([sel_grasp, sel_color, sel_seed, gsf], axis=1)
    gf = fused.reshape(BE, FRAME, 1024, NPTS)
    cond = np.broadcast_to(gf[:, :1], (BE, NF, 1024, NPTS))
    X = np.concatenate([cond, gf[:, 1:]], axis=2)               # (e, f, 2048, N)

    if 'nc' not in _CACHE:
        _CACHE['nc'] = build_kernel()
    nc = _CACHE['nc']
    wmap = _prep_weights(inp)

    in_maps = []
    for k in range(NCORES):
        xc = X[:, :, :, k * NPC:(k + 1) * NPC]                  # (e, f, c, n)
        xc = xc.transpose(2, 1, 0, 3).reshape(2048, TOK)        # (c, (f,e,n))
        m = dict(wmap)
        m['xin'] = np.ascontiguousarray(xc, dtype=np.float32)
        in_maps.append(m)

    res = run_bass_kernel_spmd(nc, in_maps, core_ids=list(range(NCORES)))

    out = np.zeros((BE * NPTS, NF, 12), np.float32)
    for k in range(NCORES):
        d6 = res.results[k]['d6'][:6]
        tr = res.results[k]['tr3'][:3]
        d6 = d6.reshape(6, NF, BE, NPC).transpose(2, 3, 1, 0)   # (e, n, f, 6)
        tr = tr.reshape(3, NF, BE, NPC).transpose(2, 3, 1, 0)
        rot = _rot6d_np(d6).reshape(BE, NPC, NF, 9)
        for e in range(BE):
            rows = slice(e * NPTS + k * NPC, e * NPTS + (k + 1) * NPC)
            out[rows, :, 0:3] = tr[e]
            out[rows, :, 3:12] = rot[e]
    return out


if __name__ == "__main__":
    build_kernel()
    print("built ok")



# revision 67
# speedup vs baseline: 1.0823x; 1.0112x over previous
import math
import os
import numpy as np
import ml_dtypes
import contextlib

import concourse.bass as bass
import concourse.tile as tile
from concourse import bacc, mybir, masks
from concourse.bass_utils import run_bass_kernel_spmd

F32 = mybir.dt.float32
F32R = mybir.dt.float32r
BF16 = mybir.dt.bfloat16
ALU = mybir.AluOpType
AF = mybir.ActivationFunctionType
AX = mybir.AxisListType

NCORES = 8
FRAME = 5
NF = FRAME - 1
D = 512
DH = 64
HEADS = 8
B = 20
NPTS = 1024
BE = B // FRAME
NPC = NPTS // NCORES     # 128 points per core
TOK = NF * BE * NPC      # 2048 tokens per core
NLAYER = 12
LNEPS = 1e-5
BNEPS = 1e-5
NBN = 16 * NPTS

CONV_DIMS = [2048, 1536, 1024, 768, 512]


def _pe_table(max_len=16, d=D):
    pos = np.arange(max_len, dtype=np.float32)[:, None]
    div = np.exp(np.arange(0, d, 2, dtype=np.float32) * (-math.log(10000.0) / d))
    pe = np.zeros((max_len, d), np.float32)
    pe[:, 0::2] = np.sin(pos * div)
    pe[:, 1::2] = np.cos(pos * div)
    return pe


def build_kernel():
    nc = bacc.Bacc("TRN2", target_bir_lowering=False, debug=False,
                   num_devices=NCORES)

    xin = nc.dram_tensor("xin", [CONV_DIMS[0], TOK], F32R, kind="ExternalInput").ap()
    convw = [nc.dram_tensor(f"convw{i}", [CONV_DIMS[i], CONV_DIMS[i + 1]], F32R,
                            kind="ExternalInput").ap() for i in range(4)]
    bnconst = [nc.dram_tensor(f"bnconst{i}", [128, 3 * (CONV_DIMS[i + 1] // 128)],
                              F32, kind="ExternalInput").ap() for i in range(3)]
    bias4 = nc.dram_tensor("bias4", [128, 4 * NF], F32, kind="ExternalInput").ap()

    wqkv_d = nc.dram_tensor("wqkv", [NLAYER, D, 3 * D], F32R, kind="ExternalInput").ap()
    wsbq_d = nc.dram_tensor("wsbq", [NLAYER, 2, 3 * D], BF16, kind="ExternalInput").ap()
    wo_d = nc.dram_tensor("wo", [NLAYER, D, D], BF16, kind="ExternalInput").ap()
    w1_d = nc.dram_tensor("w1", [NLAYER, D, D], F32R, kind="ExternalInput").ap()
    w2_d = nc.dram_tensor("w2", [NLAYER, D, D], F32R, kind="ExternalInput").ap()
    tcols_d = nc.dram_tensor("tcols", [NLAYER, 128, 12], F32, kind="ExternalInput").ap()

    projw_d = nc.dram_tensor("projw", [D, D], F32R, kind="ExternalInput").ap()
    rw1_d = nc.dram_tensor("rw1", [D, 256], F32R, kind="ExternalInput").ap()
    rw2_d = nc.dram_tensor("rw2", [256, 128], F32R, kind="ExternalInput").ap()
    rw3_d = nc.dram_tensor("rw3", [128, 8], F32R, kind="ExternalInput").ap()
    tw1_d = nc.dram_tensor("tw1", [D, 256], F32R, kind="ExternalInput").ap()
    tw2_d = nc.dram_tensor("tw2", [256, 128], F32R, kind="ExternalInput").ap()
    tw3_d = nc.dram_tensor("tw3", [128, 8], F32R, kind="ExternalInput").ap()
    hcols_d = nc.dram_tensor("hcols", [128, 10], F32, kind="ExternalInput").ap()
    sb3_d = nc.dram_tensor("sb3", [8, 2], F32, kind="ExternalInput").ap()
    ones_d = nc.dram_tensor("ones_c", [128, 128], F32, kind="ExternalInput").ap()

    d6_o = nc.dram_tensor("d6", [8, TOK], F32, kind="ExternalOutput").ap()
    tr_o = nc.dram_tensor("tr3", [8, TOK], F32, kind="ExternalOutput").ap()

    with tile.TileContext(nc) as tc, contextlib.ExitStack() as ctx:
        const_p = ctx.enter_context(tc.tile_pool(name="consts", bufs=1))
        onescol = const_p.tile([128, 1], F32R)
        onesrow = const_p.tile([1, 128], F32R)
        ident = const_p.tile([128, 128], BF16)
        nc.gpsimd.dma_start(onescol[:], ones_d[:, 0:1])
        nc.gpsimd.dma_start(onesrow[:], ones_d[0:1, :])
        ones_bf = const_p.tile([1, 128], BF16)
        nc.vector.memset(ones_bf[:], 1.0)
        masks.make_identity(nc, ident[:])

        xs_p = ctx.enter_context(tc.tile_pool(name="xstate", bufs=1))
        xA = [xs_p.tile([128, TOK], F32R, tag=f"xA{m}", name=f"xA{m}") for m in range(4)]

        stat_p = ctx.enter_context(tc.tile_pool(name="stats", bufs=1))
        dram_p = ctx.enter_context(tc.tile_pool(name="dramb", bufs=1, space="DRAM"))

        pp_mm = ctx.enter_context(tc.tile_pool(name="ppmm", bufs=4, space="PSUM"))
        pp_row = ctx.enter_context(tc.tile_pool(name="pprow", bufs=1, space="PSUM"))
        pp_bc = ctx.enter_context(tc.tile_pool(name="ppbc", bufs=2, space="PSUM"))

        y_dram = [dram_p.tile([CONV_DIMS[i], TOK], F32R, tag=f"ydram{i}", name=f"ydram{i}")
                  for i in range(1, 4)]

        # ------------------------------------------------------------------
        # conv stack (activations spilled to DRAM, BN applied on load)
        # ------------------------------------------------------------------
        b4sb = stat_p.tile([128, 4 * NF], F32, tag="b4")
        nc.sync.dma_start(b4sb[:], bias4[:])

        bn_s = {}
        bn_t = {}

        def conv_layer(li, wcp, cxp, pp_conv):
            kdim, mdim = CONV_DIMS[li - 1], CONV_DIMS[li]
            KC, MC = kdim // 128, mdim // 128
            src = xin if li == 1 else y_dram[li - 2]
            with_bn = li < 4
            if with_bn:
                sum_acc = stat_p.tile([128, MC * 4], F32, tag=f"sum{li}")
                sq_acc = stat_p.tile([128, MC * 4], F32, tag=f"sq{li}")
            cond_sb = None
            if li == 1:
                # channels 0:1024 repeat frame 0's features for all 4 frames;
                # compute their contribution once and add it at eviction
                KC = 8
                xc = cxp.tile([128, 8 * 512], F32R, tag="convc", name="convc",
                              bufs=1)
                nc.sync.dma_start(
                    xc[:].rearrange("p (k c) -> p k c", k=8),
                    src[0:1024, 0:512].rearrange("(k p) c -> p k c", p=128))
                cond_sb = [cxp.tile([128, 512], F32, tag=f"cond{m}",
                                    name=f"cond{m}") for m in range(MC)]
                for m in range(MC):
                    wslc = wcp.tile([128, 8 * 128], F32R, tag="wslc",
                                    name="wslc", bufs=3)
                    wvc = wslc[:].rearrange("p (k c) -> p k c", k=8)
                    nc.sync.dma_start(
                        wvc, convw[0][0:1024, m * 128:(m + 1) * 128]
                        .rearrange("(k p) c -> p k c", p=128))
                    ps = pp_conv.tile([128, 512], F32, tag="mm", name="ccps")
                    for k in range(8):
                        nc.tensor.matmul(
                            ps[:], wvc[:, k, :], xc[:, k * 512:(k + 1) * 512],
                            start=(k == 0), stop=(k == 7))
                    nc.scalar.copy(cond_sb[m][:], ps[:])
            for pt in range(4):
                xt = cxp.tile([128, KC * 512], F32R, tag="convx", name="convx",
                              bufs=3)
                nc.sync.dma_start(
                    xt[:].rearrange("p (k c) -> p k c", k=KC),
                    src[kdim - KC * 128:, pt * 512:(pt + 1) * 512]
                    .rearrange("(k p) c -> p k c", p=128))
                if li > 1:
                    s_p, t_p = bn_s[li - 1], bn_t[li - 1]
                    for k in range(KC):
                        nc.scalar.activation(
                            xt[:, k * 512:(k + 1) * 512],
                            xt[:, k * 512:(k + 1) * 512],
                            AF.Relu, bias=t_p[:, k:k + 1], scale=s_p[:, k:k + 1])
                for m in range(MC):
                    wsl = wcp.tile([128, KC * 128], F32R, tag="wsl", name="wsl",
                                   bufs=6)
                    wv = wsl[:].rearrange("p (k c) -> p k c", k=KC)
                    nc.sync.dma_start(
                        wv, convw[li - 1][kdim - KC * 128:,
                                          m * 128:(m + 1) * 128]
                        .rearrange("(k p) c -> p k c", p=128))
                    ps = pp_conv.tile([128, 512], F32, tag="mm", name="cps")
                    for k in range(KC):
                        nc.tensor.matmul(
                            ps[:], wv[:, k, :], xt[:, k * 512:(k + 1) * 512],
                            start=(k == 0), stop=(k == KC - 1))
                    if with_bn:
                        ot = cxp.tile([128, 512], F32R, tag="convot", name="cot",
                                      bufs=4)
                        if cond_sb is not None:
                            nc.vector.tensor_tensor(ot[:], ps[:],
                                                    cond_sb[m][:], op=ALU.add)
                            stats_src = ot[:]
                        else:
                            stats_src = ps[:]
                            nc.scalar.activation(
                                ot[:], ps[:], AF.Copy,
                                accum_out=sum_acc[:, m * 4 + pt:m * 4 + pt + 1])
                        sqs = cxp.tile([128, 512], BF16, tag="sqscr", name="sqs",
                                       bufs=4)
                        if cond_sb is not None:
                            nc.scalar.activation(
                                sqs[:], stats_src, AF.Copy,
                                accum_out=sum_acc[:, m * 4 + pt:m * 4 + pt + 1])
                        nc.scalar.activation(
                            sqs[:], stats_src, AF.Square,
                            accum_out=sq_acc[:, m * 4 + pt:m * 4 + pt + 1])
                        nc.sync.dma_start(
                            y_dram[li - 1][m * 128:(m + 1) * 128,
                                           pt * 512:(pt + 1) * 512], ot[:])
                    else:
                        nc.scalar.activation(
                            xA[m][:, pt * 512:(pt + 1) * 512], ps[:], AF.Identity,
                            bias=b4sb[:, m * 4 + pt:m * 4 + pt + 1])
            if not with_bn:
                return
            allin = stat_p.tile([128, 2 * MC], F32, tag=f"ain{li}", name="allin")
            nc.vector.tensor_reduce(
                allin[:, 0:MC], sum_acc[:].rearrange("p (m t) -> p m t", m=MC),
                axis=AX.X, op=ALU.add)
            nc.vector.tensor_reduce(
                allin[:, MC:2 * MC], sq_acc[:].rearrange("p (m t) -> p m t", m=MC),
                axis=AX.X, op=ALU.add)
            bin_ = dram_p.tile([128, 2 * MC], F32, tag=f"arin{li}", name="arin")
            bout = dram_p.tile([128, 2 * MC], F32, tag=f"arout{li}", name="arout")
            nc.sync.dma_start(bin_[:], allin[:])
            nc.gpsimd.collective_compute(
                "AllReduce", ALU.add, replica_groups=[list(range(NCORES))],
                ins=[bin_.opt()], outs=[bout.opt()])
            gl = stat_p.tile([128, 2 * MC], F32, tag=f"gl{li}", name="gl")
            nc.sync.dma_start(gl[:], bout[:])
            cst = stat_p.tile([128, 3 * MC], F32, tag=f"cst{li}", name="cst")
            nc.sync.dma_start(cst[:], bnconst[li - 1][:])
            mu = stat_p.tile([128, MC], F32, tag=f"mu{li}", name="bmu")
            var = stat_p.tile([128, MC], F32, tag=f"va{li}", name="bvar")
            s_t = stat_p.tile([128, MC], F32, tag=f"s{li}", name="bs")
            t_t = stat_p.tile([128, MC], F32, tag=f"t{li}", name="bt")
            nc.scalar.mul(mu[:], gl[:, 0:MC], 1.0 / NBN)
            nc.scalar.mul(var[:], gl[:, MC:2 * MC], 1.0 / NBN)
            msq = stat_p.tile([128, MC], F32, tag=f"ms{li}", name="bmsq")
            nc.vector.tensor_mul(msq[:], mu[:], mu[:])
            nc.vector.tensor_tensor(var[:], var[:], msq[:], op=ALU.subtract)
            nc.vector.tensor_scalar(var[:], var[:], BNEPS, None, op0=ALU.add)
            sd = stat_p.tile([128, MC], F32, tag=f"sd{li}", name="bsd")
            nc.scalar.activation(sd[:], var[:], AF.Sqrt)
            rsd = stat_p.tile([128, MC], F32, tag=f"rs{li}", name="brsd")
            nc.vector.reciprocal(rsd[:], sd[:])
            nc.vector.tensor_mul(s_t[:], rsd[:], cst[:, 0:MC])
            nc.vector.tensor_mul(t_t[:], mu[:], s_t[:])
            nc.vector.tensor_tensor(t_t[:], cst[:, MC:2 * MC], t_t[:],
                                    op=ALU.subtract)
            bn_s[li], bn_t[li] = s_t, t_t

        with tc.tile_pool(name="wcp", bufs=1) as wcp, \
             tc.tile_pool(name="cxp", bufs=1) as cxp:
            for li in (1, 2, 3, 4):
                conv_layer(li, wcp, cxp, pp_mm)

        # ------------------------------------------------------------------
        # transformer
        # ------------------------------------------------------------------
        rows_p = ctx.enter_context(tc.tile_pool(name="rows", bufs=1))
        scr = ctx.enter_context(tc.tile_pool(name="scratch", bufs=2))

        def ln_cols(xt, xview, dst_tiles, dst_cols):
            """LN per token over feature dim (stats + apply on DVE/Pool)."""
            ps_s = pp_row.tile([1, 512], F32, tag="row_s", name="ps_s")
            ps_q = pp_row.tile([1, 512], F32, tag="row_q", name="ps_q")
            for k in range(4):
                nc.tensor.matmul(ps_s[:], onescol[:], xview(k),
                                 start=(k == 0), stop=(k == 3))
            for k in range(4):
                sq = scr.tile([128, 512], F32R, tag="lnsq", name="lnsq")
                eng = nc.vector if k % 2 else nc.gpsimd
                eng.tensor_mul(sq[:], xview(k), xview(k))
                nc.tensor.matmul(ps_q[:], onescol[:], sq[:],
                                 start=(k == 0), stop=(k == 3))
            mu = rows_p.tile([1, 512], F32R, tag="mu", name="lmu", bufs=2)
            e2 = rows_p.tile([1, 512], F32, tag="e2", name="le2", bufs=2)
            r = rows_p.tile([1, 512], F32R, tag="r", name="lr", bufs=2)
            nc.scalar.mul(mu[:], ps_s[:], 1.0 / D)
            nc.scalar.mul(e2[:], ps_q[:], 1.0 / D)
            with nc.allow_low_precision(reason="f32r row math"):
                nc.vector.tensor_mul(r[:], mu[:], mu[:])
                nc.vector.scalar_tensor_tensor(
                    e2[:], e2[:], LNEPS, r[:], op0=ALU.add, op1=ALU.subtract)
                # 1/sqrt(v) = exp(-0.5 ln v): stays inside the exp/ln table
                nc.scalar.activation(e2[:], e2[:], AF.Ln)
                nc.scalar.activation(r[:], e2[:], AF.Exp, scale=-0.5)
            psb_mu = pp_bc.tile([128, 512], F32, tag="bc", name="psbmu")
            psb_r = pp_bc.tile([128, 512], F32, tag="bc", name="psbr")
            nc.tensor.matmul(psb_mu[:], onesrow[:], mu[:], start=True, stop=True)
            nc.tensor.matmul(psb_r[:], onesrow[:], r[:], start=True, stop=True)
            for k in range(4):
                tmp = scr.tile([128, 512], F32, tag="lntmp", name="lntmp")
                nc.vector.tensor_tensor(tmp[:], xview(k), psb_mu[:],
                                        op=ALU.subtract)
                nc.vector.tensor_mul(dst_tiles[k][:, dst_cols], tmp[:], psb_r[:])

        def ln1_rows(x_in, nm, rcol_all):
            """Per-frame LN stats; negmu row (K=1 fold operand) + 1/sd cols."""
            for f in range(4):
                sl = slice(f * 512, (f + 1) * 512)
                ps_s = pp_row.tile([1, 512], F32, tag="row_s", name="ps_s")
                ps_q = pp_row.tile([1, 512], F32, tag="row_q", name="ps_q")
                for k in range(4):
                    nc.tensor.matmul(ps_s[:], onescol[:], x_in[k][:, sl],
                                     start=(k == 0), stop=(k == 3))
                for k in range(4):
                    sq = scr.tile([128, 512], F32R, tag="lnsq", name="lnsq")
                    eng = nc.vector if k % 2 else nc.gpsimd
                    eng.tensor_mul(sq[:], x_in[k][:, sl], x_in[k][:, sl])
                    nc.tensor.matmul(ps_q[:], onescol[:], sq[:],
                                     start=(k == 0), stop=(k == 3))
                e2 = rows_p.tile([1, 512], F32, tag="e2", name="le2", bufs=2)
                rr = rows_p.tile([1, 512], F32, tag="rr", name="lrr", bufs=4)
                nc.scalar.mul(e2[:], ps_q[:], 1.0 / D)
                with nc.allow_low_precision(reason="ln1 rows"):
                    nc.scalar.mul(nm[0:1, sl], ps_s[:], -1.0 / D)
                    msq = rows_p.tile([1, 512], F32, tag="rr", name="lms", bufs=4)
                    nc.vector.tensor_mul(msq[:], nm[0:1, sl], nm[0:1, sl])
                    nc.vector.scalar_tensor_tensor(
                        e2[:], e2[:], LNEPS, msq[:], op0=ALU.add,
                        op1=ALU.subtract)
                    nc.scalar.activation(e2[:], e2[:], AF.Ln)
                    nc.scalar.activation(rr[:], e2[:], AF.Exp, scale=-0.5)
                for st in range(4):
                    nc.sync.dma_start(
                        rcol_all[:, f * 4 + st:f * 4 + st + 1],
                        rr[0:1, st * 128:(st + 1) * 128])

        tr_ctx = ctx.enter_context(contextlib.ExitStack())
        wp = tr_ctx.enter_context(tc.tile_pool(name="wp", bufs=1))
        wqp = tr_ctx.enter_context(tc.tile_pool(name="wqp", bufs=1))
        attn_p = tr_ctx.enter_context(tc.tile_pool(name="attn", bufs=2))
        sl_p = tr_ctx.enter_context(tc.tile_pool(name="slices", bufs=1))
        ot_p = tr_ctx.enter_context(tc.tile_pool(name="otp", bufs=1))
        otb_all = ot_p.tile([128, 4 * TOK], BF16, tag="otall", name="otall")

        def st_view(xt, k, st):
            # scattered columns {f*512 + st*128 + p} as (128, (f,p)=512)
            return xt[k][:].rearrange("p (f s) -> p f s", f=4)[:, :, st * 128:(st + 1) * 128]

        def transformer_layer(li, x_in, x_mid):
            wq = [wqp.tile([128, 3 * D], F32R, tag=f"wqkv{k}", name=f"wq{k}")
                  for k in range(4)]
            for k in range(4):
                nc.sync.dma_start(wq[k][:], wqkv_d[li, k * 128:(k + 1) * 128, :])
            ws_t = rows_p.tile([1, 3 * D], BF16, tag="wsum", name="wst", bufs=1)
            nc.sync.dma_start(ws_t[:], wsbq_d[li, 0:1, :])
            bq_t = rows_p.tile([1, 3 * D], BF16, tag="bqr", name="bqt", bufs=1)
            nc.sync.dma_start(bq_t[:], wsbq_d[li, 1:2, :])
            cols = stat_p.tile([128, 12], F32, tag="tcols", name="tcols")
            nc.sync.dma_start(cols[:], tcols_d[li])

            # q bias broadcast over the token partitions (k-bias is
            # softmax-invariant; v-bias is folded into outb on the host)
            bias_bc = attn_p.tile([128, D], BF16, tag="biasbc",
                                  name="bias_bc", bufs=1)
            psb = pp_bc.tile([128, 512], F32, tag="bc", name="psbb")
            nc.tensor.matmul(psb[:], ones_bf[:], bq_t[:, 0:512],
                             start=True, stop=True)
            nc.scalar.copy(bias_bc[:], psb[:])

            nm = attn_p.tile([1, TOK], BF16, tag="nmsd", name="nm", bufs=1)
            rcol = attn_p.tile([128, 16], F32, tag="rcol", name="rcol", bufs=1)
            ln1_rows(x_in, nm, rcol)

            for st in range(4):
                qt = attn_p.tile([128, TOK], BF16, tag="qst", name="qt", bufs=2)
                kt = attn_p.tile([128, TOK], BF16, tag="kst", name="kt", bufs=2)
                # v stored (j, d, h) so the AV multiply hits the 2x DVE mode
                vt = attn_p.tile([128, TOK], BF16, tag="vst", name="vt", bufs=2)
                for f in range(NF):
                    c0 = f * 512 + st * 128
                    for ns in range(3):
                        ps = pp_mm.tile([128, 512], F32, tag="mm", name="qps")
                        for k in range(4):
                            nc.tensor.matmul(
                                ps[:], x_in[k][:, c0:c0 + 128],
                                wq[k][:, ns * 512:(ns + 1) * 512],
                                start=(k == 0), stop=False)
                        nc.tensor.matmul(ps[:], nm[0:1, c0:c0 + 128],
                                         ws_t[:, ns * 512:(ns + 1) * 512],
                                         start=False, stop=True)
                        rc = rcol[:, f * 4 + st:f * 4 + st + 1]
                        if ns < 2:
                            nc.scalar.activation(
                                (qt if ns == 0 else kt)[:, f * 512:(f + 1) * 512],
                                ps[:], AF.Copy, scale=rc)
                        else:
                            nc.scalar.activation(
                                vt[:, f * 512:(f + 1) * 512]
                                .rearrange("p (d h) -> p h d", h=8),
                                ps[:].rearrange("p (h d) -> p h d", h=8),
                                AF.Copy, scale=rc)
                # q bias, broadcast over frames, one 2x-mode op per st
                nc.vector.tensor_tensor(
                    qt[:].rearrange("p (f c) -> p f c", f=4),
                    qt[:].rearrange("p (f c) -> p f c", f=4),
                    bias_bc[:].unsqueeze(1).broadcast_to([128, 4, 512]),
                    op=ALU.add)

                s_sc = attn_p.tile([128, 128], F32, tag="s_sc", name="s_sc",
                                   bufs=2)
                k4 = kt[:].rearrange("p (j hd) -> p j hd", j=4)
                for i in range(4):
                    pbig = attn_p.tile([128, TOK], BF16, tag="pbig",
                                       name="pbig", bufs=2)
                    qi = qt[:, i * 512:(i + 1) * 512].unsqueeze(1) \
                        .broadcast_to([128, 4, 512])
                    nc.vector.tensor_mul(
                        pbig[:].rearrange("p (j hd) -> p j hd", j=4), qi, k4)
                    st1 = attn_p.tile([128, TOK // 2], BF16, tag="qks1",
                                      name="qks1", bufs=2)
                    st2 = attn_p.tile([128, TOK // 4], BF16, tag="qks2",
                                      name="qks2", bufs=2)
                    pv = pbig[:].rearrange("p (g d) -> p g d", g=32)
                    nc.vector.tensor_tensor(
                        st1[:].rearrange("p (g d) -> p g d", g=32),
                        pv[:, :, 0:32], pv[:, :, 32:64], op=ALU.add)
                    sv = st1[:].rearrange("p (g d) -> p g d", g=32)
                    nc.vector.tensor_tensor(
                        st2[:].rearrange("p (g d) -> p g d", g=32),
                        sv[:, :, 0:16], sv[:, :, 16:32], op=ALU.add)
                    nc.vector.tensor_reduce(
                        s_sc[:, i * 32:(i + 1) * 32],
                        st2[:].rearrange("p (g d) -> p g d", g=32),
                        axis=AX.X, op=ALU.add)
                # softmax over j without max-subtraction (logits bounded)
                # S cols = (i, j, h)
                eexp = attn_p.tile([128, 128], BF16, tag="eexp", name="eexp",
                                   bufs=2)
                nc.scalar.activation(eexp[:], s_sc[:], AF.Exp)
                z = attn_p.tile([128, 32], F32, tag="z", name="zt", bufs=2)
                nc.vector.tensor_reduce(
                    z[:].rearrange("p (i h) -> p i h", i=4),
                    eexp[:].rearrange("p (i j h) -> p i h j", i=4, j=4),
                    axis=AX.X, op=ALU.add)
                zr = attn_p.tile([128, 32], F32, tag="zr", name="zr", bufs=2)
                nc.vector.reciprocal(zr[:], z[:])
                a_t = attn_p.tile([128, 128], BF16, tag="a_t", name="a_t",
                                  bufs=2)
                nc.vector.tensor_mul(
                    a_t[:].rearrange("p (i j h) -> p i j h", i=4, j=4),
                    eexp[:].rearrange("p (i j h) -> p i j h", i=4, j=4),
                    zr[:].rearrange("p (i h) -> p i h", i=4).unsqueeze(2)
                    .broadcast_to([128, 4, 4, 8]))
                for i in range(4):
                    tbig = attn_p.tile([128, TOK], BF16, tag="tbig", name="tbig", bufs=2)
                    ablk = a_t[:, i * 32:(i + 1) * 32] \
                        .rearrange("p (j h) -> p j h", j=4) \
                        .unsqueeze(2).broadcast_to([128, 4, 64, 8])
                    nc.vector.tensor_mul(
                        tbig[:].rearrange("p (j d h) -> p j d h", j=4, d=64),
                        vt[:].rearrange("p (j d h) -> p j d h", j=4, d=64),
                        ablk)
                    av01 = attn_p.tile([128, 512], BF16, tag="av01", name="av01",
                                       bufs=2)
                    av = attn_p.tile([128, 512], BF16, tag="av", name="av",
                                     bufs=2)
                    nc.vector.tensor_tensor(av01[:], tbig[:, 0:512],
                                            tbig[:, 512:1024], op=ALU.add)
                    nc.gpsimd.tensor_tensor(av[:], tbig[:, 1024:1536],
                                            tbig[:, 1536:2048], op=ALU.add)
                    nc.vector.tensor_tensor(av[:], av01[:], av[:],
                                            op=ALU.add)
                    pst = pp_bc.tile([128, 512], BF16, tag="bc", name="pst")
                    for c in range(4):
                        nc.tensor.transpose(pst[:, c * 128:(c + 1) * 128],
                                            av[:, c * 128:(c + 1) * 128],
                                            ident[:])
                    nc.scalar.copy(
                        otb_all[:].rearrange("p (c t) -> p c t", c=4)
                        [:, :, i * 512 + st * 128:i * 512 + st * 128 + 128],
                        pst[:].rearrange("p (c t) -> p c t", c=4))

            wo = [wp.tile([128, D], BF16, tag=f"wo{k}", name=f"wo{k}")
                  for k in range(4)]
            for k in range(4):
                nc.sync.dma_start(wo[k][:], wo_d[li, k * 128:(k + 1) * 128, :])
            for m in range(4):
                for ns in range(4):
                    ps = pp_mm.tile([128, 512], F32, tag="mm", name="ops")
                    for k in range(4):
                        nc.tensor.matmul(
                            ps[:], wo[k][:, m * 128:(m + 1) * 128],
                            otb_all[:, k * TOK + ns * 512:k * TOK + (ns + 1) * 512],
                            start=(k == 0), stop=(k == 3))
                    nc.vector.scalar_tensor_tensor(
                        x_mid[m][:, ns * 512:(ns + 1) * 512], ps[:],
                        cols[:, 4 + m:5 + m], x_in[m][:, ns * 512:(ns + 1) * 512],
                        op0=ALU.add, op1=ALU.add)

            w1 = [wp.tile([128, D], F32R, tag=f"w1_{k}", name=f"w1_{k}")
                  for k in range(4)]
            w2 = [wp.tile([128, D], F32R, tag=f"w2_{k}", name=f"w2_{k}")
                  for k in range(4)]
            for k in range(4):
                nc.sync.dma_start(w1[k][:], w1_d[li, k * 128:(k + 1) * 128, :])
                nc.sync.dma_start(w2[k][:], w2_d[li, k * 128:(k + 1) * 128, :])
            for ns in range(4):
                xh2 = [sl_p.tile([128, 512], F32R, tag=f"xh2_{k}", name=f"xh2_{k}")
                       for k in range(4)]
                ln_cols(x_mid,
                        lambda k: x_mid[k][:, ns * 512:(ns + 1) * 512],
                        xh2, slice(0, 512))
                hsl = [sl_p.tile([128, 512], F32R, tag=f"h_{m}", name=f"hsl{m}")
                       for m in range(4)]
                for m in range(4):
                    ps = pp_mm.tile([128, 512], F32, tag="mm", name="m1ps")
                    for k in range(4):
                        nc.tensor.matmul(
                            ps[:], w1[k][:, m * 128:(m + 1) * 128], xh2[k][:],
                            start=(k == 0), stop=(k == 3))
                    nc.scalar.activation(hsl[m][:], ps[:], AF.Gelu_apprx_tanh,
                                         bias=cols[:, m:m + 1])
                for m in range(4):
                    ps = pp_mm.tile([128, 512], F32, tag="mm", name="m2ps")
                    for k in range(4):
                        nc.tensor.matmul(
                            ps[:], w2[k][:, m * 128:(m + 1) * 128], hsl[k][:],
                            start=(k == 0), stop=(k == 3))
                    nc.vector.scalar_tensor_tensor(
                        x_mid[m][:, ns * 512:(ns + 1) * 512], ps[:],
                        cols[:, 8 + m:9 + m], x_mid[m][:, ns * 512:(ns + 1) * 512],
                        op0=ALU.add, op1=ALU.add)

        cur = xA
        _nl = int(os.environ.get("KNLAYERS", NLAYER))
        for li in range(_nl):
            transformer_layer(li, cur, cur)

        tr_ctx.close()

        # ------------------------------------------------------------------
        # heads
        # ------------------------------------------------------------------
        _skip_heads = os.environ.get("KHEADS", "1") == "0"
        if _skip_heads:
            nc.gpsimd.dma_start(d6_o[:], cur[0][0:8, :])
            nc.gpsimd.dma_start(tr_o[:], cur[1][0:8, :])
        with tc.tile_pool(name="heads", bufs=1) as hp, \
             tc.tile_pool(name="whp", bufs=1) as whp:
          if not _skip_heads:
              hc = stat_p.tile([128, 10], F32, tag="hcols", name="hc")
              nc.sync.dma_start(hc[:], hcols_d[:])
              sb3 = stat_p.tile([8, 2], F32, tag="sb3", name="sb3")
              nc.sync.dma_start(sb3[:], sb3_d[:])

              xhf = [hp.tile([128, TOK], F32R, tag=f"xhf{k}", name=f"xhf{k}")
                     for k in range(4)]
              for st in range(4):
                  ln_cols(cur, lambda k: cur[k][:, st * 512:(st + 1) * 512],
                          xhf, slice(st * 512, (st + 1) * 512))

              xp = [hp.tile([128, TOK], F32R, tag=f"xp{k}", name=f"xp{k}")
                    for k in range(4)]

              def mm_head(src_tiles, wt_dram, kdim, mdim, dst_tiles, evict):
                  KC = kdim // 128
                  MC = max(mdim // 128, 1)
                  wsb = [whp.tile([128, mdim], F32R, tag=f"wh_{kdim}_{mdim}_{k}",
                                  name=f"wh{k}") for k in range(KC)]
                  for k in range(KC):
                      nc.sync.dma_start(wsb[k][:], wt_dram[k * 128:(k + 1) * 128, :])
                  for m in range(MC):
                      for ns in range(4):
                          ps = pp_mm.tile([128, 512], F32, tag="mm", name="hps")
                          for k in range(KC):
                              nc.tensor.matmul(
                                  ps[:], wsb[k][:, m * 128:(m + 1) * 128],
                                  src_tiles[k][:, ns * 512:(ns + 1) * 512],
                                  start=(k == 0), stop=(k == KC - 1))
                          evict(ps, dst_tiles[m], m, ns)

              mm_head(xhf, projw_d, D, D, xp,
                      lambda ps, dst, m, ns: nc.scalar.activation(
                          dst[:, ns * 512:(ns + 1) * 512], ps[:], AF.Identity,
                          bias=hc[:, m:m + 1]))

              def branch(w1d, w2d, w3d, b1ofs, b2ofs, out_dram, b3col, r1, r2, pfx):
                  mm_head(xp, w1d, D, 256, r1,
                          lambda ps, dst, m, ns: nc.scalar.activation(
                              dst[:, ns * 512:(ns + 1) * 512], ps[:], AF.Relu,
                              bias=hc[:, b1ofs + m:b1ofs + m + 1]))
                  mm_head(r1, w2d, 256, 128, r2,
                          lambda ps, dst, m, ns: nc.scalar.activation(
                              dst[:, ns * 512:(ns + 1) * 512], ps[:], AF.Relu,
                              bias=hc[:, b2ofs:b2ofs + 1]))
                  w3 = whp.tile([128, 8], F32R, tag=f"w3{pfx}", name="w3")
                  nc.sync.dma_start(w3[:], w3d[:])
                  out_sb = hp.tile([8, TOK], F32, tag=f"{pfx}out", name=f"{pfx}out")
                  for ns in range(4):
                      ps = pp_mm.tile([8, 512], F32, tag="mm", name="bps")
                      nc.tensor.matmul(ps[:], w3[:], r2[0][:, ns * 512:(ns + 1) * 512],
                                       start=True, stop=True)
                      nc.scalar.activation(out_sb[:, ns * 512:(ns + 1) * 512], ps[:],
                                           AF.Identity, bias=b3col)
                  nc.sync.dma_start(out_dram[:], out_sb[:])

              # reuse dead transformer buffers for intermediates
              branch(rw1_d, rw2_d, rw3_d, 4, 8, d6_o, sb3[:, 0:1],
                     [cur[0], cur[1]], [xhf[0]], "r")
              branch(tw1_d, tw2_d, tw3_d, 6, 9, tr_o, sb3[:, 1:2],
                     [cur[2], cur[3]], [xhf[1]], "t")

    nc.compile()
    return nc


# ----------------------------------------------------------------------------
# host side
# ----------------------------------------------------------------------------

_CACHE = {}


def _normalize_np(v, eps=1e-12):
    return v / np.maximum(np.linalg.norm(v, axis=-1, keepdims=True), eps)


def _rot6d_np(d6):
    a1, a2 = d6[..., :3], d6[..., 3:]
    b1 = _normalize_np(a1)
    b2 = _normalize_np(a2 - np.sum(b1 * a2, -1, keepdims=True) * b1)
    b3 = np.cross(b1, b2)
    return np.stack([b1, b2, b3], axis=-2)


def _prep_weights(inp):
    f32 = np.float32
    wmap = {}
    for i, cw in enumerate(['c1w', 'c2w', 'c3w', 'c4w']):
        wmap[f'convw{i}'] = np.ascontiguousarray(inp[cw].T.astype(f32))
    for i, (g, b2) in enumerate([('bn1g', 'bn1b'), ('bn2g', 'bn2b'),
                                 ('bn3g', 'bn3b')]):
        M = CONV_DIMS[i + 1] // 128
        bn = np.concatenate([
            inp[g].reshape(M, 128).T, inp[b2].reshape(M, 128).T,
            inp[f'c{i + 1}b'].reshape(M, 128).T], axis=1)
        wmap[f'bnconst{i}'] = np.ascontiguousarray(bn.astype(f32))
    pe = _pe_table()[:NF]
    b4 = inp['c4b'][None, :].astype(f32) + pe                   # (4, 512)
    # cols: m*4 + pt ; frame index == pt
    wmap['bias4'] = np.ascontiguousarray(
        b4.reshape(NF, 4, 128).transpose(2, 1, 0).reshape(128, 16).astype(f32))

    qkvw = np.array(inp['qkvw'], f32)
    qkvb = np.array(inp['qkvb'], f32)
    qkvw[:, :, :512] /= math.sqrt(DH)
    qkvb[:, :512] /= math.sqrt(DH)
    g1 = np.array(inp['ln1g'], f32)
    b1 = np.array(inp['ln1b'], f32)
    wq_fold = g1[:, :, None] * qkvw
    bq_fold = qkvb + np.einsum('ld,ldf->lf', b1, qkvw)
    wmap['wqkv'] = np.ascontiguousarray(wq_fold.astype(f32))
    wsbq = np.concatenate([wq_fold.sum(axis=1, keepdims=True),
                           bq_fold[:, None, :]], axis=1)         # (L, 2, 3D)
    wmap['wsbq'] = np.ascontiguousarray(wsbq.astype(ml_dtypes.bfloat16))
    # attention output features are (d, h)-ordered; permute wo rows to match
    wo_ = np.array(inp['outw'], f32)                             # (L, 512, 512)
    d_idx, h_idx = np.meshgrid(np.arange(DH), np.arange(HEADS), indexing='ij')
    perm = (h_idx * DH + d_idx).reshape(512)     # perm[d*8+h] = h*64+d
    wmap['wo'] = np.ascontiguousarray(wo_[:, perm, :]
                                      .astype(ml_dtypes.bfloat16))
    g2 = np.array(inp['ln2g'], f32)
    bl2 = np.array(inp['ln2b'], f32)
    m1w = np.array(inp['m1w'], f32)
    w1_fold = g2[:, :, None] * m1w
    b1_fold = np.array(inp['m1b'], f32) + np.einsum('ld,ldf->lf', bl2, m1w)
    wmap['w1'] = np.ascontiguousarray(w1_fold.astype(f32))
    wmap['w2'] = np.ascontiguousarray(np.array(inp['m2w'], f32))
    cols = np.zeros((NLAYER, 128, 12), f32)
    cols[:, :, 0:4] = b1_fold.reshape(NLAYER, 4, 128).transpose(0, 2, 1)
    # v-bias is dropped at the v eviction; fold bv @ Wo into outb instead
    bv = bq_fold[:, 2 * 512:3 * 512]                             # (L, 512)
    outb_fold = np.array(inp['outb'], f32) + np.einsum('lk,lko->lo', bv, wo_)
    cols[:, :, 4:8] = outb_fold.reshape(NLAYER, 4, 128).transpose(0, 2, 1)
    cols[:, :, 8:12] = np.array(inp['m2b'], f32).reshape(NLAYER, 4, 128) \
        .transpose(0, 2, 1)
    wmap['tcols'] = cols

    gf_ = np.array(inp['lnfg'], f32)
    bf_ = np.array(inp['lnfb'], f32)
    projw = np.array(inp['projw'], f32)
    wmap['projw'] = np.ascontiguousarray(gf_[:, None] * projw)
    projb_fold = np.array(inp['projb'], f32) + bf_ @ projw
    wmap['rw1'] = np.ascontiguousarray(np.array(inp['rw1'], f32))
    wmap['rw2'] = np.ascontiguousarray(np.array(inp['rw2'], f32))
    rw3 = np.zeros((128, 8), f32)
    rw3[:, :6] = np.array(inp['rw3'], f32)
    wmap['rw3'] = rw3
    wmap['tw1'] = np.ascontiguousarray(np.array(inp['tw1'], f32))
    wmap['tw2'] = np.ascontiguousarray(np.array(inp['tw2'], f32))
    tw3 = np.zeros((128, 8), f32)
    tw3[:, :3] = np.array(inp['tw3'], f32)
    wmap['tw3'] = tw3
    hcols = np.zeros((128, 10), f32)
    hcols[:, 0:4] = projb_fold.reshape(4, 128).T
    hcols[:, 4:6] = np.array(inp['rb1'], f32).reshape(2, 128).T
    hcols[:, 6:8] = np.array(inp['tb1'], f32).reshape(2, 128).T
    hcols[:, 8] = np.array(inp['rb2'], f32)
    hcols[:, 9] = np.array(inp['tb2'], f32)
    wmap['hcols'] = hcols
    sb3 = np.zeros((8, 2), f32)
    sb3[0:6, 0] = np.array(inp['rb3'], f32)
    sb3[0:3, 1] = np.array(inp['tb3'], f32)
    wmap['sb3'] = sb3
    wmap['ones_c'] = np.ones((128, 128), f32)
    return wmap


def kernel(**inputs):
    inp = {k: np.asarray(v) for k, v in inputs.items()}

    idx = inp['seed_idxs'].reshape(B, -1).astype(np.int64)      # (B, N)
    sel_seed = np.take_along_axis(np.asarray(inp['fp2_features'], np.float32),
                                  idx[:, None, :], axis=2)
    sel_grasp = np.take_along_axis(np.asarray(inp['local_grasp_features'], np.float32),
                                   idx[:, None, :], axis=2)
    sel_color = np.take_along_axis(np.asarray(inp['local_color_features'], np.float32),
                                   idx[:, None, :], axis=2)
    sel_pose = np.take_along_axis(np.asarray(inp['grasp_pose_feature'], np.float32),
                                  idx[:, None, :], axis=2)
    gsf = np.asarray(inp['sa4_features'], np.float32).max(axis=-1)
    gsf = np.broadcast_to(gsf[:, :, None], (B, 256, NPTS))
    fused = sel_pose + np.concatenate([sel_grasp, sel_color, sel_seed, gsf], axis=1)
    gf = fused.reshape(BE, FRAME, 1024, NPTS)
    cond = np.broadcast_to(gf[:, :1], (BE, NF, 1024, NPTS))
    X = np.concatenate([cond, gf[:, 1:]], axis=2)               # (e, f, 2048, N)

    if 'nc' not in _CACHE:
        _CACHE['nc'] = build_kernel()
    nc = _CACHE['nc']
    wmap = _prep_weights(inp)

    in_maps = []
    for k in range(NCORES):
        xc = X[:, :, :, k * NPC:(k + 1) * NPC]                  # (e, f, c, n)
        xc = xc.transpose(2, 1, 0, 3).reshape(2048, TOK)        # (c, (f,e,n))
        m = dict(wmap)
        m['xin'] = np.ascontiguousarray(xc, dtype=np.float32)
        in_maps.append(m)

    res = run_bass_kernel_spmd(nc, in_maps, core_ids=list(range(NCORES)))

    out = np.zeros((BE * NPTS, NF, 12), np.float32)
    for k in range(NCORES):
        d6 = res.results[k]['d6'][:6]
        tr = res.results[k]['tr3'][:3]
        d6 = d6.reshape(6, NF, BE, NPC).transpose(2, 3, 1, 0)   # (e, n, f, 6)
        tr = tr.reshape(3, NF, BE, NPC).transpose(2, 3, 1, 0)
        rot = _rot6d_np(d6).reshape(BE, NPC, NF, 9)
        for e in range(BE):
            rows = slice(e * NPTS + k * NPC, e * NPTS + (k + 1) * NPC)
            out[rows, :, 0:3] = tr[e]
            out[rows, :, 3:12] = rot[e]
    return out


if __name__ == "__main__":
    build_kernel()
    print("built ok")



# revision 68
# speedup vs baseline: 1.0872x; 1.0045x over previous
import math
import os
import numpy as np
import ml_dtypes
import contextlib

import concourse.bass as bass
import concourse.tile as tile
from concourse import bacc, mybir, masks
from concourse.bass_utils import run_bass_kernel_spmd

F32 = mybir.dt.float32
F32R = mybir.dt.float32r
BF16 = mybir.dt.bfloat16
ALU = mybir.AluOpType
AF = mybir.ActivationFunctionType
AX = mybir.AxisListType

NCORES = 8
FRAME = 5
NF = FRAME - 1
D = 512
DH = 64
HEADS = 8
B = 20
NPTS = 1024
BE = B // FRAME
NPC = NPTS // NCORES     # 128 points per core
TOK = NF * BE * NPC      # 2048 tokens per core
NLAYER = 12
LNEPS = 1e-5
BNEPS = 1e-5
NBN = 16 * NPTS

CONV_DIMS = [2048, 1536, 1024, 768, 512]


def _pe_table(max_len=16, d=D):
    pos = np.arange(max_len, dtype=np.float32)[:, None]
    div = np.exp(np.arange(0, d, 2, dtype=np.float32) * (-math.log(10000.0) / d))
    pe = np.zeros((max_len, d), np.float32)
    pe[:, 0::2] = np.sin(pos * div)
    pe[:, 1::2] = np.cos(pos * div)
    return pe


def build_kernel():
    nc = bacc.Bacc("TRN2", target_bir_lowering=False, debug=False,
                   num_devices=NCORES)

    xin = nc.dram_tensor("xin", [CONV_DIMS[0], TOK], F32R, kind="ExternalInput").ap()
    convw = [nc.dram_tensor(f"convw{i}", [CONV_DIMS[i], CONV_DIMS[i + 1]], F32R,
                            kind="ExternalInput").ap() for i in range(4)]
    bnconst = [nc.dram_tensor(f"bnconst{i}", [128, 3 * (CONV_DIMS[i + 1] // 128)],
                              F32, kind="ExternalInput").ap() for i in range(3)]
    bias4 = nc.dram_tensor("bias4", [128, 4 * NF], F32, kind="ExternalInput").ap()

    wqkv_d = nc.dram_tensor("wqkv", [NLAYER, D, 3 * D], F32R, kind="ExternalInput").ap()
    wsbq_d = nc.dram_tensor("wsbq", [NLAYER, 2, 3 * D], BF16, kind="ExternalInput").ap()
    wo_d = nc.dram_tensor("wo", [NLAYER, D, D], BF16, kind="ExternalInput").ap()
    w1_d = nc.dram_tensor("w1", [NLAYER, D, D], F32R, kind="ExternalInput").ap()
    w2_d = nc.dram_tensor("w2", [NLAYER, D, D], F32R, kind="ExternalInput").ap()
    tcols_d = nc.dram_tensor("tcols", [NLAYER, 128, 12], F32, kind="ExternalInput").ap()

    projw_d = nc.dram_tensor("projw", [D, D], F32R, kind="ExternalInput").ap()
    rw1_d = nc.dram_tensor("rw1", [D, 256], F32R, kind="ExternalInput").ap()
    rw2_d = nc.dram_tensor("rw2", [256, 128], F32R, kind="ExternalInput").ap()
    rw3_d = nc.dram_tensor("rw3", [128, 8], F32R, kind="ExternalInput").ap()
    tw1_d = nc.dram_tensor("tw1", [D, 256], F32R, kind="ExternalInput").ap()
    tw2_d = nc.dram_tensor("tw2", [256, 128], F32R, kind="ExternalInput").ap()
    tw3_d = nc.dram_tensor("tw3", [128, 8], F32R, kind="ExternalInput").ap()
    hcols_d = nc.dram_tensor("hcols", [128, 10], F32, kind="ExternalInput").ap()
    sb3_d = nc.dram_tensor("sb3", [8, 2], F32, kind="ExternalInput").ap()
    ones_d = nc.dram_tensor("ones_c", [128, 128], F32, kind="ExternalInput").ap()

    d6_o = nc.dram_tensor("d6", [8, TOK], F32, kind="ExternalOutput").ap()
    tr_o = nc.dram_tensor("tr3", [8, TOK], F32, kind="ExternalOutput").ap()

    with tile.TileContext(nc) as tc, contextlib.ExitStack() as ctx:
        const_p = ctx.enter_context(tc.tile_pool(name="consts", bufs=1))
        onescol = const_p.tile([128, 1], F32R)
        onesrow = const_p.tile([1, 128], F32R)
        ident = const_p.tile([128, 128], BF16)
        nc.gpsimd.dma_start(onescol[:], ones_d[:, 0:1])
        nc.gpsimd.dma_start(onesrow[:], ones_d[0:1, :])
        ones_bf = const_p.tile([1, 128], BF16)
        nc.vector.memset(ones_bf[:], 1.0)
        masks.make_identity(nc, ident[:])

        xs_p = ctx.enter_context(tc.tile_pool(name="xstate", bufs=1))
        xA = [xs_p.tile([128, TOK], F32R, tag=f"xA{m}", name=f"xA{m}") for m in range(4)]

        stat_p = ctx.enter_context(tc.tile_pool(name="stats", bufs=1))
        dram_p = ctx.enter_context(tc.tile_pool(name="dramb", bufs=1, space="DRAM"))

        pp_mm = ctx.enter_context(tc.tile_pool(name="ppmm", bufs=4, space="PSUM"))
        pp_row = ctx.enter_context(tc.tile_pool(name="pprow", bufs=1, space="PSUM"))
        pp_bc = ctx.enter_context(tc.tile_pool(name="ppbc", bufs=2, space="PSUM"))

        y_dram = [dram_p.tile([CONV_DIMS[i], TOK], F32R, tag=f"ydram{i}", name=f"ydram{i}")
                  for i in range(1, 4)]

        # ------------------------------------------------------------------
        # conv stack (activations spilled to DRAM, BN applied on load)
        # ------------------------------------------------------------------
        b4sb = stat_p.tile([128, 4 * NF], F32, tag="b4")
        nc.sync.dma_start(b4sb[:], bias4[:])

        bn_s = {}
        bn_t = {}

        def conv_layer(li, wcp, cxp, pp_conv):
            kdim, mdim = CONV_DIMS[li - 1], CONV_DIMS[li]
            KC, MC = kdim // 128, mdim // 128
            src = xin if li == 1 else y_dram[li - 2]
            with_bn = li < 4
            if with_bn:
                sum_acc = stat_p.tile([128, MC * 4], F32, tag=f"sum{li}")
                sq_acc = stat_p.tile([128, MC * 4], F32, tag=f"sq{li}")
            cond_sb = None
            if li == 1:
                # channels 0:1024 repeat frame 0's features for all 4 frames;
                # compute their contribution once and add it at eviction
                KC = 8
                xc = cxp.tile([128, 8 * 512], F32R, tag="convc", name="convc",
                              bufs=1)
                nc.sync.dma_start(
                    xc[:].rearrange("p (k c) -> p k c", k=8),
                    src[0:1024, 0:512].rearrange("(k p) c -> p k c", p=128))
                cond_sb = [cxp.tile([128, 512], F32, tag=f"cond{m}",
                                    name=f"cond{m}") for m in range(MC)]
                for m in range(MC):
                    wslc = wcp.tile([128, 8 * 128], F32R, tag="wslc",
                                    name="wslc", bufs=3)
                    wvc = wslc[:].rearrange("p (k c) -> p k c", k=8)
                    nc.sync.dma_start(
                        wvc, convw[0][0:1024, m * 128:(m + 1) * 128]
                        .rearrange("(k p) c -> p k c", p=128))
                    ps = pp_conv.tile([128, 512], F32, tag="mm", name="ccps")
                    for k in range(8):
                        nc.tensor.matmul(
                            ps[:], wvc[:, k, :], xc[:, k * 512:(k + 1) * 512],
                            start=(k == 0), stop=(k == 7))
                    nc.scalar.copy(cond_sb[m][:], ps[:])
            for pt in range(4):
                xt = cxp.tile([128, KC * 512], F32R, tag="convx", name="convx",
                              bufs=3)
                nc.sync.dma_start(
                    xt[:].rearrange("p (k c) -> p k c", k=KC),
                    src[kdim - KC * 128:, pt * 512:(pt + 1) * 512]
                    .rearrange("(k p) c -> p k c", p=128))
                if li > 1:
                    s_p, t_p = bn_s[li - 1], bn_t[li - 1]
                    for k in range(KC):
                        nc.scalar.activation(
                            xt[:, k * 512:(k + 1) * 512],
                            xt[:, k * 512:(k + 1) * 512],
                            AF.Relu, bias=t_p[:, k:k + 1], scale=s_p[:, k:k + 1])
                for m in range(MC):
                    wsl = wcp.tile([128, KC * 128], F32R, tag="wsl", name="wsl",
                                   bufs=6)
                    wv = wsl[:].rearrange("p (k c) -> p k c", k=KC)
                    nc.sync.dma_start(
                        wv, convw[li - 1][kdim - KC * 128:,
                                          m * 128:(m + 1) * 128]
                        .rearrange("(k p) c -> p k c", p=128))
                    ps = pp_conv.tile([128, 512], F32, tag="mm", name="cps")
                    for k in range(KC):
                        nc.tensor.matmul(
                            ps[:], wv[:, k, :], xt[:, k * 512:(k + 1) * 512],
                            start=(k == 0), stop=(k == KC - 1))
                    if with_bn:
                        ot = cxp.tile([128, 512], F32R, tag="convot", name="cot",
                                      bufs=4)
                        if cond_sb is not None:
                            nc.vector.tensor_tensor(ot[:], ps[:],
                                                    cond_sb[m][:], op=ALU.add)
                            stats_src = ot[:]
                        else:
                            stats_src = ps[:]
                            nc.scalar.activation(
                                ot[:], ps[:], AF.Copy,
                                accum_out=sum_acc[:, m * 4 + pt:m * 4 + pt + 1])
                        sqs = cxp.tile([128, 512], BF16, tag="sqscr", name="sqs",
                                       bufs=4)
                        if cond_sb is not None:
                            nc.scalar.activation(
                                sqs[:], stats_src, AF.Copy,
                                accum_out=sum_acc[:, m * 4 + pt:m * 4 + pt + 1])
                        nc.scalar.activation(
                            sqs[:], stats_src, AF.Square,
                            accum_out=sq_acc[:, m * 4 + pt:m * 4 + pt + 1])
                        nc.sync.dma_start(
                            y_dram[li - 1][m * 128:(m + 1) * 128,
                                           pt * 512:(pt + 1) * 512], ot[:])
                    else:
                        nc.scalar.activation(
                            xA[m][:, pt * 512:(pt + 1) * 512], ps[:], AF.Identity,
                            bias=b4sb[:, m * 4 + pt:m * 4 + pt + 1])
            if not with_bn:
                return
            allin = stat_p.tile([128, 2 * MC], F32, tag=f"ain{li}", name="allin")
            nc.vector.tensor_reduce(
                allin[:, 0:MC], sum_acc[:].rearrange("p (m t) -> p m t", m=MC),
                axis=AX.X, op=ALU.add)
            nc.vector.tensor_reduce(
                allin[:, MC:2 * MC], sq_acc[:].rearrange("p (m t) -> p m t", m=MC),
                axis=AX.X, op=ALU.add)
            bin_ = dram_p.tile([128, 2 * MC], F32, tag=f"arin{li}", name="arin")
            bout = dram_p.tile([128, 2 * MC], F32, tag=f"arout{li}", name="arout")
            nc.sync.dma_start(bin_[:], allin[:])
            nc.gpsimd.collective_compute(
                "AllReduce", ALU.add, replica_groups=[list(range(NCORES))],
                ins=[bin_.opt()], outs=[bout.opt()])
            gl = stat_p.tile([128, 2 * MC], F32, tag=f"gl{li}", name="gl")
            nc.sync.dma_start(gl[:], bout[:])
            cst = stat_p.tile([128, 3 * MC], F32, tag=f"cst{li}", name="cst")
            nc.sync.dma_start(cst[:], bnconst[li - 1][:])
            mu = stat_p.tile([128, MC], F32, tag=f"mu{li}", name="bmu")
            var = stat_p.tile([128, MC], F32, tag=f"va{li}", name="bvar")
            s_t = stat_p.tile([128, MC], F32, tag=f"s{li}", name="bs")
            t_t = stat_p.tile([128, MC], F32, tag=f"t{li}", name="bt")
            nc.scalar.mul(mu[:], gl[:, 0:MC], 1.0 / NBN)
            nc.scalar.mul(var[:], gl[:, MC:2 * MC], 1.0 / NBN)
            msq = stat_p.tile([128, MC], F32, tag=f"ms{li}", name="bmsq")
            nc.vector.tensor_mul(msq[:], mu[:], mu[:])
            nc.vector.tensor_tensor(var[:], var[:], msq[:], op=ALU.subtract)
            nc.vector.tensor_scalar(var[:], var[:], BNEPS, None, op0=ALU.add)
            sd = stat_p.tile([128, MC], F32, tag=f"sd{li}", name="bsd")
            nc.scalar.activation(sd[:], var[:], AF.Sqrt)
            rsd = stat_p.tile([128, MC], F32, tag=f"rs{li}", name="brsd")
            nc.vector.reciprocal(rsd[:], sd[:])
            nc.vector.tensor_mul(s_t[:], rsd[:], cst[:, 0:MC])
            nc.vector.tensor_mul(t_t[:], mu[:], s_t[:])
            nc.vector.tensor_tensor(t_t[:], cst[:, MC:2 * MC], t_t[:],
                                    op=ALU.subtract)
            bn_s[li], bn_t[li] = s_t, t_t

        with tc.tile_pool(name="wcp", bufs=1) as wcp, \
             tc.tile_pool(name="cxp", bufs=1) as cxp:
            for li in (1, 2, 3, 4):
                conv_layer(li, wcp, cxp, pp_mm)

        # ------------------------------------------------------------------
        # transformer
        # ------------------------------------------------------------------
        rows_p = ctx.enter_context(tc.tile_pool(name="rows", bufs=1))
        scr = ctx.enter_context(tc.tile_pool(name="scratch", bufs=2))

        def ln_cols(xt, xview, dst_tiles, dst_cols):
            """LN per token over feature dim (stats + apply on DVE/Pool)."""
            ps_s = pp_row.tile([1, 512], F32, tag="row_s", name="ps_s")
            ps_q = pp_row.tile([1, 512], F32, tag="row_q", name="ps_q")
            for k in range(4):
                nc.tensor.matmul(ps_s[:], onescol[:], xview(k),
                                 start=(k == 0), stop=(k == 3))
            for k in range(4):
                sq = scr.tile([128, 512], F32R, tag="lnsq", name="lnsq")
                eng = nc.vector if k % 2 else nc.gpsimd
                eng.tensor_mul(sq[:], xview(k), xview(k))
                nc.tensor.matmul(ps_q[:], onescol[:], sq[:],
                                 start=(k == 0), stop=(k == 3))
            mu = rows_p.tile([1, 512], F32R, tag="mu", name="lmu", bufs=2)
            e2 = rows_p.tile([1, 512], F32, tag="e2", name="le2", bufs=2)
            r = rows_p.tile([1, 512], F32R, tag="r", name="lr", bufs=2)
            nc.scalar.mul(mu[:], ps_s[:], 1.0 / D)
            nc.scalar.mul(e2[:], ps_q[:], 1.0 / D)
            with nc.allow_low_precision(reason="f32r row math"):
                nc.vector.tensor_mul(r[:], mu[:], mu[:])
                nc.vector.scalar_tensor_tensor(
                    e2[:], e2[:], LNEPS, r[:], op0=ALU.add, op1=ALU.subtract)
                # 1/sqrt(v) = exp(-0.5 ln v): stays inside the exp/ln table
                nc.scalar.activation(e2[:], e2[:], AF.Ln)
                nc.scalar.activation(r[:], e2[:], AF.Exp, scale=-0.5)
            psb_mu = pp_bc.tile([128, 512], F32, tag="bc", name="psbmu")
            psb_r = pp_bc.tile([128, 512], F32, tag="bc", name="psbr")
            nc.tensor.matmul(psb_mu[:], onesrow[:], mu[:], start=True, stop=True)
            nc.tensor.matmul(psb_r[:], onesrow[:], r[:], start=True, stop=True)
            for k in range(4):
                tmp = scr.tile([128, 512], F32, tag="lntmp", name="lntmp")
                nc.vector.tensor_tensor(tmp[:], xview(k), psb_mu[:],
                                        op=ALU.subtract)
                nc.vector.tensor_mul(dst_tiles[k][:, dst_cols], tmp[:], psb_r[:])

        def ln1_rows(x_in, nm, rcol_all):
            """Per-frame LN stats; negmu row (K=1 fold operand) + 1/sd cols."""
            for f in range(4):
                sl = slice(f * 512, (f + 1) * 512)
                ps_s = pp_row.tile([1, 512], F32, tag="row_s", name="ps_s")
                ps_q = pp_row.tile([1, 512], F32, tag="row_q", name="ps_q")
                for k in range(4):
                    nc.tensor.matmul(ps_s[:], onescol[:], x_in[k][:, sl],
                                     start=(k == 0), stop=(k == 3))
                for k in range(4):
                    sq = scr.tile([128, 512], F32R, tag="lnsq", name="lnsq")
                    eng = nc.vector if k % 2 else nc.gpsimd
                    eng.tensor_mul(sq[:], x_in[k][:, sl], x_in[k][:, sl])
                    nc.tensor.matmul(ps_q[:], onescol[:], sq[:],
                                     start=(k == 0), stop=(k == 3))
                e2 = rows_p.tile([1, 512], F32, tag="e2", name="le2", bufs=2)
                rr = rows_p.tile([1, 512], F32, tag="rr", name="lrr", bufs=4)
                nc.scalar.mul(e2[:], ps_q[:], 1.0 / D)
                with nc.allow_low_precision(reason="ln1 rows"):
                    nc.scalar.mul(nm[0:1, sl], ps_s[:], -1.0 / D)
                    msq = rows_p.tile([1, 512], F32, tag="rr", name="lms", bufs=4)
                    nc.vector.tensor_mul(msq[:], nm[0:1, sl], nm[0:1, sl])
                    nc.vector.scalar_tensor_tensor(
                        e2[:], e2[:], LNEPS, msq[:], op0=ALU.add,
                        op1=ALU.subtract)
                    nc.scalar.activation(e2[:], e2[:], AF.Ln)
                    nc.scalar.activation(rr[:], e2[:], AF.Exp, scale=-0.5)
                for st in range(4):
                    nc.sync.dma_start(
                        rcol_all[:, f * 4 + st:f * 4 + st + 1],
                        rr[0:1, st * 128:(st + 1) * 128])

        tr_ctx = ctx.enter_context(contextlib.ExitStack())
        wp = tr_ctx.enter_context(tc.tile_pool(name="wp", bufs=1))
        wqp = tr_ctx.enter_context(tc.tile_pool(name="wqp", bufs=1))
        attn_p = tr_ctx.enter_context(tc.tile_pool(name="attn", bufs=2))
        sl_p = tr_ctx.enter_context(tc.tile_pool(name="slices", bufs=1))
        ot_p = tr_ctx.enter_context(tc.tile_pool(name="otp", bufs=1))
        otb_all = ot_p.tile([128, 4 * TOK], BF16, tag="otall", name="otall")

        def st_view(xt, k, st):
            # scattered columns {f*512 + st*128 + p} as (128, (f,p)=512)
            return xt[k][:].rearrange("p (f s) -> p f s", f=4)[:, :, st * 128:(st + 1) * 128]

        def transformer_layer(li, x_in, x_mid):
            wq = [wqp.tile([128, 3 * D], F32R, tag=f"wqkv{k}", name=f"wq{k}")
                  for k in range(4)]
            for k in range(4):
                nc.sync.dma_start(wq[k][:], wqkv_d[li, k * 128:(k + 1) * 128, :])
            ws_t = rows_p.tile([1, 3 * D], BF16, tag="wsum", name="wst", bufs=1)
            nc.sync.dma_start(ws_t[:], wsbq_d[li, 0:1, :])
            bq_t = rows_p.tile([1, 3 * D], BF16, tag="bqr", name="bqt", bufs=1)
            nc.sync.dma_start(bq_t[:], wsbq_d[li, 1:2, :])
            cols = stat_p.tile([128, 12], F32, tag="tcols", name="tcols")
            nc.sync.dma_start(cols[:], tcols_d[li])

            # q bias broadcast over the token partitions (k-bias is
            # softmax-invariant; v-bias is folded into outb on the host)
            bias_bc = attn_p.tile([128, D], BF16, tag="biasbc",
                                  name="bias_bc", bufs=1)
            psb = pp_bc.tile([128, 512], F32, tag="bc", name="psbb")
            nc.tensor.matmul(psb[:], ones_bf[:], bq_t[:, 0:512],
                             start=True, stop=True)
            nc.scalar.copy(bias_bc[:], psb[:])

            nm = attn_p.tile([1, TOK], BF16, tag="nmsd", name="nm", bufs=1)
            rcol = attn_p.tile([128, 16], F32, tag="rcol", name="rcol", bufs=1)
            ln1_rows(x_in, nm, rcol)

            for st in range(4):
                qt = attn_p.tile([128, TOK], BF16, tag="qst", name="qt", bufs=2)
                kt = attn_p.tile([128, TOK], BF16, tag="kst", name="kt", bufs=2)
                # v stored (j, d, h) so the AV multiply hits the 2x DVE mode
                vt = attn_p.tile([128, TOK], BF16, tag="vst", name="vt", bufs=2)
                for f in range(NF):
                    c0 = f * 512 + st * 128
                    for ns in range(3):
                        ps = pp_mm.tile([128, 512], F32, tag="mm", name="qps")
                        for k in range(4):
                            nc.tensor.matmul(
                                ps[:], x_in[k][:, c0:c0 + 128],
                                wq[k][:, ns * 512:(ns + 1) * 512],
                                start=(k == 0), stop=False)
                        nc.tensor.matmul(ps[:], nm[0:1, c0:c0 + 128],
                                         ws_t[:, ns * 512:(ns + 1) * 512],
                                         start=False, stop=True)
                        rc = rcol[:, f * 4 + st:f * 4 + st + 1]
                        if ns < 2:
                            nc.scalar.activation(
                                (qt if ns == 0 else kt)[:, f * 512:(f + 1) * 512],
                                ps[:], AF.Copy, scale=rc)
                        else:
                            nc.scalar.activation(
                                vt[:, f * 512:(f + 1) * 512]
                                .rearrange("p (d h) -> p h d", h=8),
                                ps[:].rearrange("p (h d) -> p h d", h=8),
                                AF.Copy, scale=rc)
                # q bias, broadcast over frames, one 2x-mode op per st
                nc.vector.tensor_tensor(
                    qt[:].rearrange("p (f c) -> p f c", f=4),
                    qt[:].rearrange("p (f c) -> p f c", f=4),
                    bias_bc[:].unsqueeze(1).broadcast_to([128, 4, 512]),
                    op=ALU.add)

                s_sc = attn_p.tile([128, 128], F32, tag="s_sc", name="s_sc",
                                   bufs=2)
                k4 = kt[:].rearrange("p (j hd) -> p j hd", j=4)
                for i in range(4):
                    pbig = attn_p.tile([128, TOK], BF16, tag="pbig",
                                       name="pbig", bufs=2)
                    qi = qt[:, i * 512:(i + 1) * 512].unsqueeze(1) \
                        .broadcast_to([128, 4, 512])
                    nc.vector.tensor_mul(
                        pbig[:].rearrange("p (j hd) -> p j hd", j=4), qi, k4)
                    st1 = attn_p.tile([128, TOK // 2], BF16, tag="qks1",
                                      name="qks1", bufs=2)
                    st2 = attn_p.tile([128, TOK // 4], BF16, tag="qks2",
                                      name="qks2", bufs=2)
                    pv = pbig[:].rearrange("p (g d) -> p g d", g=32)
                    nc.vector.tensor_tensor(
                        st1[:].rearrange("p (g d) -> p g d", g=32),
                        pv[:, :, 0:32], pv[:, :, 32:64], op=ALU.add)
                    sv = st1[:].rearrange("p (g d) -> p g d", g=32)
                    nc.vector.tensor_tensor(
                        st2[:].rearrange("p (g d) -> p g d", g=32),
                        sv[:, :, 0:16], sv[:, :, 16:32], op=ALU.add)
                    nc.vector.tensor_reduce(
                        s_sc[:, i * 32:(i + 1) * 32],
                        st2[:].rearrange("p (g d) -> p g d", g=32),
                        axis=AX.X, op=ALU.add)
                # softmax over j without max-subtraction (logits bounded)
                # S cols = (i, j, h)
                eexp = attn_p.tile([128, 128], BF16, tag="eexp", name="eexp",
                                   bufs=2)
                nc.scalar.activation(eexp[:], s_sc[:], AF.Exp)
                z = attn_p.tile([128, 32], F32, tag="z", name="zt", bufs=2)
                nc.vector.tensor_reduce(
                    z[:].rearrange("p (i h) -> p i h", i=4),
                    eexp[:].rearrange("p (i j h) -> p i h j", i=4, j=4),
                    axis=AX.X, op=ALU.add)
                zr = attn_p.tile([128, 32], F32, tag="zr", name="zr", bufs=2)
                nc.vector.reciprocal(zr[:], z[:])
                a_t = attn_p.tile([128, 128], BF16, tag="a_t", name="a_t",
                                  bufs=2)
                nc.vector.tensor_mul(
                    a_t[:].rearrange("p (i j h) -> p i j h", i=4, j=4),
                    eexp[:].rearrange("p (i j h) -> p i j h", i=4, j=4),
                    zr[:].rearrange("p (i h) -> p i h", i=4).unsqueeze(2)
                    .broadcast_to([128, 4, 4, 8]))
                for i in range(4):
                    tbig = attn_p.tile([128, TOK], BF16, tag="tbig", name="tbig", bufs=2)
                    ablk = a_t[:, i * 32:(i + 1) * 32] \
                        .rearrange("p (j h) -> p j h", j=4) \
                        .unsqueeze(2).broadcast_to([128, 4, 64, 8])
                    nc.vector.tensor_mul(
                        tbig[:].rearrange("p (j d h) -> p j d h", j=4, d=64),
                        vt[:].rearrange("p (j d h) -> p j d h", j=4, d=64),
                        ablk)
                    av01 = attn_p.tile([128, 512], BF16, tag="av01", name="av01",
                                       bufs=2)
                    av = attn_p.tile([128, 512], BF16, tag="av", name="av",
                                     bufs=2)
                    nc.vector.tensor_tensor(av01[:], tbig[:, 0:512],
                                            tbig[:, 512:1024], op=ALU.add)
                    nc.vector.tensor_tensor(av[:], tbig[:, 1024:1536],
                                            tbig[:, 1536:2048], op=ALU.add)
                    nc.gpsimd.tensor_tensor(av[:], av01[:], av[:],
                                            op=ALU.add)
                    pst = pp_bc.tile([128, 512], BF16, tag="bc", name="pst")
                    for c in range(4):
                        nc.tensor.transpose(pst[:, c * 128:(c + 1) * 128],
                                            av[:, c * 128:(c + 1) * 128],
                                            ident[:])
                    nc.scalar.copy(
                        otb_all[:].rearrange("p (c t) -> p c t", c=4)
                        [:, :, i * 512 + st * 128:i * 512 + st * 128 + 128],
                        pst[:].rearrange("p (c t) -> p c t", c=4))

            wo = [wp.tile([128, D], BF16, tag=f"wo{k}", name=f"wo{k}")
                  for k in range(4)]
            for k in range(4):
                nc.sync.dma_start(wo[k][:], wo_d[li, k * 128:(k + 1) * 128, :])
            for m in range(4):
                for ns in range(4):
                    ps = pp_mm.tile([128, 512], F32, tag="mm", name="ops")
                    for k in range(4):
                        nc.tensor.matmul(
                            ps[:], wo[k][:, m * 128:(m + 1) * 128],
                            otb_all[:, k * TOK + ns * 512:k * TOK + (ns + 1) * 512],
                            start=(k == 0), stop=(k == 3))
                    nc.vector.scalar_tensor_tensor(
                        x_mid[m][:, ns * 512:(ns + 1) * 512], ps[:],
                        cols[:, 4 + m:5 + m], x_in[m][:, ns * 512:(ns + 1) * 512],
                        op0=ALU.add, op1=ALU.add)

            w1 = [wp.tile([128, D], F32R, tag=f"w1_{k}", name=f"w1_{k}")
                  for k in range(4)]
            w2 = [wp.tile([128, D], F32R, tag=f"w2_{k}", name=f"w2_{k}")
                  for k in range(4)]
            for k in range(4):
                nc.sync.dma_start(w1[k][:], w1_d[li, k * 128:(k + 1) * 128, :])
                nc.sync.dma_start(w2[k][:], w2_d[li, k * 128:(k + 1) * 128, :])
            for ns in range(4):
                xh2 = [sl_p.tile([128, 512], F32R, tag=f"xh2_{k}", name=f"xh2_{k}")
                       for k in range(4)]
                ln_cols(x_mid,
                        lambda k: x_mid[k][:, ns * 512:(ns + 1) * 512],
                        xh2, slice(0, 512))
                hsl = [sl_p.tile([128, 512], F32R, tag=f"h_{m}", name=f"hsl{m}")
                       for m in range(4)]
                for m in range(4):
                    ps = pp_mm.tile([128, 512], F32, tag="mm", name="m1ps")
                    for k in range(4):
                        nc.tensor.matmul(
                            ps[:], w1[k][:, m * 128:(m + 1) * 128], xh2[k][:],
                            start=(k == 0), stop=(k == 3))
                    nc.scalar.activation(hsl[m][:], ps[:], AF.Gelu_apprx_tanh,
                                         bias=cols[:, m:m + 1])
                for m in range(4):
                    ps = pp_mm.tile([128, 512], F32, tag="mm", name="m2ps")
                    for k in range(4):
                        nc.tensor.matmul(
                            ps[:], w2[k][:, m * 128:(m + 1) * 128], hsl[k][:],
                            start=(k == 0), stop=(k == 3))
                    nc.vector.scalar_tensor_tensor(
                        x_mid[m][:, ns * 512:(ns + 1) * 512], ps[:],
                        cols[:, 8 + m:9 + m], x_mid[m][:, ns * 512:(ns + 1) * 512],
                        op0=ALU.add, op1=ALU.add)

        cur = xA
        _nl = int(os.environ.get("KNLAYERS", NLAYER))
        for li in range(_nl):
            transformer_layer(li, cur, cur)

        tr_ctx.close()

        # ------------------------------------------------------------------
        # heads
        # ------------------------------------------------------------------
        _skip_heads = os.environ.get("KHEADS", "1") == "0"
        if _skip_heads:
            nc.gpsimd.dma_start(d6_o[:], cur[0][0:8, :])
            nc.gpsimd.dma_start(tr_o[:], cur[1][0:8, :])
        with tc.tile_pool(name="heads", bufs=1) as hp, \
             tc.tile_pool(name="whp", bufs=1) as whp:
          if not _skip_heads:
              hc = stat_p.tile([128, 10], F32, tag="hcols", name="hc")
              nc.sync.dma_start(hc[:], hcols_d[:])
              sb3 = stat_p.tile([8, 2], F32, tag="sb3", name="sb3")
              nc.sync.dma_start(sb3[:], sb3_d[:])

              xhf = [hp.tile([128, TOK], F32R, tag=f"xhf{k}", name=f"xhf{k}")
                     for k in range(4)]
              for st in range(4):
                  ln_cols(cur, lambda k: cur[k][:, st * 512:(st + 1) * 512],
                          xhf, slice(st * 512, (st + 1) * 512))

              xp = [hp.tile([128, TOK], F32R, tag=f"xp{k}", name=f"xp{k}")
                    for k in range(4)]

              def mm_head(src_tiles, wt_dram, kdim, mdim, dst_tiles, evict):
                  KC = kdim // 128
                  MC = max(mdim // 128, 1)
                  wsb = [whp.tile([128, mdim], F32R, tag=f"wh_{kdim}_{mdim}_{k}",
                                  name=f"wh{k}") for k in range(KC)]
                  for k in range(KC):
                      nc.sync.dma_start(wsb[k][:], wt_dram[k * 128:(k + 1) * 128, :])
                  for m in range(MC):
                      for ns in range(4):
                          ps = pp_mm.tile([128, 512], F32, tag="mm", name="hps")
                          for k in range(KC):
                              nc.tensor.matmul(
                                  ps[:], wsb[k][:, m * 128:(m + 1) * 128],
                                  src_tiles[k][:, ns * 512:(ns + 1) * 512],
                                  start=(k == 0), stop=(k == KC - 1))
                          evict(ps, dst_tiles[m], m, ns)

              mm_head(xhf, projw_d, D, D, xp,
                      lambda ps, dst, m, ns: nc.scalar.activation(
                          dst[:, ns * 512:(ns + 1) * 512], ps[:], AF.Identity,
                          bias=hc[:, m:m + 1]))

              def branch(w1d, w2d, w3d, b1ofs, b2ofs, out_dram, b3col, r1, r2, pfx):
                  mm_head(xp, w1d, D, 256, r1,
                          lambda ps, dst, m, ns: nc.scalar.activation(
                              dst[:, ns * 512:(ns + 1) * 512], ps[:], AF.Relu,
                              bias=hc[:, b1ofs + m:b1ofs + m + 1]))
                  mm_head(r1, w2d, 256, 128, r2,
                          lambda ps, dst, m, ns: nc.scalar.activation(
                              dst[:, ns * 512:(ns + 1) * 512], ps[:], AF.Relu,
                              bias=hc[:, b2ofs:b2ofs + 1]))
                  w3 = whp.tile([128, 8], F32R, tag=f"w3{pfx}", name="w3")
                  nc.sync.dma_start(w3[:], w3d[:])
                  out_sb = hp.tile([8, TOK], F32, tag=f"{pfx}out", name=f"{pfx}out")
                  for ns in range(4):
                      ps = pp_mm.tile([8, 512], F32, tag="mm", name="bps")
                      nc.tensor.matmul(ps[:], w3[:], r2[0][:, ns * 512:(ns + 1) * 512],
                                       start=True, stop=True)
                      nc.scalar.activation(out_sb[:, ns * 512:(ns + 1) * 512], ps[:],
                                           AF.Identity, bias=b3col)
                  nc.sync.dma_start(out_dram[:], out_sb[:])

              # reuse dead transformer buffers for intermediates
              branch(rw1_d, rw2_d, rw3_d, 4, 8, d6_o, sb3[:, 0:1],
                     [cur[0], cur[1]], [xhf[0]], "r")
              branch(tw1_d, tw2_d, tw3_d, 6, 9, tr_o, sb3[:, 1:2],
                     [cur[2], cur[3]], [xhf[1]], "t")

    nc.compile()
    return nc


# ----------------------------------------------------------------------------
# host side
# ----------------------------------------------------------------------------

_CACHE = {}


def _normalize_np(v, eps=1e-12):
    return v / np.maximum(np.linalg.norm(v, axis=-1, keepdims=True), eps)


def _rot6d_np(d6):
    a1, a2 = d6[..., :3], d6[..., 3:]
    b1 = _normalize_np(a1)
    b2 = _normalize_np(a2 - np.sum(b1 * a2, -1, keepdims=True) * b1)
    b3 = np.cross(b1, b2)
    return np.stack([b1, b2, b3], axis=-2)


def _prep_weights(inp):
    f32 = np.float32
    wmap = {}
    for i, cw in enumerate(['c1w', 'c2w', 'c3w', 'c4w']):
        wmap[f'convw{i}'] = np.ascontiguousarray(inp[cw].T.astype(f32))
    for i, (g, b2) in enumerate([('bn1g', 'bn1b'), ('bn2g', 'bn2b'),
                                 ('bn3g', 'bn3b')]):
        M = CONV_DIMS[i + 1] // 128
        bn = np.concatenate([
            inp[g].reshape(M, 128).T, inp[b2].reshape(M, 128).T,
            inp[f'c{i + 1}b'].reshape(M, 128).T], axis=1)
        wmap[f'bnconst{i}'] = np.ascontiguousarray(bn.astype(f32))
    pe = _pe_table()[:NF]
    b4 = inp['c4b'][None, :].astype(f32) + pe                   # (4, 512)
    # cols: m*4 + pt ; frame index == pt
    wmap['bias4'] = np.ascontiguousarray(
        b4.reshape(NF, 4, 128).transpose(2, 1, 0).reshape(128, 16).astype(f32))

    qkvw = np.array(inp['qkvw'], f32)
    qkvb = np.array(inp['qkvb'], f32)
    qkvw[:, :, :512] /= math.sqrt(DH)
    qkvb[:, :512] /= math.sqrt(DH)
    g1 = np.array(inp['ln1g'], f32)
    b1 = np.array(inp['ln1b'], f32)
    wq_fold = g1[:, :, None] * qkvw
    bq_fold = qkvb + np.einsum('ld,ldf->lf', b1, qkvw)
    wmap['wqkv'] = np.ascontiguousarray(wq_fold.astype(f32))
    wsbq = np.concatenate([wq_fold.sum(axis=1, keepdims=True),
                           bq_fold[:, None, :]], axis=1)         # (L, 2, 3D)
    wmap['wsbq'] = np.ascontiguousarray(wsbq.astype(ml_dtypes.bfloat16))
    # attention output features are (d, h)-ordered; permute wo rows to match
    wo_ = np.array(inp['outw'], f32)                             # (L, 512, 512)
    d_idx, h_idx = np.meshgrid(np.arange(DH), np.arange(HEADS), indexing='ij')
    perm = (h_idx * DH + d_idx).reshape(512)     # perm[d*8+h] = h*64+d
    wmap['wo'] = np.ascontiguousarray(wo_[:, perm, :]
                                      .astype(ml_dtypes.bfloat16))
    g2 = np.array(inp['ln2g'], f32)
    bl2 = np.array(inp['ln2b'], f32)
    m1w = np.array(inp['m1w'], f32)
    w1_fold = g2[:, :, None] * m1w
    b1_fold = np.array(inp['m1b'], f32) + np.einsum('ld,ldf->lf', bl2, m1w)
    wmap['w1'] = np.ascontiguousarray(w1_fold.astype(f32))
    wmap['w2'] = np.ascontiguousarray(np.array(inp['m2w'], f32))
    cols = np.zeros((NLAYER, 128, 12), f32)
    cols[:, :, 0:4] = b1_fold.reshape(NLAYER, 4, 128).transpose(0, 2, 1)
    # v-bias is dropped at the v eviction; fold bv @ Wo into outb instead
    bv = bq_fold[:, 2 * 512:3 * 512]                             # (L, 512)
    outb_fold = np.array(inp['outb'], f32) + np.einsum('lk,lko->lo', bv, wo_)
    cols[:, :, 4:8] = outb_fold.reshape(NLAYER, 4, 128).transpose(0, 2, 1)
    cols[:, :, 8:12] = np.array(inp['m2b'], f32).reshape(NLAYER, 4, 128) \
        .transpose(0, 2, 1)
    wmap['tcols'] = cols

    gf_ = np.array(inp['lnfg'], f32)
    bf_ = np.array(inp['lnfb'], f32)
    projw = np.array(inp['projw'], f32)
    wmap['projw'] = np.ascontiguousarray(gf_[:, None] * projw)
    projb_fold = np.array(inp['projb'], f32) + bf_ @ projw
    wmap['rw1'] = np.ascontiguousarray(np.array(inp['rw1'], f32))
    wmap['rw2'] = np.ascontiguousarray(np.array(inp['rw2'], f32))
    rw3 = np.zeros((128, 8), f32)
    rw3[:, :6] = np.array(inp['rw3'], f32)
    wmap['rw3'] = rw3
    wmap['tw1'] = np.ascontiguousarray(np.array(inp['tw1'], f32))
    wmap['tw2'] = np.ascontiguousarray(np.array(inp['tw2'], f32))
    tw3 = np.zeros((128, 8), f32)
    tw3[:, :3] = np.array(inp['tw3'], f32)
    wmap['tw3'] = tw3
    hcols = np.zeros((128, 10), f32)
    hcols[:, 0:4] = projb_fold.reshape(4, 128).T
    hcols[:, 4:6] = np.array(inp['rb1'], f32).reshape(2, 128).T
    hcols[:, 6:8] = np.array(inp['tb1'], f32).reshape(2, 128).T
    hcols[:, 8] = np.array(inp['rb2'], f32)
    hcols[:, 9] = np.array(inp['tb2'], f32)
    wmap['hcols'] = hcols
    sb3 = np.zeros((8, 2), f32)
    sb3[0:6, 0] = np.array(inp['rb3'], f32)
    sb3[0:3, 1] = np.array(inp['tb3'], f32)
    wmap['sb3'] = sb3
    wmap['ones_c'] = np.ones((128, 128), f32)
    return wmap


def kernel(**inputs):
    inp = {k: np.asarray(v) for k, v in inputs.items()}

    idx = inp['seed_idxs'].reshape(B, -1).astype(np.int64)      # (B, N)
    sel_seed = np.take_along_axis(np.asarray(inp['fp2_features'], np.float32),
                                  idx[:, None, :], axis=2)
    sel_grasp = np.take_along_axis(np.asarray(inp['local_grasp_features'], np.float32),
                                   idx[:, None, :], axis=2)
    sel_color = np.take_along_axis(np.asarray(inp['local_color_features'], np.float32),
                                   idx[:, None, :], axis=2)
    sel_pose = np.take_along_axis(np.asarray(inp['grasp_pose_feature'], np.float32),
                                  idx[:, None, :], axis=2)
    gsf = np.asarray(inp['sa4_features'], np.float32).max(axis=-1)
    gsf = np.broadcast_to(gsf[:, :, None], (B, 256, NPTS))
    fused = sel_pose + np.concatenate([sel_grasp, sel_color, sel_seed, gsf], axis=1)
    gf = fused.reshape(BE, FRAME, 1024, NPTS)
    cond = np.broadcast_to(gf[:, :1], (BE, NF, 1024, NPTS))
    X = np.concatenate([cond, gf[:, 1:]], axis=2)               # (e, f, 2048, N)

    if 'nc' not in _CACHE:
        _CACHE['nc'] = build_kernel()
    nc = _CACHE['nc']
    wmap = _prep_weights(inp)

    in_maps = []
    for k in range(NCORES):
        xc = X[:, :, :, k * NPC:(k + 1) * NPC]                  # (e, f, c, n)
        xc = xc.transpose(2, 1, 0, 3).reshape(2048, TOK)        # (c, (f,e,n))
        m = dict(wmap)
        m['xin'] = np.ascontiguousarray(xc, dtype=np.float32)
        in_maps.append(m)

    res = run_bass_kernel_spmd(nc, in_maps, core_ids=list(range(NCORES)))

    out = np.zeros((BE * NPTS, NF, 12), np.float32)
    for k in range(NCORES):
        d6 = res.results[k]['d6'][:6]
        tr = res.results[k]['tr3'][:3]
        d6 = d6.reshape(6, NF, BE, NPC).transpose(2, 3, 1, 0)   # (e, n, f, 6)
        tr = tr.reshape(3, NF, BE, NPC).transpose(2, 3, 1, 0)
        rot = _rot6d_np(d6).reshape(BE, NPC, NF, 9)
        for e in range(BE):
            rows = slice(e * NPTS + k * NPC, e * NPTS + (k + 1) * NPC)
            out[rows, :, 0:3] = tr[e]
            out[rows, :, 3:12] = rot[e]
    return out


if __name__ == "__main__":
    build_kernel()
    print("built ok")



# revision 69
# speedup vs baseline: 1.0976x; 1.0095x over previous
import math
import os
import numpy as np
import ml_dtypes
import contextlib

import concourse.bass as bass
import concourse.tile as tile
from concourse import bacc, mybir, masks
from concourse.bass_utils import run_bass_kernel_spmd

F32 = mybir.dt.float32
F32R = mybir.dt.float32r
BF16 = mybir.dt.bfloat16
ALU = mybir.AluOpType
AF = mybir.ActivationFunctionType
AX = mybir.AxisListType

NCORES = 8
FRAME = 5
NF = FRAME - 1
D = 512
DH = 64
HEADS = 8
B = 20
NPTS = 1024
BE = B // FRAME
NPC = NPTS // NCORES     # 128 points per core
TOK = NF * BE * NPC      # 2048 tokens per core
NLAYER = 12
LNEPS = 1e-5
BNEPS = 1e-5
NBN = 16 * NPTS

CONV_DIMS = [2048, 1536, 1024, 768, 512]


def _pe_table(max_len=16, d=D):
    pos = np.arange(max_len, dtype=np.float32)[:, None]
    div = np.exp(np.arange(0, d, 2, dtype=np.float32) * (-math.log(10000.0) / d))
    pe = np.zeros((max_len, d), np.float32)
    pe[:, 0::2] = np.sin(pos * div)
    pe[:, 1::2] = np.cos(pos * div)
    return pe


def build_kernel():
    nc = bacc.Bacc("TRN2", target_bir_lowering=False, debug=False,
                   num_devices=NCORES)

    xin = nc.dram_tensor("xin", [CONV_DIMS[0], TOK], F32R, kind="ExternalInput").ap()
    convw = [nc.dram_tensor(f"convw{i}", [CONV_DIMS[i], CONV_DIMS[i + 1]], F32R,
                            kind="ExternalInput").ap() for i in range(4)]
    bnconst = [nc.dram_tensor(f"bnconst{i}", [128, 3 * (CONV_DIMS[i + 1] // 128)],
                              F32, kind="ExternalInput").ap() for i in range(3)]
    bias4 = nc.dram_tensor("bias4", [128, 4 * NF], F32, kind="ExternalInput").ap()

    wqkv_d = nc.dram_tensor("wqkv", [NLAYER, D, 3 * D], F32R, kind="ExternalInput").ap()
    wsbq_d = nc.dram_tensor("wsbq", [NLAYER, 2, 3 * D], BF16, kind="ExternalInput").ap()
    wo_d = nc.dram_tensor("wo", [NLAYER, D, D], BF16, kind="ExternalInput").ap()
    w1_d = nc.dram_tensor("w1", [NLAYER, D, D], F32R, kind="ExternalInput").ap()
    w2_d = nc.dram_tensor("w2", [NLAYER, D, D], F32R, kind="ExternalInput").ap()
    tcols_d = nc.dram_tensor("tcols", [NLAYER, 128, 12], F32, kind="ExternalInput").ap()

    projw_d = nc.dram_tensor("projw", [D, D], F32R, kind="ExternalInput").ap()
    rw1_d = nc.dram_tensor("rw1", [D, 256], F32R, kind="ExternalInput").ap()
    rw2_d = nc.dram_tensor("rw2", [256, 128], F32R, kind="ExternalInput").ap()
    rw3_d = nc.dram_tensor("rw3", [128, 8], F32R, kind="ExternalInput").ap()
    tw1_d = nc.dram_tensor("tw1", [D, 256], F32R, kind="ExternalInput").ap()
    tw2_d = nc.dram_tensor("tw2", [256, 128], F32R, kind="ExternalInput").ap()
    tw3_d = nc.dram_tensor("tw3", [128, 8], F32R, kind="ExternalInput").ap()
    hcols_d = nc.dram_tensor("hcols", [128, 10], F32, kind="ExternalInput").ap()
    sb3_d = nc.dram_tensor("sb3", [8, 2], F32, kind="ExternalInput").ap()
    ones_d = nc.dram_tensor("ones_c", [128, 128], F32, kind="ExternalInput").ap()

    d6_o = nc.dram_tensor("d6", [8, TOK], F32, kind="ExternalOutput").ap()
    tr_o = nc.dram_tensor("tr3", [8, TOK], F32, kind="ExternalOutput").ap()

    with tile.TileContext(nc) as tc, contextlib.ExitStack() as ctx:
        const_p = ctx.enter_context(tc.tile_pool(name="consts", bufs=1))
        onescol = const_p.tile([128, 1], F32R)
        onesrow = const_p.tile([1, 128], F32R)
        ident = const_p.tile([128, 128], BF16)
        nc.gpsimd.dma_start(onescol[:], ones_d[:, 0:1])
        nc.gpsimd.dma_start(onesrow[:], ones_d[0:1, :])
        ones_bf = const_p.tile([1, 128], BF16)
        nc.vector.memset(ones_bf[:], 1.0)
        eps_c = const_p.tile([1, 1], F32)
        nc.vector.memset(eps_c[:], LNEPS)
        masks.make_identity(nc, ident[:])

        xs_p = ctx.enter_context(tc.tile_pool(name="xstate", bufs=1))
        xA = [xs_p.tile([128, TOK], F32R, tag=f"xA{m}", name=f"xA{m}") for m in range(4)]

        stat_p = ctx.enter_context(tc.tile_pool(name="stats", bufs=1))
        dram_p = ctx.enter_context(tc.tile_pool(name="dramb", bufs=1, space="DRAM"))

        pp_mm = ctx.enter_context(tc.tile_pool(name="ppmm", bufs=4, space="PSUM"))
        pp_row = ctx.enter_context(tc.tile_pool(name="pprow", bufs=1, space="PSUM"))
        pp_bc = ctx.enter_context(tc.tile_pool(name="ppbc", bufs=2, space="PSUM"))

        y_dram = [dram_p.tile([CONV_DIMS[i], TOK], F32R, tag=f"ydram{i}", name=f"ydram{i}")
                  for i in range(1, 4)]

        # ------------------------------------------------------------------
        # conv stack (activations spilled to DRAM, BN applied on load)
        # ------------------------------------------------------------------
        b4sb = stat_p.tile([128, 4 * NF], F32, tag="b4")
        nc.sync.dma_start(b4sb[:], bias4[:])

        bn_s = {}
        bn_t = {}

        def conv_layer(li, wcp, cxp, pp_conv):
            kdim, mdim = CONV_DIMS[li - 1], CONV_DIMS[li]
            KC, MC = kdim // 128, mdim // 128
            src = xin if li == 1 else y_dram[li - 2]
            with_bn = li < 4
            if with_bn:
                sum_acc = stat_p.tile([128, MC * 4], F32, tag=f"sum{li}")
                sq_acc = stat_p.tile([128, MC * 4], F32, tag=f"sq{li}")
            cond_sb = None
            if li == 1:
                # channels 0:1024 repeat frame 0's features for all 4 frames;
                # compute their contribution once and add it at eviction
                KC = 8
                xc = cxp.tile([128, 8 * 512], F32R, tag="convc", name="convc",
                              bufs=1)
                nc.sync.dma_start(
                    xc[:].rearrange("p (k c) -> p k c", k=8),
                    src[0:1024, 0:512].rearrange("(k p) c -> p k c", p=128))
                cond_sb = [cxp.tile([128, 512], F32, tag=f"cond{m}",
                                    name=f"cond{m}") for m in range(MC)]
                for m in range(MC):
                    wslc = wcp.tile([128, 8 * 128], F32R, tag="wslc",
                                    name="wslc", bufs=3)
                    wvc = wslc[:].rearrange("p (k c) -> p k c", k=8)
                    nc.sync.dma_start(
                        wvc, convw[0][0:1024, m * 128:(m + 1) * 128]
                        .rearrange("(k p) c -> p k c", p=128))
                    ps = pp_conv.tile([128, 512], F32, tag="mm", name="ccps")
                    for k in range(8):
                        nc.tensor.matmul(
                            ps[:], wvc[:, k, :], xc[:, k * 512:(k + 1) * 512],
                            start=(k == 0), stop=(k == 7))
                    nc.scalar.copy(cond_sb[m][:], ps[:])
            for pt in range(4):
                xt = cxp.tile([128, KC * 512], F32R, tag="convx", name="convx",
                              bufs=3)
                nc.sync.dma_start(
                    xt[:].rearrange("p (k c) -> p k c", k=KC),
                    src[kdim - KC * 128:, pt * 512:(pt + 1) * 512]
                    .rearrange("(k p) c -> p k c", p=128))
                if li > 1:
                    s_p, t_p = bn_s[li - 1], bn_t[li - 1]
                    for k in range(KC):
                        nc.scalar.activation(
                            xt[:, k * 512:(k + 1) * 512],
                            xt[:, k * 512:(k + 1) * 512],
                            AF.Relu, bias=t_p[:, k:k + 1], scale=s_p[:, k:k + 1])
                for m in range(MC):
                    wsl = wcp.tile([128, KC * 128], F32R, tag="wsl", name="wsl",
                                   bufs=6)
                    wv = wsl[:].rearrange("p (k c) -> p k c", k=KC)
                    nc.sync.dma_start(
                        wv, convw[li - 1][kdim - KC * 128:,
                                          m * 128:(m + 1) * 128]
                        .rearrange("(k p) c -> p k c", p=128))
                    ps = pp_conv.tile([128, 512], F32, tag="mm", name="cps")
                    for k in range(KC):
                        nc.tensor.matmul(
                            ps[:], wv[:, k, :], xt[:, k * 512:(k + 1) * 512],
                            start=(k == 0), stop=(k == KC - 1))
                    if with_bn:
                        ot = cxp.tile([128, 512], F32R, tag="convot", name="cot",
                                      bufs=4)
                        if cond_sb is not None:
                            nc.vector.tensor_tensor(ot[:], ps[:],
                                                    cond_sb[m][:], op=ALU.add)
                            stats_src = ot[:]
                        else:
                            stats_src = ps[:]
                            nc.scalar.activation(
                                ot[:], ps[:], AF.Copy,
                                accum_out=sum_acc[:, m * 4 + pt:m * 4 + pt + 1])
                        sqs = cxp.tile([128, 512], BF16, tag="sqscr", name="sqs",
                                       bufs=4)
                        if cond_sb is not None:
                            nc.scalar.activation(
                                sqs[:], stats_src, AF.Copy,
                                accum_out=sum_acc[:, m * 4 + pt:m * 4 + pt + 1])
                        nc.scalar.activation(
                            sqs[:], stats_src, AF.Square,
                            accum_out=sq_acc[:, m * 4 + pt:m * 4 + pt + 1])
                        nc.sync.dma_start(
                            y_dram[li - 1][m * 128:(m + 1) * 128,
                                           pt * 512:(pt + 1) * 512], ot[:])
                    else:
                        nc.scalar.activation(
                            xA[m][:, pt * 512:(pt + 1) * 512], ps[:], AF.Identity,
                            bias=b4sb[:, m * 4 + pt:m * 4 + pt + 1])
            if not with_bn:
                return
            allin = stat_p.tile([128, 2 * MC], F32, tag=f"ain{li}", name="allin")
            nc.vector.tensor_reduce(
                allin[:, 0:MC], sum_acc[:].rearrange("p (m t) -> p m t", m=MC),
                axis=AX.X, op=ALU.add)
            nc.vector.tensor_reduce(
                allin[:, MC:2 * MC], sq_acc[:].rearrange("p (m t) -> p m t", m=MC),
                axis=AX.X, op=ALU.add)
            bin_ = dram_p.tile([128, 2 * MC], F32, tag=f"arin{li}", name="arin")
            bout = dram_p.tile([128, 2 * MC], F32, tag=f"arout{li}", name="arout")
            nc.sync.dma_start(bin_[:], allin[:])
            nc.gpsimd.collective_compute(
                "AllReduce", ALU.add, replica_groups=[list(range(NCORES))],
                ins=[bin_.opt()], outs=[bout.opt()])
            gl = stat_p.tile([128, 2 * MC], F32, tag=f"gl{li}", name="gl")
            nc.sync.dma_start(gl[:], bout[:])
            cst = stat_p.tile([128, 3 * MC], F32, tag=f"cst{li}", name="cst")
            nc.sync.dma_start(cst[:], bnconst[li - 1][:])
            mu = stat_p.tile([128, MC], F32, tag=f"mu{li}", name="bmu")
            var = stat_p.tile([128, MC], F32, tag=f"va{li}", name="bvar")
            s_t = stat_p.tile([128, MC], F32, tag=f"s{li}", name="bs")
            t_t = stat_p.tile([128, MC], F32, tag=f"t{li}", name="bt")
            nc.scalar.mul(mu[:], gl[:, 0:MC], 1.0 / NBN)
            nc.scalar.mul(var[:], gl[:, MC:2 * MC], 1.0 / NBN)
            msq = stat_p.tile([128, MC], F32, tag=f"ms{li}", name="bmsq")
            nc.vector.tensor_mul(msq[:], mu[:], mu[:])
            nc.vector.tensor_tensor(var[:], var[:], msq[:], op=ALU.subtract)
            nc.vector.tensor_scalar(var[:], var[:], BNEPS, None, op0=ALU.add)
            sd = stat_p.tile([128, MC], F32, tag=f"sd{li}", name="bsd")
            nc.scalar.activation(sd[:], var[:], AF.Sqrt)
            rsd = stat_p.tile([128, MC], F32, tag=f"rs{li}", name="brsd")
            nc.vector.reciprocal(rsd[:], sd[:])
            nc.vector.tensor_mul(s_t[:], rsd[:], cst[:, 0:MC])
            nc.vector.tensor_mul(t_t[:], mu[:], s_t[:])
            nc.vector.tensor_tensor(t_t[:], cst[:, MC:2 * MC], t_t[:],
                                    op=ALU.subtract)
            bn_s[li], bn_t[li] = s_t, t_t

        with tc.tile_pool(name="wcp", bufs=1) as wcp, \
             tc.tile_pool(name="cxp", bufs=1) as cxp:
            for li in (1, 2, 3, 4):
                conv_layer(li, wcp, cxp, pp_mm)

        # ------------------------------------------------------------------
        # transformer
        # ------------------------------------------------------------------
        rows_p = ctx.enter_context(tc.tile_pool(name="rows", bufs=1))
        scr = ctx.enter_context(tc.tile_pool(name="scratch", bufs=2))

        def ln_cols(xt, xview, dst_tiles, dst_cols):
            """LN per token over feature dim (stats + apply on DVE/Pool)."""
            ps_s = pp_row.tile([1, 512], F32, tag="row_s", name="ps_s")
            ps_q = pp_row.tile([1, 512], F32, tag="row_q", name="ps_q")
            for k in range(4):
                nc.tensor.matmul(ps_s[:], onescol[:], xview(k),
                                 start=(k == 0), stop=(k == 3))
            for k in range(4):
                sq = scr.tile([128, 512], F32R, tag="lnsq", name="lnsq")
                eng = nc.vector if k % 2 else nc.gpsimd
                eng.tensor_mul(sq[:], xview(k), xview(k))
                nc.tensor.matmul(ps_q[:], onescol[:], sq[:],
                                 start=(k == 0), stop=(k == 3))
            mu = rows_p.tile([1, 512], F32R, tag="mu", name="lmu", bufs=2)
            e2 = rows_p.tile([1, 512], F32, tag="e2", name="le2", bufs=2)
            r = rows_p.tile([1, 512], F32R, tag="r", name="lr", bufs=2)
            nc.scalar.mul(mu[:], ps_s[:], 1.0 / D)
            with nc.allow_low_precision(reason="f32r row math"):
                nc.vector.tensor_mul(r[:], mu[:], mu[:])
                nc.vector.scalar_tensor_tensor(
                    e2[:], ps_q[:], 1.0 / D, r[:], op0=ALU.mult,
                    op1=ALU.subtract)
                # 1/sqrt(v) = exp(-0.5 ln(v + eps)): single exp/ln act table
                nc.scalar.activation(e2[:], e2[:], AF.Ln, bias=eps_c[:])
                nc.scalar.activation(r[:], e2[:], AF.Exp, scale=-0.5)
            psb_mu = pp_bc.tile([128, 512], F32, tag="bc", name="psbmu")
            psb_r = pp_bc.tile([128, 512], F32, tag="bc", name="psbr")
            nc.tensor.matmul(psb_mu[:], onesrow[:], mu[:], start=True, stop=True)
            nc.tensor.matmul(psb_r[:], onesrow[:], r[:], start=True, stop=True)
            for k in range(4):
                tmp = scr.tile([128, 512], F32, tag="lntmp", name="lntmp")
                nc.vector.tensor_tensor(tmp[:], xview(k), psb_mu[:],
                                        op=ALU.subtract)
                nc.vector.tensor_mul(dst_tiles[k][:, dst_cols], tmp[:], psb_r[:])

        def ln1_rows(x_in, nm, rcol_all):
            """Per-frame LN stats; negmu row (K=1 fold operand) + 1/sd cols."""
            for f in range(4):
                sl = slice(f * 512, (f + 1) * 512)
                ps_s = pp_row.tile([1, 512], F32, tag="row_s", name="ps_s")
                ps_q = pp_row.tile([1, 512], F32, tag="row_q", name="ps_q")
                for k in range(4):
                    nc.tensor.matmul(ps_s[:], onescol[:], x_in[k][:, sl],
                                     start=(k == 0), stop=(k == 3))
                for k in range(4):
                    sq = scr.tile([128, 512], F32R, tag="lnsq", name="lnsq")
                    eng = nc.vector if k % 2 else nc.gpsimd
                    eng.tensor_mul(sq[:], x_in[k][:, sl], x_in[k][:, sl])
                    nc.tensor.matmul(ps_q[:], onescol[:], sq[:],
                                     start=(k == 0), stop=(k == 3))
                e2 = rows_p.tile([1, 512], F32, tag="e2", name="le2", bufs=2)
                rr = rows_p.tile([1, 512], F32, tag="rr", name="lrr", bufs=4)
                with nc.allow_low_precision(reason="ln1 rows"):
                    nc.scalar.mul(nm[0:1, sl], ps_s[:], -1.0 / D)
                    msq = rows_p.tile([1, 512], F32, tag="rr", name="lms", bufs=4)
                    nc.vector.tensor_mul(msq[:], nm[0:1, sl], nm[0:1, sl])
                    nc.vector.scalar_tensor_tensor(
                        e2[:], ps_q[:], 1.0 / D, msq[:], op0=ALU.mult,
                        op1=ALU.subtract)
                    nc.scalar.activation(e2[:], e2[:], AF.Ln, bias=eps_c[:])
                    nc.scalar.activation(rr[:], e2[:], AF.Exp, scale=-0.5)
                for st in range(4):
                    nc.sync.dma_start(
                        rcol_all[:, f * 4 + st:f * 4 + st + 1],
                        rr[0:1, st * 128:(st + 1) * 128])

        tr_ctx = ctx.enter_context(contextlib.ExitStack())
        wp = tr_ctx.enter_context(tc.tile_pool(name="wp", bufs=1))
        wqp = tr_ctx.enter_context(tc.tile_pool(name="wqp", bufs=1))
        attn_p = tr_ctx.enter_context(tc.tile_pool(name="attn", bufs=2))
        sl_p = tr_ctx.enter_context(tc.tile_pool(name="slices", bufs=1))
        ot_p = tr_ctx.enter_context(tc.tile_pool(name="otp", bufs=1))
        otb_all = ot_p.tile([128, 4 * TOK], BF16, tag="otall", name="otall")

        def st_view(xt, k, st):
            # scattered columns {f*512 + st*128 + p} as (128, (f,p)=512)
            return xt[k][:].rearrange("p (f s) -> p f s", f=4)[:, :, st * 128:(st + 1) * 128]

        def transformer_layer(li, x_in, x_mid):
            wq = [wqp.tile([128, 3 * D], F32R, tag=f"wqkv{k}", name=f"wq{k}")
                  for k in range(4)]
            for k in range(4):
                nc.sync.dma_start(wq[k][:], wqkv_d[li, k * 128:(k + 1) * 128, :])
            ws_t = rows_p.tile([1, 3 * D], BF16, tag="wsum", name="wst", bufs=1)
            nc.sync.dma_start(ws_t[:], wsbq_d[li, 0:1, :])
            bq_t = rows_p.tile([1, 3 * D], BF16, tag="bqr", name="bqt", bufs=1)
            nc.sync.dma_start(bq_t[:], wsbq_d[li, 1:2, :])
            cols = stat_p.tile([128, 12], F32, tag="tcols", name="tcols")
            nc.sync.dma_start(cols[:], tcols_d[li])

            # q bias broadcast over the token partitions (k-bias is
            # softmax-invariant; v-bias is folded into outb on the host)
            bias_bc = attn_p.tile([128, D], BF16, tag="biasbc",
                                  name="bias_bc", bufs=1)
            psb = pp_bc.tile([128, 512], F32, tag="bc", name="psbb")
            nc.tensor.matmul(psb[:], ones_bf[:], bq_t[:, 0:512],
                             start=True, stop=True)
            nc.scalar.copy(bias_bc[:], psb[:])

            nm = attn_p.tile([1, TOK], BF16, tag="nmsd", name="nm", bufs=1)
            rcol = attn_p.tile([128, 16], F32, tag="rcol", name="rcol", bufs=1)
            ln1_rows(x_in, nm, rcol)

            for st in range(4):
                qt = attn_p.tile([128, TOK], BF16, tag="qst", name="qt", bufs=2)
                kt = attn_p.tile([128, TOK], BF16, tag="kst", name="kt", bufs=2)
                # v stored (j, d, h) so the AV multiply hits the 2x DVE mode
                vt = attn_p.tile([128, TOK], BF16, tag="vst", name="vt", bufs=2)
                for f in range(NF):
                    c0 = f * 512 + st * 128
                    for ns in range(3):
                        ps = pp_mm.tile([128, 512], F32, tag="mm", name="qps")
                        for k in range(4):
                            nc.tensor.matmul(
                                ps[:], x_in[k][:, c0:c0 + 128],
                                wq[k][:, ns * 512:(ns + 1) * 512],
                                start=(k == 0), stop=False)
                        nc.tensor.matmul(ps[:], nm[0:1, c0:c0 + 128],
                                         ws_t[:, ns * 512:(ns + 1) * 512],
                                         start=False, stop=True)
                        rc = rcol[:, f * 4 + st:f * 4 + st + 1]
                        if ns < 2:
                            nc.scalar.activation(
                                (qt if ns == 0 else kt)[:, f * 512:(f + 1) * 512],
                                ps[:], AF.Copy, scale=rc)
                        else:
                            nc.scalar.activation(
                                vt[:, f * 512:(f + 1) * 512]
                                .rearrange("p (d h) -> p h d", h=8),
                                ps[:].rearrange("p (h d) -> p h d", h=8),
                                AF.Copy, scale=rc)
                # q bias, broadcast over frames, one 2x-mode op per st
                nc.vector.tensor_tensor(
                    qt[:].rearrange("p (f c) -> p f c", f=4),
                    qt[:].rearrange("p (f c) -> p f c", f=4),
                    bias_bc[:].unsqueeze(1).broadcast_to([128, 4, 512]),
                    op=ALU.add)

                s_sc = attn_p.tile([128, 128], F32, tag="s_sc", name="s_sc",
                                   bufs=2)
                k4 = kt[:].rearrange("p (j hd) -> p j hd", j=4)
                for i in range(4):
                    pbig = attn_p.tile([128, TOK], BF16, tag="pbig",
                                       name="pbig", bufs=2)
                    qi = qt[:, i * 512:(i + 1) * 512].unsqueeze(1) \
                        .broadcast_to([128, 4, 512])
                    nc.vector.tensor_mul(
                        pbig[:].rearrange("p (j hd) -> p j hd", j=4), qi, k4)
                    st1 = attn_p.tile([128, TOK // 2], BF16, tag="qks1",
                                      name="qks1", bufs=2)
                    st2 = attn_p.tile([128, TOK // 4], BF16, tag="qks2",
                                      name="qks2", bufs=2)
                    pv = pbig[:].rearrange("p (g d) -> p g d", g=32)
                    nc.vector.tensor_tensor(
                        st1[:].rearrange("p (g d) -> p g d", g=32),
                        pv[:, :, 0:32], pv[:, :, 32:64], op=ALU.add)
                    sv = st1[:].rearrange("p (g d) -> p g d", g=32)
                    nc.vector.tensor_tensor(
                        st2[:].rearrange("p (g d) -> p g d", g=32),
                        sv[:, :, 0:16], sv[:, :, 16:32], op=ALU.add)
                    nc.vector.tensor_reduce(
                        s_sc[:, i * 32:(i + 1) * 32],
                        st2[:].rearrange("p (g d) -> p g d", g=32),
                        axis=AX.X, op=ALU.add)
                # softmax over j without max-subtraction (logits bounded)
                # S cols = (i, j, h)
                eexp = attn_p.tile([128, 128], BF16, tag="eexp", name="eexp",
                                   bufs=2)
                nc.scalar.activation(eexp[:], s_sc[:], AF.Exp)
                z = attn_p.tile([128, 32], F32, tag="z", name="zt", bufs=2)
                nc.vector.tensor_reduce(
                    z[:].rearrange("p (i h) -> p i h", i=4),
                    eexp[:].rearrange("p (i j h) -> p i h j", i=4, j=4),
                    axis=AX.X, op=ALU.add)
                zr = attn_p.tile([128, 32], F32, tag="zr", name="zr", bufs=2)
                nc.vector.reciprocal(zr[:], z[:])
                a_t = attn_p.tile([128, 128], BF16, tag="a_t", name="a_t",
                                  bufs=2)
                nc.vector.tensor_mul(
                    a_t[:].rearrange("p (i j h) -> p i j h", i=4, j=4),
                    eexp[:].rearrange("p (i j h) -> p i j h", i=4, j=4),
                    zr[:].rearrange("p (i h) -> p i h", i=4).unsqueeze(2)
                    .broadcast_to([128, 4, 4, 8]))
                for i in range(4):
                    tbig = attn_p.tile([128, TOK], BF16, tag="tbig", name="tbig", bufs=2)
                    ablk = a_t[:, i * 32:(i + 1) * 32] \
                        .rearrange("p (j h) -> p j h", j=4) \
                        .unsqueeze(2).broadcast_to([128, 4, 64, 8])
                    nc.vector.tensor_mul(
                        tbig[:].rearrange("p (j d h) -> p j d h", j=4, d=64),
                        vt[:].rearrange("p (j d h) -> p j d h", j=4, d=64),
                        ablk)
                    av01 = attn_p.tile([128, 512], BF16, tag="av01", name="av01",
                                       bufs=2)
                    av = attn_p.tile([128, 512], BF16, tag="av", name="av",
                                     bufs=2)
                    nc.vector.tensor_tensor(av01[:], tbig[:, 0:512],
                                            tbig[:, 512:1024], op=ALU.add)
                    nc.vector.tensor_tensor(av[:], tbig[:, 1024:1536],
                                            tbig[:, 1536:2048], op=ALU.add)
                    nc.gpsimd.tensor_tensor(av[:], av01[:], av[:],
                                            op=ALU.add)
                    pst = pp_bc.tile([128, 512], BF16, tag="bc", name="pst")
                    for c in range(4):
                        nc.tensor.transpose(pst[:, c * 128:(c + 1) * 128],
                                            av[:, c * 128:(c + 1) * 128],
                                            ident[:])
                    nc.scalar.copy(
                        otb_all[:].rearrange("p (c t) -> p c t", c=4)
                        [:, :, i * 512 + st * 128:i * 512 + st * 128 + 128],
                        pst[:].rearrange("p (c t) -> p c t", c=4))

            wo = [wp.tile([128, D], BF16, tag=f"wo{k}", name=f"wo{k}")
                  for k in range(4)]
            for k in range(4):
                nc.sync.dma_start(wo[k][:], wo_d[li, k * 128:(k + 1) * 128, :])
            for m in range(4):
                for ns in range(4):
                    ps = pp_mm.tile([128, 512], F32, tag="mm", name="ops")
                    for k in range(4):
                        nc.tensor.matmul(
                            ps[:], wo[k][:, m * 128:(m + 1) * 128],
                            otb_all[:, k * TOK + ns * 512:k * TOK + (ns + 1) * 512],
                            start=(k == 0), stop=(k == 3))
                    nc.vector.scalar_tensor_tensor(
                        x_mid[m][:, ns * 512:(ns + 1) * 512], ps[:],
                        cols[:, 4 + m:5 + m], x_in[m][:, ns * 512:(ns + 1) * 512],
                        op0=ALU.add, op1=ALU.add)

            w1 = [wp.tile([128, D], F32R, tag=f"w1_{k}", name=f"w1_{k}")
                  for k in range(4)]
            w2 = [wp.tile([128, D], F32R, tag=f"w2_{k}", name=f"w2_{k}")
                  for k in range(4)]
            for k in range(4):
                nc.sync.dma_start(w1[k][:], w1_d[li, k * 128:(k + 1) * 128, :])
                nc.sync.dma_start(w2[k][:], w2_d[li, k * 128:(k + 1) * 128, :])
            for ns in range(4):
                xh2 = [sl_p.tile([128, 512], F32R, tag=f"xh2_{k}", name=f"xh2_{k}")
                       for k in range(4)]
                ln_cols(x_mid,
                        lambda k: x_mid[k][:, ns * 512:(ns + 1) * 512],
                        xh2, slice(0, 512))
                hsl = [sl_p.tile([128, 512], F32R, tag=f"h_{m}", name=f"hsl{m}")
                       for m in range(4)]
                for m in range(4):
                    ps = pp_mm.tile([128, 512], F32, tag="mm", name="m1ps")
                    for k in range(4):
                        nc.tensor.matmul(
                            ps[:], w1[k][:, m * 128:(m + 1) * 128], xh2[k][:],
                            start=(k == 0), stop=(k == 3))
                    nc.scalar.activation(hsl[m][:], ps[:], AF.Gelu_apprx_tanh,
                                         bias=cols[:, m:m + 1])
                for m in range(4):
                    ps = pp_mm.tile([128, 512], F32, tag="mm", name="m2ps")
                    for k in range(4):
                        nc.tensor.matmul(
                            ps[:], w2[k][:, m * 128:(m + 1) * 128], hsl[k][:],
                            start=(k == 0), stop=(k == 3))
                    nc.vector.scalar_tensor_tensor(
                        x_mid[m][:, ns * 512:(ns + 1) * 512], ps[:],
                        cols[:, 8 + m:9 + m], x_mid[m][:, ns * 512:(ns + 1) * 512],
                        op0=ALU.add, op1=ALU.add)

        cur = xA
        _nl = int(os.environ.get("KNLAYERS", NLAYER))
        for li in range(_nl):
            transformer_layer(li, cur, cur)

        tr_ctx.close()

        # ------------------------------------------------------------------
        # heads
        # ------------------------------------------------------------------
        _skip_heads = os.environ.get("KHEADS", "1") == "0"
        if _skip_heads:
            nc.gpsimd.dma_start(d6_o[:], cur[0][0:8, :])
            nc.gpsimd.dma_start(tr_o[:], cur[1][0:8, :])
        with tc.tile_pool(name="heads", bufs=1) as hp, \
             tc.tile_pool(name="whp", bufs=1) as whp:
          if not _skip_heads:
              hc = stat_p.tile([128, 10], F32, tag="hcols", name="hc")
              nc.sync.dma_start(hc[:], hcols_d[:])
              sb3 = stat_p.tile([8, 2], F32, tag="sb3", name="sb3")
              nc.sync.dma_start(sb3[:], sb3_d[:])

              xhf = [hp.tile([128, TOK], F32R, tag=f"xhf{k}", name=f"xhf{k}")
                     for k in range(4)]
              for st in range(4):
                  ln_cols(cur, lambda k: cur[k][:, st * 512:(st + 1) * 512],
                          xhf, slice(st * 512, (st + 1) * 512))

              xp = [hp.tile([128, TOK], F32R, tag=f"xp{k}", name=f"xp{k}")
                    for k in range(4)]

              def mm_head(src_tiles, wt_dram, kdim, mdim, dst_tiles, evict):
                  KC = kdim // 128
                  MC = max(mdim // 128, 1)
                  wsb = [whp.tile([128, mdim], F32R, tag=f"wh_{kdim}_{mdim}_{k}",
                                  name=f"wh{k}") for k in range(KC)]
                  for k in range(KC):
                      nc.sync.dma_start(wsb[k][:], wt_dram[k * 128:(k + 1) * 128, :])
                  for m in range(MC):
                      for ns in range(4):
                          ps = pp_mm.tile([128, 512], F32, tag="mm", name="hps")
                          for k in range(KC):
                              nc.tensor.matmul(
                                  ps[:], wsb[k][:, m * 128:(m + 1) * 128],
                                  src_tiles[k][:, ns * 512:(ns + 1) * 512],
                                  start=(k == 0), stop=(k == KC - 1))
                          evict(ps, dst_tiles[m], m, ns)

              mm_head(xhf, projw_d, D, D, xp,
                      lambda ps, dst, m, ns: nc.scalar.activation(
                          dst[:, ns * 512:(ns + 1) * 512], ps[:], AF.Identity,
                          bias=hc[:, m:m + 1]))

              def branch(w1d, w2d, w3d, b1ofs, b2ofs, out_dram, b3col, r1, r2, pfx):
                  mm_head(xp, w1d, D, 256, r1,
                          lambda ps, dst, m, ns: nc.scalar.activation(
                              dst[:, ns * 512:(ns + 1) * 512], ps[:], AF.Relu,
                              bias=hc[:, b1ofs + m:b1ofs + m + 1]))
                  mm_head(r1, w2d, 256, 128, r2,
                          lambda ps, dst, m, ns: nc.scalar.activation(
                              dst[:, ns * 512:(ns + 1) * 512], ps[:], AF.Relu,
                              bias=hc[:, b2ofs:b2ofs + 1]))
                  w3 = whp.tile([128, 8], F32R, tag=f"w3{pfx}", name="w3")
                  nc.sync.dma_start(w3[:], w3d[:])
                  out_sb = hp.tile([8, TOK], F32, tag=f"{pfx}out", name=f"{pfx}out")
                  for ns in range(4):
                      ps = pp_mm.tile([8, 512], F32, tag="mm", name="bps")
                      nc.tensor.matmul(ps[:], w3[:], r2[0][:, ns * 512:(ns + 1) * 512],
                                       start=True, stop=True)
                      nc.scalar.activation(out_sb[:, ns * 512:(ns + 1) * 512], ps[:],
                                           AF.Identity, bias=b3col)
                      nc.sync.dma_start(out_dram[:, ns * 512:(ns + 1) * 512],
                                        out_sb[:, ns * 512:(ns + 1) * 512])

              # reuse dead transformer buffers for intermediates
              branch(rw1_d, rw2_d, rw3_d, 4, 8, d6_o, sb3[:, 0:1],
                     [cur[0], cur[1]], [xhf[0]], "r")
              branch(tw1_d, tw2_d, tw3_d, 6, 9, tr_o, sb3[:, 1:2],
                     [cur[2], cur[3]], [xhf[1]], "t")

    nc.compile()
    return nc


# ----------------------------------------------------------------------------
# host side
# ----------------------------------------------------------------------------

_CACHE = {}


def _normalize_np(v, eps=1e-12):
    return v / np.maximum(np.linalg.norm(v, axis=-1, keepdims=True), eps)


def _rot6d_np(d6):
    a1, a2 = d6[..., :3], d6[..., 3:]
    b1 = _normalize_np(a1)
    b2 = _normalize_np(a2 - np.sum(b1 * a2, -1, keepdims=True) * b1)
    b3 = np.cross(b1, b2)
    return np.stack([b1, b2, b3], axis=-2)


def _prep_weights(inp):
    f32 = np.float32
    wmap = {}
    for i, cw in enumerate(['c1w', 'c2w', 'c3w', 'c4w']):
        wmap[f'convw{i}'] = np.ascontiguousarray(inp[cw].T.astype(f32))
    for i, (g, b2) in enumerate([('bn1g', 'bn1b'), ('bn2g', 'bn2b'),
                                 ('bn3g', 'bn3b')]):
        M = CONV_DIMS[i + 1] // 128
        bn = np.concatenate([
            inp[g].reshape(M, 128).T, inp[b2].reshape(M, 128).T,
            inp[f'c{i + 1}b'].reshape(M, 128).T], axis=1)
        wmap[f'bnconst{i}'] = np.ascontiguousarray(bn.astype(f32))
    pe = _pe_table()[:NF]
    b4 = inp['c4b'][None, :].astype(f32) + pe                   # (4, 512)
    # cols: m*4 + pt ; frame index == pt
    wmap['bias4'] = np.ascontiguousarray(
        b4.reshape(NF, 4, 128).transpose(2, 1, 0).reshape(128, 16).astype(f32))

    qkvw = np.array(inp['qkvw'], f32)
    qkvb = np.array(inp['qkvb'], f32)
    qkvw[:, :, :512] /= math.sqrt(DH)
    qkvb[:, :512] /= math.sqrt(DH)
    g1 = np.array(inp['ln1g'], f32)
    b1 = np.array(inp['ln1b'], f32)
    wq_fold = g1[:, :, None] * qkvw
    bq_fold = qkvb + np.einsum('ld,ldf->lf', b1, qkvw)
    wmap['wqkv'] = np.ascontiguousarray(wq_fold.astype(f32))
    wsbq = np.concatenate([wq_fold.sum(axis=1, keepdims=True),
                           bq_fold[:, None, :]], axis=1)         # (L, 2, 3D)
    wmap['wsbq'] = np.ascontiguousarray(wsbq.astype(ml_dtypes.bfloat16))
    # attention output features are (d, h)-ordered; permute wo rows to match
    wo_ = np.array(inp['outw'], f32)                             # (L, 512, 512)
    d_idx, h_idx = np.meshgrid(np.arange(DH), np.arange(HEADS), indexing='ij')
    perm = (h_idx * DH + d_idx).reshape(512)     # perm[d*8+h] = h*64+d
    wmap['wo'] = np.ascontiguousarray(wo_[:, perm, :]
                                      .astype(ml_dtypes.bfloat16))
    g2 = np.array(inp['ln2g'], f32)
    bl2 = np.array(inp['ln2b'], f32)
    m1w = np.array(inp['m1w'], f32)
    w1_fold = g2[:, :, None] * m1w
    b1_fold = np.array(inp['m1b'], f32) + np.einsum('ld,ldf->lf', bl2, m1w)
    wmap['w1'] = np.ascontiguousarray(w1_fold.astype(f32))
    wmap['w2'] = np.ascontiguousarray(np.array(inp['m2w'], f32))
    cols = np.zeros((NLAYER, 128, 12), f32)
    cols[:, :, 0:4] = b1_fold.reshape(NLAYER, 4, 128).transpose(0, 2, 1)
    # v-bias is dropped at the v eviction; fold bv @ Wo into outb instead
    bv = bq_fold[:, 2 * 512:3 * 512]                             # (L, 512)
    outb_fold = np.array(inp['outb'], f32) + np.einsum('lk,lko->lo', bv, wo_)
    cols[:, :, 4:8] = outb_fold.reshape(NLAYER, 4, 128).transpose(0, 2, 1)
    cols[:, :, 8:12] = np.array(inp['m2b'], f32).reshape(NLAYER, 4, 128) \
        .transpose(0, 2, 1)
    wmap['tcols'] = cols

    gf_ = np.array(inp['lnfg'], f32)
    bf_ = np.array(inp['lnfb'], f32)
    projw = np.array(inp['projw'], f32)
    wmap['projw'] = np.ascontiguousarray(gf_[:, None] * projw)
    projb_fold = np.array(inp['projb'], f32) + bf_ @ projw
    wmap['rw1'] = np.ascontiguousarray(np.array(inp['rw1'], f32))
    wmap['rw2'] = np.ascontiguousarray(np.array(inp['rw2'], f32))
    rw3 = np.zeros((128, 8), f32)
    rw3[:, :6] = np.array(inp['rw3'], f32)
    wmap['rw3'] = rw3
    wmap['tw1'] = np.ascontiguousarray(np.array(inp['tw1'], f32))
    wmap['tw2'] = np.ascontiguousarray(np.array(inp['tw2'], f32))
    tw3 = np.zeros((128, 8), f32)
    tw3[:, :3] = np.array(inp['tw3'], f32)
    wmap['tw3'] = tw3
    hcols = np.zeros((128, 10), f32)
    hcols[:, 0:4] = projb_fold.reshape(4, 128).T
    hcols[:, 4:6] = np.array(inp['rb1'], f32).reshape(2, 128).T
    hcols[:, 6:8] = np.array(inp['tb1'], f32).reshape(2, 128).T
    hcols[:, 8] = np.array(inp['rb2'], f32)
    hcols[:, 9] = np.array(inp['tb2'], f32)
    wmap['hcols'] = hcols
    sb3 = np.zeros((8, 2), f32)
    sb3[0:6, 0] = np.array(inp['rb3'], f32)
    sb3[0:3, 1] = np.array(inp['tb3'], f32)
    wmap['sb3'] = sb3
    wmap['ones_c'] = np.ones((128, 128), f32)
    return wmap


def kernel(**inputs):
    inp = {k: np.asarray(v) for k, v in inputs.items()}

    idx = inp['seed_idxs'].reshape(B, -1).astype(np.int64)      # (B, N)
    sel_seed = np.take_along_axis(np.asarray(inp['fp2_features'], np.float32),
                                  idx[:, None, :], axis=2)
    sel_grasp = np.take_along_axis(np.asarray(inp['local_grasp_features'], np.float32),
                                   idx[:, None, :], axis=2)
    sel_color = np.take_along_axis(np.asarray(inp['local_color_features'], np.float32),
                                   idx[:, None, :], axis=2)
    sel_pose = np.take_along_axis(np.asarray(inp['grasp_pose_feature'], np.float32),
                                  idx[:, None, :], axis=2)
    gsf = np.asarray(inp['sa4_features'], np.float32).max(axis=-1)
    gsf = np.broadcast_to(gsf[:, :, None], (B, 256, NPTS))
    fused = sel_pose + np.concatenate([sel_grasp, sel_color, sel_seed, gsf], axis=1)
    gf = fused.reshape(BE, FRAME, 1024, NPTS)
    cond = np.broadcast_to(gf[:, :1], (BE, NF, 1024, NPTS))
    X = np.concatenate([cond, gf[:, 1:]], axis=2)               # (e, f, 2048, N)

    if 'nc' not in _CACHE:
        _CACHE['nc'] = build_kernel()
    nc = _CACHE['nc']
    wmap = _prep_weights(inp)

    in_maps = []
    for k in range(NCORES):
        xc = X[:, :, :, k * NPC:(k + 1) * NPC]                  # (e, f, c, n)
        xc = xc.transpose(2, 1, 0, 3).reshape(2048, TOK)        # (c, (f,e,n))
        m = dict(wmap)
        m['xin'] = np.ascontiguousarray(xc, dtype=np.float32)
        in_maps.append(m)

    res = run_bass_kernel_spmd(nc, in_maps, core_ids=list(range(NCORES)))

    out = np.zeros((BE * NPTS, NF, 12), np.float32)
    for k in range(NCORES):
        d6 = res.results[k]['d6'][:6]
        tr = res.results[k]['tr3'][:3]
        d6 = d6.reshape(6, NF, BE, NPC).transpose(2, 3, 1, 0)   # (e, n, f, 6)
        tr = tr.reshape(3, NF, BE, NPC).transpose(2, 3, 1, 0)
        rot = _rot6d_np(d6).reshape(BE, NPC, NF, 9)
        for e in range(BE):
            rows = slice(e * NPTS + k * NPC, e * NPTS + (k + 1) * NPC)
            out[rows, :, 0:3] = tr[e]
            out[rows, :, 3:12] = rot[e]
    return out


if __name__ == "__main__":
    build_kernel()
    print("built ok")



# revision 70
# speedup vs baseline: 1.1000x; 1.0022x over previous
import math
import os
import numpy as np
import ml_dtypes
import contextlib

import concourse.bass as bass
import concourse.tile as tile
from concourse import bacc, mybir, masks
from concourse.bass_utils import run_bass_kernel_spmd

F32 = mybir.dt.float32
F32R = mybir.dt.float32r
BF16 = mybir.dt.bfloat16
ALU = mybir.AluOpType
AF = mybir.ActivationFunctionType
AX = mybir.AxisListType

NCORES = 8
FRAME = 5
NF = FRAME - 1
D = 512
DH = 64
HEADS = 8
B = 20
NPTS = 1024
BE = B // FRAME
NPC = NPTS // NCORES     # 128 points per core
TOK = NF * BE * NPC      # 2048 tokens per core
NLAYER = 12
LNEPS = 1e-5
BNEPS = 1e-5
NBN = 16 * NPTS

CONV_DIMS = [2048, 1536, 1024, 768, 512]


def _pe_table(max_len=16, d=D):
    pos = np.arange(max_len, dtype=np.float32)[:, None]
    div = np.exp(np.arange(0, d, 2, dtype=np.float32) * (-math.log(10000.0) / d))
    pe = np.zeros((max_len, d), np.float32)
    pe[:, 0::2] = np.sin(pos * div)
    pe[:, 1::2] = np.cos(pos * div)
    return pe


def build_kernel():
    nc = bacc.Bacc("TRN2", target_bir_lowering=False, debug=False,
                   num_devices=NCORES)

    xin = nc.dram_tensor("xin", [CONV_DIMS[0], TOK], F32R, kind="ExternalInput").ap()
    convw = [nc.dram_tensor(f"convw{i}", [CONV_DIMS[i], CONV_DIMS[i + 1]], F32R,
                            kind="ExternalInput").ap() for i in range(4)]
    bnconst = [nc.dram_tensor(f"bnconst{i}", [128, 3 * (CONV_DIMS[i + 1] // 128)],
                              F32, kind="ExternalInput").ap() for i in range(3)]
    bias4 = nc.dram_tensor("bias4", [128, 4 * NF], F32, kind="ExternalInput").ap()

    wqkv_d = nc.dram_tensor("wqkv", [NLAYER, D, 3 * D], F32R, kind="ExternalInput").ap()
    wsbq_d = nc.dram_tensor("wsbq", [NLAYER, 2, 3 * D], BF16, kind="ExternalInput").ap()
    wo_d = nc.dram_tensor("wo", [NLAYER, D, D], BF16, kind="ExternalInput").ap()
    w1_d = nc.dram_tensor("w1", [NLAYER, D, D], F32R, kind="ExternalInput").ap()
    w2_d = nc.dram_tensor("w2", [NLAYER, D, D], F32R, kind="ExternalInput").ap()
    tcols_d = nc.dram_tensor("tcols", [NLAYER, 128, 12], F32, kind="ExternalInput").ap()

    projw_d = nc.dram_tensor("projw", [D, D], F32R, kind="ExternalInput").ap()
    rw1_d = nc.dram_tensor("rw1", [D, 256], F32R, kind="ExternalInput").ap()
    rw2_d = nc.dram_tensor("rw2", [256, 128], F32R, kind="ExternalInput").ap()
    rw3_d = nc.dram_tensor("rw3", [128, 8], F32R, kind="ExternalInput").ap()
    tw1_d = nc.dram_tensor("tw1", [D, 256], F32R, kind="ExternalInput").ap()
    tw2_d = nc.dram_tensor("tw2", [256, 128], F32R, kind="ExternalInput").ap()
    tw3_d = nc.dram_tensor("tw3", [128, 8], F32R, kind="ExternalInput").ap()
    hcols_d = nc.dram_tensor("hcols", [128, 10], F32, kind="ExternalInput").ap()
    sb3_d = nc.dram_tensor("sb3", [8, 2], F32, kind="ExternalInput").ap()
    ones_d = nc.dram_tensor("ones_c", [128, 128], F32, kind="ExternalInput").ap()

    d6_o = nc.dram_tensor("d6", [8, TOK], F32, kind="ExternalOutput").ap()
    tr_o = nc.dram_tensor("tr3", [8, TOK], F32, kind="ExternalOutput").ap()

    with tile.TileContext(nc) as tc, contextlib.ExitStack() as ctx:
        const_p = ctx.enter_context(tc.tile_pool(name="consts", bufs=1))
        onescol = const_p.tile([128, 1], F32R)
        onesrow = const_p.tile([1, 128], F32R)
        ident = const_p.tile([128, 128], BF16)
        nc.gpsimd.dma_start(onescol[:], ones_d[:, 0:1])
        nc.gpsimd.dma_start(onesrow[:], ones_d[0:1, :])
        ones_bf = const_p.tile([1, 128], BF16)
        nc.vector.memset(ones_bf[:], 1.0)
        eps_c = const_p.tile([1, 1], F32)
        nc.vector.memset(eps_c[:], LNEPS)
        masks.make_identity(nc, ident[:])

        xs_p = ctx.enter_context(tc.tile_pool(name="xstate", bufs=1))
        xA = [xs_p.tile([128, TOK], F32R, tag=f"xA{m}", name=f"xA{m}") for m in range(4)]

        stat_p = ctx.enter_context(tc.tile_pool(name="stats", bufs=1))
        dram_p = ctx.enter_context(tc.tile_pool(name="dramb", bufs=1, space="DRAM"))

        pp_mm = ctx.enter_context(tc.tile_pool(name="ppmm", bufs=4, space="PSUM"))
        pp_row = ctx.enter_context(tc.tile_pool(name="pprow", bufs=1, space="PSUM"))
        pp_bc = ctx.enter_context(tc.tile_pool(name="ppbc", bufs=2, space="PSUM"))

        y_dram = [dram_p.tile([CONV_DIMS[i], TOK], F32R, tag=f"ydram{i}", name=f"ydram{i}")
                  for i in range(1, 4)]

        # ------------------------------------------------------------------
        # conv stack (activations spilled to DRAM, BN applied on load)
        # ------------------------------------------------------------------
        b4sb = stat_p.tile([128, 4 * NF], F32, tag="b4")
        nc.sync.dma_start(b4sb[:], bias4[:])

        bn_s = {}
        bn_t = {}

        def conv_layer(li, wcp, cxp, pp_conv):
            kdim, mdim = CONV_DIMS[li - 1], CONV_DIMS[li]
            KC, MC = kdim // 128, mdim // 128
            src = xin if li == 1 else y_dram[li - 2]
            with_bn = li < 4
            if with_bn:
                sum_acc = stat_p.tile([128, MC * 4], F32, tag=f"sum{li}")
                sq_acc = stat_p.tile([128, MC * 4], F32, tag=f"sq{li}")
            cond_sb = None
            if li == 1:
                # channels 0:1024 repeat frame 0's features for all 4 frames;
                # compute their contribution once and add it at eviction
                KC = 8
                xc = cxp.tile([128, 8 * 512], F32R, tag="convc", name="convc",
                              bufs=1)
                nc.sync.dma_start(
                    xc[:].rearrange("p (k c) -> p k c", k=8),
                    src[0:1024, 0:512].rearrange("(k p) c -> p k c", p=128))
                cond_sb = [cxp.tile([128, 512], F32, tag=f"cond{m}",
                                    name=f"cond{m}") for m in range(MC)]
                for m in range(MC):
                    wslc = wcp.tile([128, 8 * 128], F32R, tag="wslc",
                                    name="wslc", bufs=3)
                    wvc = wslc[:].rearrange("p (k c) -> p k c", k=8)
                    nc.sync.dma_start(
                        wvc, convw[0][0:1024, m * 128:(m + 1) * 128]
                        .rearrange("(k p) c -> p k c", p=128))
                    ps = pp_conv.tile([128, 512], F32, tag="mm", name="ccps")
                    for k in range(8):
                        nc.tensor.matmul(
                            ps[:], wvc[:, k, :], xc[:, k * 512:(k + 1) * 512],
                            start=(k == 0), stop=(k == 7))
                    nc.scalar.copy(cond_sb[m][:], ps[:])
            for pt in range(4):
                xt = cxp.tile([128, KC * 512], F32R, tag="convx", name="convx",
                              bufs=3)
                nc.sync.dma_start(
                    xt[:].rearrange("p (k c) -> p k c", k=KC),
                    src[kdim - KC * 128:, pt * 512:(pt + 1) * 512]
                    .rearrange("(k p) c -> p k c", p=128))
                if li > 1:
                    s_p, t_p = bn_s[li - 1], bn_t[li - 1]
                    for k in range(KC):
                        nc.scalar.activation(
                            xt[:, k * 512:(k + 1) * 512],
                            xt[:, k * 512:(k + 1) * 512],
                            AF.Relu, bias=t_p[:, k:k + 1], scale=s_p[:, k:k + 1])
                for m in range(MC):
                    wsl = wcp.tile([128, KC * 128], F32R, tag="wsl", name="wsl",
                                   bufs=6)
                    wv = wsl[:].rearrange("p (k c) -> p k c", k=KC)
                    nc.sync.dma_start(
                        wv, convw[li - 1][kdim - KC * 128:,
                                          m * 128:(m + 1) * 128]
                        .rearrange("(k p) c -> p k c", p=128))
                    ps = pp_conv.tile([128, 512], F32, tag="mm", name="cps")
                    for k in range(KC):
                        nc.tensor.matmul(
                            ps[:], wv[:, k, :], xt[:, k * 512:(k + 1) * 512],
                            start=(k == 0), stop=(k == KC - 1))
                    if with_bn:
                        ot = cxp.tile([128, 512], F32R, tag="convot", name="cot",
                                      bufs=4)
                        if cond_sb is not None:
                            nc.vector.tensor_tensor(ot[:], ps[:],
                                                    cond_sb[m][:], op=ALU.add)
                            stats_src = ot[:]
                        else:
                            stats_src = ps[:]
                            nc.scalar.activation(
                                ot[:], ps[:], AF.Copy,
                                accum_out=sum_acc[:, m * 4 + pt:m * 4 + pt + 1])
                        sqs = cxp.tile([128, 512], BF16, tag="sqscr", name="sqs",
                                       bufs=4)
                        if cond_sb is not None:
                            nc.scalar.activation(
                                sqs[:], stats_src, AF.Copy,
                                accum_out=sum_acc[:, m * 4 + pt:m * 4 + pt + 1])
                        nc.scalar.activation(
                            sqs[:], stats_src, AF.Square,
                            accum_out=sq_acc[:, m * 4 + pt:m * 4 + pt + 1])
                        nc.sync.dma_start(
                            y_dram[li - 1][m * 128:(m + 1) * 128,
                                           pt * 512:(pt + 1) * 512], ot[:])
                    else:
                        nc.scalar.activation(
                            xA[m][:, pt * 512:(pt + 1) * 512], ps[:], AF.Identity,
                            bias=b4sb[:, m * 4 + pt:m * 4 + pt + 1])
            if not with_bn:
                return
            allin = stat_p.tile([128, 2 * MC], F32, tag=f"ain{li}", name="allin")
            nc.vector.tensor_reduce(
                allin[:, 0:MC], sum_acc[:].rearrange("p (m t) -> p m t", m=MC),
                axis=AX.X, op=ALU.add)
            nc.vector.tensor_reduce(
                allin[:, MC:2 * MC], sq_acc[:].rearrange("p (m t) -> p m t", m=MC),
                axis=AX.X, op=ALU.add)
            bin_ = dram_p.tile([128, 2 * MC], F32, tag=f"arin{li}", name="arin")
            bout = dram_p.tile([128, 2 * MC], F32, tag=f"arout{li}", name="arout")
            nc.sync.dma_start(bin_[:], allin[:])
            nc.gpsimd.collective_compute(
                "AllReduce", ALU.add, replica_groups=[list(range(NCORES))],
                ins=[bin_.opt()], outs=[bout.opt()])
            gl = stat_p.tile([128, 2 * MC], F32, tag=f"gl{li}", name="gl")
            nc.sync.dma_start(gl[:], bout[:])
            cst = stat_p.tile([128, 3 * MC], F32, tag=f"cst{li}", name="cst")
            nc.sync.dma_start(cst[:], bnconst[li - 1][:])
            mu = stat_p.tile([128, MC], F32, tag=f"mu{li}", name="bmu")
            var = stat_p.tile([128, MC], F32, tag=f"va{li}", name="bvar")
            s_t = stat_p.tile([128, MC], F32, tag=f"s{li}", name="bs")
            t_t = stat_p.tile([128, MC], F32, tag=f"t{li}", name="bt")
            nc.scalar.mul(mu[:], gl[:, 0:MC], 1.0 / NBN)
            nc.scalar.mul(var[:], gl[:, MC:2 * MC], 1.0 / NBN)
            msq = stat_p.tile([128, MC], F32, tag=f"ms{li}", name="bmsq")
            nc.vector.tensor_mul(msq[:], mu[:], mu[:])
            nc.vector.tensor_tensor(var[:], var[:], msq[:], op=ALU.subtract)
            nc.vector.tensor_scalar(var[:], var[:], BNEPS, None, op0=ALU.add)
            sd = stat_p.tile([128, MC], F32, tag=f"sd{li}", name="bsd")
            nc.scalar.activation(sd[:], var[:], AF.Sqrt)
            rsd = stat_p.tile([128, MC], F32, tag=f"rs{li}", name="brsd")
            nc.vector.reciprocal(rsd[:], sd[:])
            nc.vector.tensor_mul(s_t[:], rsd[:], cst[:, 0:MC])
            nc.vector.tensor_mul(t_t[:], mu[:], s_t[:])
            nc.vector.tensor_tensor(t_t[:], cst[:, MC:2 * MC], t_t[:],
                                    op=ALU.subtract)
            bn_s[li], bn_t[li] = s_t, t_t

        with tc.tile_pool(name="wcp", bufs=1) as wcp, \
             tc.tile_pool(name="cxp", bufs=1) as cxp:
            for li in (1, 2, 3, 4):
                conv_layer(li, wcp, cxp, pp_mm)

        # ------------------------------------------------------------------
        # transformer
        # ------------------------------------------------------------------
        rows_p = ctx.enter_context(tc.tile_pool(name="rows", bufs=1))
        scr = ctx.enter_context(tc.tile_pool(name="scratch", bufs=2))

        def ln_cols(xt, xview, dst_tiles, dst_cols):
            """LN per token over feature dim (stats + apply on DVE/Pool)."""
            ps_s = pp_row.tile([1, 512], F32, tag="row_s", name="ps_s")
            ps_q = pp_row.tile([1, 512], F32, tag="row_q", name="ps_q")
            for k in range(4):
                nc.tensor.matmul(ps_s[:], onescol[:], xview(k),
                                 start=(k == 0), stop=(k == 3))
            for k in range(4):
                sq = scr.tile([128, 512], F32R, tag="lnsq", name="lnsq")
                eng = nc.vector if k % 2 else nc.gpsimd
                eng.tensor_mul(sq[:], xview(k), xview(k))
                nc.tensor.matmul(ps_q[:], onescol[:], sq[:],
                                 start=(k == 0), stop=(k == 3))
            mu = rows_p.tile([1, 512], F32R, tag="mu", name="lmu", bufs=2)
            e2 = rows_p.tile([1, 512], F32, tag="e2", name="le2", bufs=2)
            r = rows_p.tile([1, 512], F32R, tag="r", name="lr", bufs=2)
            nc.scalar.mul(mu[:], ps_s[:], 1.0 / D)
            with nc.allow_low_precision(reason="f32r row math"):
                nc.vector.tensor_mul(r[:], mu[:], mu[:])
                nc.vector.scalar_tensor_tensor(
                    e2[:], ps_q[:], 1.0 / D, r[:], op0=ALU.mult,
                    op1=ALU.subtract)
                # 1/sqrt(v) = exp(-0.5 ln(v + eps)): single exp/ln act table
                nc.scalar.activation(e2[:], e2[:], AF.Ln, bias=eps_c[:])
                nc.scalar.activation(r[:], e2[:], AF.Exp, scale=-0.5)
            psb_mu = pp_bc.tile([128, 512], F32, tag="bc", name="psbmu")
            psb_r = pp_bc.tile([128, 512], F32, tag="bc", name="psbr")
            nc.tensor.matmul(psb_mu[:], onesrow[:], mu[:], start=True, stop=True)
            nc.tensor.matmul(psb_r[:], onesrow[:], r[:], start=True, stop=True)
            for k in range(4):
                tmp = scr.tile([128, 512], F32, tag="lntmp", name="lntmp")
                nc.vector.tensor_tensor(tmp[:], xview(k), psb_mu[:],
                                        op=ALU.subtract)
                nc.vector.tensor_mul(dst_tiles[k][:, dst_cols], tmp[:], psb_r[:])

        def ln1_rows(x_in, nm, rcol_all):
            """Per-frame LN stats; negmu row (K=1 fold operand) + 1/sd cols."""
            for f in range(4):
                sl = slice(f * 512, (f + 1) * 512)
                ps_s = pp_row.tile([1, 512], F32, tag="row_s", name="ps_s")
                ps_q = pp_row.tile([1, 512], F32, tag="row_q", name="ps_q")
                for k in range(4):
                    nc.tensor.matmul(ps_s[:], onescol[:], x_in[k][:, sl],
                                     start=(k == 0), stop=(k == 3))
                for k in range(4):
                    sq = scr.tile([128, 512], F32R, tag="lnsq", name="lnsq")
                    eng = nc.vector if k % 2 else nc.gpsimd
                    eng.tensor_mul(sq[:], x_in[k][:, sl], x_in[k][:, sl])
                    nc.tensor.matmul(ps_q[:], onescol[:], sq[:],
                                     start=(k == 0), stop=(k == 3))
                e2 = rows_p.tile([1, 512], F32, tag="e2", name="le2", bufs=2)
                rr = rows_p.tile([1, 512], F32, tag="rr", name="lrr", bufs=4)
                with nc.allow_low_precision(reason="ln1 rows"):
                    nc.scalar.mul(nm[0:1, sl], ps_s[:], -1.0 / D)
                    msq = rows_p.tile([1, 512], F32, tag="rr", name="lms", bufs=4)
                    nc.scalar.activation(msq[:], ps_s[:], AF.Square,
                                         scale=1.0 / D)
                    nc.vector.scalar_tensor_tensor(
                        e2[:], ps_q[:], 1.0 / D, msq[:], op0=ALU.mult,
                        op1=ALU.subtract)
                    nc.scalar.activation(e2[:], e2[:], AF.Ln, bias=eps_c[:])
                    nc.scalar.activation(rr[:], e2[:], AF.Exp, scale=-0.5)
                for st in range(4):
                    nc.sync.dma_start(
                        rcol_all[:, f * 4 + st:f * 4 + st + 1],
                        rr[0:1, st * 128:(st + 1) * 128])

        tr_ctx = ctx.enter_context(contextlib.ExitStack())
        wp = tr_ctx.enter_context(tc.tile_pool(name="wp", bufs=1))
        wqp = tr_ctx.enter_context(tc.tile_pool(name="wqp", bufs=1))
        attn_p = tr_ctx.enter_context(tc.tile_pool(name="attn", bufs=2))
        sl_p = tr_ctx.enter_context(tc.tile_pool(name="slices", bufs=1))
        ot_p = tr_ctx.enter_context(tc.tile_pool(name="otp", bufs=1))
        otb_all = ot_p.tile([128, 4 * TOK], BF16, tag="otall", name="otall")

        def st_view(xt, k, st):
            # scattered columns {f*512 + st*128 + p} as (128, (f,p)=512)
            return xt[k][:].rearrange("p (f s) -> p f s", f=4)[:, :, st * 128:(st + 1) * 128]

        def transformer_layer(li, x_in, x_mid):
            wq = [wqp.tile([128, 3 * D], F32R, tag=f"wqkv{k}", name=f"wq{k}")
                  for k in range(4)]
            for k in range(4):
                nc.sync.dma_start(wq[k][:], wqkv_d[li, k * 128:(k + 1) * 128, :])
            ws_t = rows_p.tile([1, 3 * D], BF16, tag="wsum", name="wst", bufs=1)
            nc.sync.dma_start(ws_t[:], wsbq_d[li, 0:1, :])
            bq_t = rows_p.tile([1, 3 * D], BF16, tag="bqr", name="bqt", bufs=1)
            nc.sync.dma_start(bq_t[:], wsbq_d[li, 1:2, :])
            cols = stat_p.tile([128, 12], F32, tag="tcols", name="tcols")
            nc.sync.dma_start(cols[:], tcols_d[li])

            # q bias broadcast over the token partitions (k-bias is
            # softmax-invariant; v-bias is folded into outb on the host)
            bias_bc = attn_p.tile([128, D], BF16, tag="biasbc",
                                  name="bias_bc", bufs=1)
            psb = pp_bc.tile([128, 512], F32, tag="bc", name="psbb")
            nc.tensor.matmul(psb[:], ones_bf[:], bq_t[:, 0:512],
                             start=True, stop=True)
            nc.scalar.copy(bias_bc[:], psb[:])

            nm = attn_p.tile([1, TOK], BF16, tag="nmsd", name="nm", bufs=1)
            rcol = attn_p.tile([128, 16], F32, tag="rcol", name="rcol", bufs=1)
            ln1_rows(x_in, nm, rcol)

            for st in range(4):
                qt = attn_p.tile([128, TOK], BF16, tag="qst", name="qt", bufs=2)
                kt = attn_p.tile([128, TOK], BF16, tag="kst", name="kt", bufs=2)
                # v stored (j, d, h) so the AV multiply hits the 2x DVE mode
                vt = attn_p.tile([128, TOK], BF16, tag="vst", name="vt", bufs=2)
                for f in range(NF):
                    c0 = f * 512 + st * 128
                    for ns in range(3):
                        ps = pp_mm.tile([128, 512], F32, tag="mm", name="qps")
                        for k in range(4):
                            nc.tensor.matmul(
                                ps[:], x_in[k][:, c0:c0 + 128],
                                wq[k][:, ns * 512:(ns + 1) * 512],
                                start=(k == 0), stop=False)
                        nc.tensor.matmul(ps[:], nm[0:1, c0:c0 + 128],
                                         ws_t[:, ns * 512:(ns + 1) * 512],
                                         start=False, stop=True)
                        rc = rcol[:, f * 4 + st:f * 4 + st + 1]
                        if ns < 2:
                            nc.scalar.activation(
                                (qt if ns == 0 else kt)[:, f * 512:(f + 1) * 512],
                                ps[:], AF.Copy, scale=rc)
                        else:
                            nc.scalar.activation(
                                vt[:, f * 512:(f + 1) * 512]
                                .rearrange("p (d h) -> p h d", h=8),
                                ps[:].rearrange("p (h d) -> p h d", h=8),
                                AF.Copy, scale=rc)
                # q bias, broadcast over frames, one 2x-mode op per st
                nc.vector.tensor_tensor(
                    qt[:].rearrange("p (f c) -> p f c", f=4),
                    qt[:].rearrange("p (f c) -> p f c", f=4),
                    bias_bc[:].unsqueeze(1).broadcast_to([128, 4, 512]),
                    op=ALU.add)

                s_sc = attn_p.tile([128, 128], F32, tag="s_sc", name="s_sc",
                                   bufs=2)
                k4 = kt[:].rearrange("p (j hd) -> p j hd", j=4)
                for i in range(4):
                    pbig = attn_p.tile([128, TOK], BF16, tag="pbig",
                                       name="pbig", bufs=2)
                    qi = qt[:, i * 512:(i + 1) * 512].unsqueeze(1) \
                        .broadcast_to([128, 4, 512])
                    nc.vector.tensor_mul(
                        pbig[:].rearrange("p (j hd) -> p j hd", j=4), qi, k4)
                    st1 = attn_p.tile([128, TOK // 2], BF16, tag="qks1",
                                      name="qks1", bufs=2)
                    st2 = attn_p.tile([128, TOK // 4], BF16, tag="qks2",
                                      name="qks2", bufs=2)
                    pv = pbig[:].rearrange("p (g d) -> p g d", g=32)
                    nc.vector.tensor_tensor(
                        st1[:].rearrange("p (g d) -> p g d", g=32),
                        pv[:, :, 0:32], pv[:, :, 32:64], op=ALU.add)
                    sv = st1[:].rearrange("p (g d) -> p g d", g=32)
                    nc.vector.tensor_tensor(
                        st2[:].rearrange("p (g d) -> p g d", g=32),
                        sv[:, :, 0:16], sv[:, :, 16:32], op=ALU.add)
                    nc.vector.tensor_reduce(
                        s_sc[:, i * 32:(i + 1) * 32],
                        st2[:].rearrange("p (g d) -> p g d", g=32),
                        axis=AX.X, op=ALU.add)
                # softmax over j without max-subtraction (logits bounded)
                # S cols = (i, j, h)
                eexp = attn_p.tile([128, 128], BF16, tag="eexp", name="eexp",
                                   bufs=2)
                nc.scalar.activation(eexp[:], s_sc[:], AF.Exp)
                z = attn_p.tile([128, 32], F32, tag="z", name="zt", bufs=2)
                nc.vector.tensor_reduce(
                    z[:].rearrange("p (i h) -> p i h", i=4),
                    eexp[:].rearrange("p (i j h) -> p i h j", i=4, j=4),
                    axis=AX.X, op=ALU.add)
                zr = attn_p.tile([128, 32], F32, tag="zr", name="zr", bufs=2)
                nc.vector.reciprocal(zr[:], z[:])
                a_t = attn_p.tile([128, 128], BF16, tag="a_t", name="a_t",
                                  bufs=2)
                nc.vector.tensor_mul(
                    a_t[:].rearrange("p (i j h) -> p i j h", i=4, j=4),
                    eexp[:].rearrange("p (i j h) -> p i j h", i=4, j=4),
                    zr[:].rearrange("p (i h) -> p i h", i=4).unsqueeze(2)
                    .broadcast_to([128, 4, 4, 8]))
                for i in range(4):
                    tbig = attn_p.tile([128, TOK], BF16, tag="tbig", name="tbig", bufs=2)
                    ablk = a_t[:, i * 32:(i + 1) * 32] \
                        .rearrange("p (j h) -> p j h", j=4) \
                        .unsqueeze(2).broadcast_to([128, 4, 64, 8])
                    nc.vector.tensor_mul(
                        tbig[:].rearrange("p (j d h) -> p j d h", j=4, d=64),
                        vt[:].rearrange("p (j d h) -> p j d h", j=4, d=64),
                        ablk)
                    av01 = attn_p.tile([128, 512], BF16, tag="av01", name="av01",
                                       bufs=2)
                    av = attn_p.tile([128, 512], BF16, tag="av", name="av",
                                     bufs=2)
                    nc.vector.tensor_tensor(av01[:], tbig[:, 0:512],
                                            tbig[:, 512:1024], op=ALU.add)
                    nc.vector.tensor_tensor(av[:], tbig[:, 1024:1536],
                                            tbig[:, 1536:2048], op=ALU.add)
                    nc.gpsimd.tensor_tensor(av[:], av01[:], av[:],
                                            op=ALU.add)
                    pst = pp_bc.tile([128, 512], BF16, tag="bc", name="pst")
                    for c in range(4):
                        nc.tensor.transpose(pst[:, c * 128:(c + 1) * 128],
                                            av[:, c * 128:(c + 1) * 128],
                                            ident[:])
                    nc.scalar.copy(
                        otb_all[:].rearrange("p (c t) -> p c t", c=4)
                        [:, :, i * 512 + st * 128:i * 512 + st * 128 + 128],
                        pst[:].rearrange("p (c t) -> p c t", c=4))

            wo = [wp.tile([128, D], BF16, tag=f"wo{k}", name=f"wo{k}")
                  for k in range(4)]
            for k in range(4):
                nc.sync.dma_start(wo[k][:], wo_d[li, k * 128:(k + 1) * 128, :])
            for m in range(4):
                for ns in range(4):
                    ps = pp_mm.tile([128, 512], F32, tag="mm", name="ops")
                    for k in range(4):
                        nc.tensor.matmul(
                            ps[:], wo[k][:, m * 128:(m + 1) * 128],
                            otb_all[:, k * TOK + ns * 512:k * TOK + (ns + 1) * 512],
                            start=(k == 0), stop=(k == 3))
                    nc.vector.scalar_tensor_tensor(
                        x_mid[m][:, ns * 512:(ns + 1) * 512], ps[:],
                        cols[:, 4 + m:5 + m], x_in[m][:, ns * 512:(ns + 1) * 512],
                        op0=ALU.add, op1=ALU.add)

            w1 = [wp.tile([128, D], F32R, tag=f"w1_{k}", name=f"w1_{k}")
                  for k in range(4)]
            w2 = [wp.tile([128, D], F32R, tag=f"w2_{k}", name=f"w2_{k}")
                  for k in range(4)]
            for k in range(4):
                nc.sync.dma_start(w1[k][:], w1_d[li, k * 128:(k + 1) * 128, :])
                nc.sync.dma_start(w2[k][:], w2_d[li, k * 128:(k + 1) * 128, :])
            for ns in range(4):
                xh2 = [sl_p.tile([128, 512], F32R, tag=f"xh2_{k}", name=f"xh2_{k}")
                       for k in range(4)]
                ln_cols(x_mid,
                        lambda k: x_mid[k][:, ns * 512:(ns + 1) * 512],
                        xh2, slice(0, 512))
                hsl = [sl_p.tile([128, 512], F32R, tag=f"h_{m}", name=f"hsl{m}")
                       for m in range(4)]
                for m in range(4):
                    ps = pp_mm.tile([128, 512], F32, tag="mm", name="m1ps")
                    for k in range(4):
                        nc.tensor.matmul(
                            ps[:], w1[k][:, m * 128:(m + 1) * 128], xh2[k][:],
                            start=(k == 0), stop=(k == 3))
                    nc.scalar.activation(hsl[m][:], ps[:], AF.Gelu_apprx_tanh,
                                         bias=cols[:, m:m + 1])
                for m in range(4):
                    ps = pp_mm.tile([128, 512], F32, tag="mm", name="m2ps")
                    for k in range(4):
                        nc.tensor.matmul(
                            ps[:], w2[k][:, m * 128:(m + 1) * 128], hsl[k][:],
                            start=(k == 0), stop=(k == 3))
                    nc.vector.scalar_tensor_tensor(
                        x_mid[m][:, ns * 512:(ns + 1) * 512], ps[:],
                        cols[:, 8 + m:9 + m], x_mid[m][:, ns * 512:(ns + 1) * 512],
                        op0=ALU.add, op1=ALU.add)

        cur = xA
        _nl = int(os.environ.get("KNLAYERS", NLAYER))
        for li in range(_nl):
            transformer_layer(li, cur, cur)

        tr_ctx.close()

        # ------------------------------------------------------------------
        # heads
        # ------------------------------------------------------------------
        _skip_heads = os.environ.get("KHEADS", "1") == "0"
        if _skip_heads:
            nc.gpsimd.dma_start(d6_o[:], cur[0][0:8, :])
            nc.gpsimd.dma_start(tr_o[:], cur[1][0:8, :])
        with tc.tile_pool(name="heads", bufs=1) as hp, \
             tc.tile_pool(name="whp", bufs=1) as whp:
          if not _skip_heads:
              hc = stat_p.tile([128, 10], F32, tag="hcols", name="hc")
              nc.sync.dma_start(hc[:], hcols_d[:])
              sb3 = stat_p.tile([8, 2], F32, tag="sb3", name="sb3")
              nc.sync.dma_start(sb3[:], sb3_d[:])

              xhf = [hp.tile([128, TOK], F32R, tag=f"xhf{k}", name=f"xhf{k}")
                     for k in range(4)]
              for st in range(4):
                  ln_cols(cur, lambda k: cur[k][:, st * 512:(st + 1) * 512],
                          xhf, slice(st * 512, (st + 1) * 512))

              xp = [hp.tile([128, TOK], F32R, tag=f"xp{k}", name=f"xp{k}")
                    for k in range(4)]

              def mm_head(src_tiles, wt_dram, kdim, mdim, dst_tiles, evict):
                  KC = kdim // 128
                  MC = max(mdim // 128, 1)
                  wsb = [whp.tile([128, mdim], F32R, tag=f"wh_{kdim}_{mdim}_{k}",
                                  name=f"wh{k}") for k in range(KC)]
                  for k in range(KC):
                      nc.sync.dma_start(wsb[k][:], wt_dram[k * 128:(k + 1) * 128, :])
                  for m in range(MC):
                      for ns in range(4):
                          ps = pp_mm.tile([128, 512], F32, tag="mm", name="hps")
                          for k in range(KC):
                              nc.tensor.matmul(
                                  ps[:], wsb[k][:, m * 128:(m + 1) * 128],
                                  src_tiles[k][:, ns * 512:(ns + 1) * 512],
                                  start=(k == 0), stop=(k == KC - 1))
                          evict(ps, dst_tiles[m], m, ns)

              mm_head(xhf, projw_d, D, D, xp,
                      lambda ps, dst, m, ns: nc.scalar.activation(
                          dst[:, ns * 512:(ns + 1) * 512], ps[:], AF.Identity,
                          bias=hc[:, m:m + 1]))

              def branch(w1d, w2d, w3d, b1ofs, b2ofs, out_dram, b3col, r1, r2, pfx):
                  mm_head(xp, w1d, D, 256, r1,
                          lambda ps, dst, m, ns: nc.scalar.activation(
                              dst[:, ns * 512:(ns + 1) * 512], ps[:], AF.Relu,
                              bias=hc[:, b1ofs + m:b1ofs + m + 1]))
                  mm_head(r1, w2d, 256, 128, r2,
                          lambda ps, dst, m, ns: nc.scalar.activation(
                              dst[:, ns * 512:(ns + 1) * 512], ps[:], AF.Relu,
                              bias=hc[:, b2ofs:b2ofs + 1]))
                  w3 = whp.tile([128, 8], F32R, tag=f"w3{pfx}", name="w3")
                  nc.sync.dma_start(w3[:], w3d[:])
                  out_sb = hp.tile([8, TOK], F32, tag=f"{pfx}out", name=f"{pfx}out")
                  for ns in range(4):
                      ps = pp_mm.tile([8, 512], F32, tag="mm", name="bps")
                      nc.tensor.matmul(ps[:], w3[:], r2[0][:, ns * 512:(ns + 1) * 512],
                                       start=True, stop=True)
                      nc.scalar.activation(out_sb[:, ns * 512:(ns + 1) * 512], ps[:],
                                           AF.Identity, bias=b3col)
                      nc.sync.dma_start(out_dram[:, ns * 512:(ns + 1) * 512],
                                        out_sb[:, ns * 512:(ns + 1) * 512])

              # reuse dead transformer buffers for intermediates
              branch(rw1_d, rw2_d, rw3_d, 4, 8, d6_o, sb3[:, 0:1],
                     [cur[0], cur[1]], [xhf[0]], "r")
              branch(tw1_d, tw2_d, tw3_d, 6, 9, tr_o, sb3[:, 1:2],
                     [cur[2], cur[3]], [xhf[1]], "t")

    nc.compile()
    return nc


# ----------------------------------------------------------------------------
# host side
# ----------------------------------------------------------------------------

_CACHE = {}


def _normalize_np(v, eps=1e-12):
    return v / np.maximum(np.linalg.norm(v, axis=-1, keepdims=True), eps)


def _rot6d_np(d6):
    a1, a2 = d6[..., :3], d6[..., 3:]
    b1 = _normalize_np(a1)
    b2 = _normalize_np(a2 - np.sum(b1 * a2, -1, keepdims=True) * b1)
    b3 = np.cross(b1, b2)
    return np.stack([b1, b2, b3], axis=-2)


def _prep_weights(inp):
    f32 = np.float32
    wmap = {}
    for i, cw in enumerate(['c1w', 'c2w', 'c3w', 'c4w']):
        wmap[f'convw{i}'] = np.ascontiguousarray(inp[cw].T.astype(f32))
    for i, (g, b2) in enumerate([('bn1g', 'bn1b'), ('bn2g', 'bn2b'),
                                 ('bn3g', 'bn3b')]):
        M = CONV_DIMS[i + 1] // 128
        bn = np.concatenate([
            inp[g].reshape(M, 128).T, inp[b2].reshape(M, 128).T,
            inp[f'c{i + 1}b'].reshape(M, 128).T], axis=1)
        wmap[f'bnconst{i}'] = np.ascontiguousarray(bn.astype(f32))
    pe = _pe_table()[:NF]
    b4 = inp['c4b'][None, :].astype(f32) + pe                   # (4, 512)
    # cols: m*4 + pt ; frame index == pt
    wmap['bias4'] = np.ascontiguousarray(
        b4.reshape(NF, 4, 128).transpose(2, 1, 0).reshape(128, 16).astype(f32))

    qkvw = np.array(inp['qkvw'], f32)
    qkvb = np.array(inp['qkvb'], f32)
    qkvw[:, :, :512] /= math.sqrt(DH)
    qkvb[:, :512] /= math.sqrt(DH)
    g1 = np.array(inp['ln1g'], f32)
    b1 = np.array(inp['ln1b'], f32)
    wq_fold = g1[:, :, None] * qkvw
    bq_fold = qkvb + np.einsum('ld,ldf->lf', b1, qkvw)
    wmap['wqkv'] = np.ascontiguousarray(wq_fold.astype(f32))
    wsbq = np.concatenate([wq_fold.sum(axis=1, keepdims=True),
                           bq_fold[:, None, :]], axis=1)         # (L, 2, 3D)
    wmap['wsbq'] = np.ascontiguousarray(wsbq.astype(ml_dtypes.bfloat16))
    # attention output features are (d, h)-ordered; permute wo rows to match
    wo_ = np.array(inp['outw'], f32)                             # (L, 512, 512)
    d_idx, h_idx = np.meshgrid(np.arange(DH), np.arange(HEADS), indexing='ij')
    perm = (h_idx * DH + d_idx).reshape(512)     # perm[d*8+h] = h*64+d
    wmap['wo'] = np.ascontiguousarray(wo_[:, perm, :]
                                      .astype(ml_dtypes.bfloat16))
    g2 = np.array(inp['ln2g'], f32)
    bl2 = np.array(inp['ln2b'], f32)
    m1w = np.array(inp['m1w'], f32)
    w1_fold = g2[:, :, None] * m1w
    b1_fold = np.array(inp['m1b'], f32) + np.einsum('ld,ldf->lf', bl2, m1w)
    wmap['w1'] = np.ascontiguousarray(w1_fold.astype(f32))
    wmap['w2'] = np.ascontiguousarray(np.array(inp['m2w'], f32))
    cols = np.zeros((NLAYER, 128, 12), f32)
    cols[:, :, 0:4] = b1_fold.reshape(NLAYER, 4, 128).transpose(0, 2, 1)
    # v-bias is dropped at the v eviction; fold bv @ Wo into outb instead
    bv = bq_fold[:, 2 * 512:3 * 512]                             # (L, 512)
    outb_fold = np.array(inp['outb'], f32) + np.einsum('lk,lko->lo', bv, wo_)
    cols[:, :, 4:8] = outb_fold.reshape(NLAYER, 4, 128).transpose(0, 2, 1)
    cols[:, :, 8:12] = np.array(inp['m2b'], f32).reshape(NLAYER, 4, 128) \
        .transpose(0, 2, 1)
    wmap['tcols'] = cols

    gf_ = np.array(inp['lnfg'], f32)
    bf_ = np.array(inp['lnfb'], f32)
    projw = np.array(inp['projw'], f32)
    wmap['projw'] = np.ascontiguousarray(gf_[:, None] * projw)
    projb_fold = np.array(inp['projb'], f32) + bf_ @ projw
    wmap['rw1'] = np.ascontiguousarray(np.array(inp['rw1'], f32))
    wmap['rw2'] = np.ascontiguousarray(np.array(inp['rw2'], f32))
    rw3 = np.zeros((128, 8), f32)
    rw3[:, :6] = np.array(inp['rw3'], f32)
    wmap['rw3'] = rw3
    wmap['tw1'] = np.ascontiguousarray(np.array(inp['tw1'], f32))
    wmap['tw2'] = np.ascontiguousarray(np.array(inp['tw2'], f32))
    tw3 = np.zeros((128, 8), f32)
    tw3[:, :3] = np.array(inp['tw3'], f32)
    wmap['tw3'] = tw3
    hcols = np.zeros((128, 10), f32)
    hcols[:, 0:4] = projb_fold.reshape(4, 128).T
    hcols[:, 4:6] = np.array(inp['rb1'], f32).reshape(2, 128).T
    hcols[:, 6:8] = np.array(inp['tb1'], f32).reshape(2, 128).T
    hcols[:, 8] = np.array(inp['rb2'], f32)
    hcols[:, 9] = np.array(inp['tb2'], f32)
    wmap['hcols'] = hcols
    sb3 = np.zeros((8, 2), f32)
    sb3[0:6, 0] = np.array(inp['rb3'], f32)
    sb3[0:3, 1] = np.array(inp['tb3'], f32)
    wmap['sb3'] = sb3
    wmap['ones_c'] = np.ones((128, 128), f32)
    return wmap


def kernel(**inputs):
    inp = {k: np.asarray(v) for k, v in inputs.items()}

    idx = inp['seed_idxs'].reshape(B, -1).astype(np.int64)      # (B, N)
    sel_seed = np.take_along_axis(np.asarray(inp['fp2_features'], np.float32),
                                  idx[:, None, :], axis=2)
    sel_grasp = np.take_along_axis(np.asarray(inp['local_grasp_features'], np.float32),
                                   idx[:, None, :], axis=2)
    sel_color = np.take_along_axis(np.asarray(inp['local_color_features'], np.float32),
                                   idx[:, None, :], axis=2)
    sel_pose = np.take_along_axis(np.asarray(inp['grasp_pose_feature'], np.float32),
                                  idx[:, None, :], axis=2)
    gsf = np.asarray(inp['sa4_features'], np.float32).max(axis=-1)
    gsf = np.broadcast_to(gsf[:, :, None], (B, 256, NPTS))
    fused = sel_pose + np.concatenate([sel_grasp, sel_color, sel_seed, gsf], axis=1)
    gf = fused.reshape(BE, FRAME, 1024, NPTS)
    cond = np.broadcast_to(gf[:, :1], (BE, NF, 1024, NPTS))
    X = np.concatenate([cond, gf[:, 1:]], axis=2)               # (e, f, 2048, N)

    if 'nc' not in _CACHE:
        _CACHE['nc'] = build_kernel()
    nc = _CACHE['nc']
    wmap = _prep_weights(inp)

    in_maps = []
    for k in range(NCORES):
        xc = X[:, :, :, k * NPC:(k + 1) * NPC]                  # (e, f, c, n)
        xc = xc.transpose(2, 1, 0, 3).reshape(2048, TOK)        # (c, (f,e,n))
        m = dict(wmap)
        m['xin'] = np.ascontiguousarray(xc, dtype=np.float32)
        in_maps.append(m)

    res = run_bass_kernel_spmd(nc, in_maps, core_ids=list(range(NCORES)))

    out = np.zeros((BE * NPTS, NF, 12), np.float32)
    for k in range(NCORES):
        d6 = res.results[k]['d6'][:6]
        tr = res.results[k]['tr3'][:3]
        d6 = d6.reshape(6, NF, BE, NPC).transpose(2, 3, 1, 0)   # (e, n, f, 6)
        tr = tr.reshape(3, NF, BE, NPC).transpose(2, 3, 1, 0)
        rot = _rot6d_np(d6).reshape(BE, NPC, NF, 9)
        for e in range(BE):
            rows = slice(e * NPTS + k * NPC, e * NPTS + (k + 1) * NPC)
            out[rows, :, 0:3] = tr[e]
            out[rows, :, 3:12] = rot[e]
    return out


if __name__ == "__main__":
    build_kernel()
    print("built ok")



# revision 71
# speedup vs baseline: 1.1065x; 1.0059x over previous
import math
import os
import numpy as np
import ml_dtypes
import contextlib

import concourse.bass as bass
import concourse.tile as tile
from concourse import bacc, mybir, masks
from concourse.bass_utils import run_bass_kernel_spmd

F32 = mybir.dt.float32
F32R = mybir.dt.float32r
BF16 = mybir.dt.bfloat16
ALU = mybir.AluOpType
AF = mybir.ActivationFunctionType
AX = mybir.AxisListType

NCORES = 8
FRAME = 5
NF = FRAME - 1
D = 512
DH = 64
HEADS = 8
B = 20
NPTS = 1024
BE = B // FRAME
NPC = NPTS // NCORES     # 128 points per core
TOK = NF * BE * NPC      # 2048 tokens per core
NLAYER = 12
LNEPS = 1e-5
BNEPS = 1e-5
NBN = 16 * NPTS

CONV_DIMS = [2048, 1536, 1024, 768, 512]


def _pe_table(max_len=16, d=D):
    pos = np.arange(max_len, dtype=np.float32)[:, None]
    div = np.exp(np.arange(0, d, 2, dtype=np.float32) * (-math.log(10000.0) / d))
    pe = np.zeros((max_len, d), np.float32)
    pe[:, 0::2] = np.sin(pos * div)
    pe[:, 1::2] = np.cos(pos * div)
    return pe


def build_kernel():
    nc = bacc.Bacc("TRN2", target_bir_lowering=False, debug=False,
                   num_devices=NCORES)

    xin = nc.dram_tensor("xin", [CONV_DIMS[0], TOK], F32R, kind="ExternalInput").ap()
    convw = [nc.dram_tensor(f"convw{i}", [CONV_DIMS[i], CONV_DIMS[i + 1]], F32R,
                            kind="ExternalInput").ap() for i in range(4)]
    bnconst = [nc.dram_tensor(f"bnconst{i}", [128, 3 * (CONV_DIMS[i + 1] // 128)],
                              F32, kind="ExternalInput").ap() for i in range(3)]
    bias4 = nc.dram_tensor("bias4", [128, 4 * NF], F32, kind="ExternalInput").ap()

    wqkv_d = nc.dram_tensor("wqkv", [NLAYER, D, 3 * D], F32R, kind="ExternalInput").ap()
    wsbq_d = nc.dram_tensor("wsbq", [NLAYER, 2, 3 * D], BF16, kind="ExternalInput").ap()
    wo_d = nc.dram_tensor("wo", [NLAYER, D, D], BF16, kind="ExternalInput").ap()
    w1_d = nc.dram_tensor("w1", [NLAYER, D, D], F32R, kind="ExternalInput").ap()
    w2_d = nc.dram_tensor("w2", [NLAYER, D, D], F32R, kind="ExternalInput").ap()
    tcols_d = nc.dram_tensor("tcols", [NLAYER, 128, 12], F32, kind="ExternalInput").ap()

    projw_d = nc.dram_tensor("projw", [D, D], F32R, kind="ExternalInput").ap()
    rw1_d = nc.dram_tensor("rw1", [D, 256], F32R, kind="ExternalInput").ap()
    rw2_d = nc.dram_tensor("rw2", [256, 128], F32R, kind="ExternalInput").ap()
    rw3_d = nc.dram_tensor("rw3", [128, 8], F32R, kind="ExternalInput").ap()
    tw1_d = nc.dram_tensor("tw1", [D, 256], F32R, kind="ExternalInput").ap()
    tw2_d = nc.dram_tensor("tw2", [256, 128], F32R, kind="ExternalInput").ap()
    tw3_d = nc.dram_tensor("tw3", [128, 8], F32R, kind="ExternalInput").ap()
    hcols_d = nc.dram_tensor("hcols", [128, 10], F32, kind="ExternalInput").ap()
    sb3_d = nc.dram_tensor("sb3", [8, 2], F32, kind="ExternalInput").ap()
    ones_d = nc.dram_tensor("ones_c", [128, 128], F32, kind="ExternalInput").ap()

    d6_o = nc.dram_tensor("d6", [8, TOK], F32, kind="ExternalOutput").ap()
    tr_o = nc.dram_tensor("tr3", [8, TOK], F32, kind="ExternalOutput").ap()

    with tile.TileContext(nc) as tc, contextlib.ExitStack() as ctx:
        const_p = ctx.enter_context(tc.tile_pool(name="consts", bufs=1))
        onescol = const_p.tile([128, 1], F32R)
        onesrow = const_p.tile([1, 128], F32R)
        ident = const_p.tile([128, 128], BF16)
        nc.gpsimd.dma_start(onescol[:], ones_d[:, 0:1])
        nc.gpsimd.dma_start(onesrow[:], ones_d[0:1, :])
        ones_bf = const_p.tile([1, 128], BF16)
        nc.vector.memset(ones_bf[:], 1.0)
        eps_c = const_p.tile([1, 1], F32)
        nc.vector.memset(eps_c[:], LNEPS)
        masks.make_identity(nc, ident[:])

        xs_p = ctx.enter_context(tc.tile_pool(name="xstate", bufs=1))
        xA = [xs_p.tile([128, TOK], F32R, tag=f"xA{m}", name=f"xA{m}") for m in range(4)]

        stat_p = ctx.enter_context(tc.tile_pool(name="stats", bufs=1))
        dram_p = ctx.enter_context(tc.tile_pool(name="dramb", bufs=1, space="DRAM"))

        pp_mm = ctx.enter_context(tc.tile_pool(name="ppmm", bufs=4, space="PSUM"))
        pp_row = ctx.enter_context(tc.tile_pool(name="pprow", bufs=1, space="PSUM"))
        pp_bc = ctx.enter_context(tc.tile_pool(name="ppbc", bufs=2, space="PSUM"))

        y_dram = [dram_p.tile([CONV_DIMS[i], TOK], F32R, tag=f"ydram{i}", name=f"ydram{i}")
                  for i in range(1, 4)]

        # ------------------------------------------------------------------
        # conv stack (activations spilled to DRAM, BN applied on load)
        # ------------------------------------------------------------------
        b4sb = stat_p.tile([128, 4 * NF], F32, tag="b4")
        nc.sync.dma_start(b4sb[:], bias4[:])

        bn_s = {}
        bn_t = {}

        def conv_layer(li, wcp, cxp, pp_conv):
            kdim, mdim = CONV_DIMS[li - 1], CONV_DIMS[li]
            KC, MC = kdim // 128, mdim // 128
            src = xin if li == 1 else y_dram[li - 2]
            with_bn = li < 4
            if with_bn:
                sum_acc = stat_p.tile([128, MC * 4], F32, tag=f"sum{li}")
                sq_acc = stat_p.tile([128, MC * 4], F32, tag=f"sq{li}")
            cond_sb = None
            if li == 1:
                # channels 0:1024 repeat frame 0's features for all 4 frames;
                # compute their contribution once and add it at eviction
                KC = 8
                xc = cxp.tile([128, 8 * 512], F32R, tag="convc", name="convc",
                              bufs=1)
                nc.sync.dma_start(
                    xc[:].rearrange("p (k c) -> p k c", k=8),
                    src[0:1024, 0:512].rearrange("(k p) c -> p k c", p=128))
                cond_sb = [cxp.tile([128, 512], F32, tag=f"cond{m}",
                                    name=f"cond{m}") for m in range(MC)]
                for m in range(MC):
                    wslc = wcp.tile([128, 8 * 128], F32R, tag="wslc",
                                    name="wslc", bufs=3)
                    wvc = wslc[:].rearrange("p (k c) -> p k c", k=8)
                    nc.sync.dma_start(
                        wvc, convw[0][0:1024, m * 128:(m + 1) * 128]
                        .rearrange("(k p) c -> p k c", p=128))
                    ps = pp_conv.tile([128, 512], F32, tag="mm", name="ccps")
                    for k in range(8):
                        nc.tensor.matmul(
                            ps[:], wvc[:, k, :], xc[:, k * 512:(k + 1) * 512],
                            start=(k == 0), stop=(k == 7))
                    nc.scalar.copy(cond_sb[m][:], ps[:])
            for pt in range(4):
                xt = cxp.tile([128, KC * 512], F32R, tag="convx", name="convx",
                              bufs=3)
                nc.sync.dma_start(
                    xt[:].rearrange("p (k c) -> p k c", k=KC),
                    src[kdim - KC * 128:, pt * 512:(pt + 1) * 512]
                    .rearrange("(k p) c -> p k c", p=128))
                if li > 1:
                    s_p, t_p = bn_s[li - 1], bn_t[li - 1]
                    for k in range(KC):
                        nc.scalar.activation(
                            xt[:, k * 512:(k + 1) * 512],
                            xt[:, k * 512:(k + 1) * 512],
                            AF.Relu, bias=t_p[:, k:k + 1], scale=s_p[:, k:k + 1])
                for m in range(MC):
                    wsl = wcp.tile([128, KC * 128], F32R, tag="wsl", name="wsl",
                                   bufs=6)
                    wv = wsl[:].rearrange("p (k c) -> p k c", k=KC)
                    nc.sync.dma_start(
                        wv, convw[li - 1][kdim - KC * 128:,
                                          m * 128:(m + 1) * 128]
                        .rearrange("(k p) c -> p k c", p=128))
                    ps = pp_conv.tile([128, 512], F32, tag="mm", name="cps")
                    for k in range(KC):
                        nc.tensor.matmul(
                            ps[:], wv[:, k, :], xt[:, k * 512:(k + 1) * 512],
                            start=(k == 0), stop=(k == KC - 1))
                    if with_bn:
                        ot = cxp.tile([128, 512], F32R, tag="convot", name="cot",
                                      bufs=4)
                        if cond_sb is not None:
                            nc.vector.tensor_tensor(ot[:], ps[:],
                                                    cond_sb[m][:], op=ALU.add)
                            stats_src = ot[:]
                        else:
                            stats_src = ps[:]
                            nc.scalar.activation(
                                ot[:], ps[:], AF.Copy,
                                accum_out=sum_acc[:, m * 4 + pt:m * 4 + pt + 1])
                        sqs = cxp.tile([128, 512], BF16, tag="sqscr", name="sqs",
                                       bufs=4)
                        if cond_sb is not None:
                            nc.scalar.activation(
                                sqs[:], stats_src, AF.Copy,
                                accum_out=sum_acc[:, m * 4 + pt:m * 4 + pt + 1])
                        nc.scalar.activation(
                            sqs[:], stats_src, AF.Square,
                            accum_out=sq_acc[:, m * 4 + pt:m * 4 + pt + 1])
                        nc.sync.dma_start(
                            y_dram[li - 1][m * 128:(m + 1) * 128,
                                           pt * 512:(pt + 1) * 512], ot[:])
                    else:
                        nc.scalar.activation(
                            xA[m][:, pt * 512:(pt + 1) * 512], ps[:], AF.Identity,
                            bias=b4sb[:, m * 4 + pt:m * 4 + pt + 1])
            if not with_bn:
                return
            allin = stat_p.tile([128, 2 * MC], F32, tag=f"ain{li}", name="allin")
            nc.vector.tensor_reduce(
                allin[:, 0:MC], sum_acc[:].rearrange("p (m t) -> p m t", m=MC),
                axis=AX.X, op=ALU.add)
            nc.vector.tensor_reduce(
                allin[:, MC:2 * MC], sq_acc[:].rearrange("p (m t) -> p m t", m=MC),
                axis=AX.X, op=ALU.add)
            bin_ = dram_p.tile([128, 2 * MC], F32, tag=f"arin{li}", name="arin")
            bout = dram_p.tile([128, 2 * MC], F32, tag=f"arout{li}", name="arout")
            nc.sync.dma_start(bin_[:], allin[:])
            nc.gpsimd.collective_compute(
                "AllReduce", ALU.add, replica_groups=[list(range(NCORES))],
                ins=[bin_.opt()], outs=[bout.opt()])
            gl = stat_p.tile([128, 2 * MC], F32, tag=f"gl{li}", name="gl")
            nc.sync.dma_start(gl[:], bout[:])
            cst = stat_p.tile([128, 3 * MC], F32, tag=f"cst{li}", name="cst")
            nc.sync.dma_start(cst[:], bnconst[li - 1][:])
            mu = stat_p.tile([128, MC], F32, tag=f"mu{li}", name="bmu")
            var = stat_p.tile([128, MC], F32, tag=f"va{li}", name="bvar")
            s_t = stat_p.tile([128, MC], F32, tag=f"s{li}", name="bs")
            t_t = stat_p.tile([128, MC], F32, tag=f"t{li}", name="bt")
            nc.scalar.mul(mu[:], gl[:, 0:MC], 1.0 / NBN)
            nc.scalar.mul(var[:], gl[:, MC:2 * MC], 1.0 / NBN)
            msq = stat_p.tile([128, MC], F32, tag=f"ms{li}", name="bmsq")
            nc.vector.tensor_mul(msq[:], mu[:], mu[:])
            nc.vector.tensor_tensor(var[:], var[:], msq[:], op=ALU.subtract)
            nc.vector.tensor_scalar(var[:], var[:], BNEPS, None, op0=ALU.add)
            sd = stat_p.tile([128, MC], F32, tag=f"sd{li}", name="bsd")
            nc.scalar.activation(sd[:], var[:], AF.Sqrt)
            rsd = stat_p.tile([128, MC], F32, tag=f"rs{li}", name="brsd")
            nc.vector.reciprocal(rsd[:], sd[:])
            nc.vector.tensor_mul(s_t[:], rsd[:], cst[:, 0:MC])
            nc.vector.tensor_mul(t_t[:], mu[:], s_t[:])
            nc.vector.tensor_tensor(t_t[:], cst[:, MC:2 * MC], t_t[:],
                                    op=ALU.subtract)
            bn_s[li], bn_t[li] = s_t, t_t

        with tc.tile_pool(name="wcp", bufs=1) as wcp, \
             tc.tile_pool(name="cxp", bufs=1) as cxp:
            for li in (1, 2, 3, 4):
                conv_layer(li, wcp, cxp, pp_mm)

        # ------------------------------------------------------------------
        # transformer
        # ------------------------------------------------------------------
        rows_p = ctx.enter_context(tc.tile_pool(name="rows", bufs=1))
        scr = ctx.enter_context(tc.tile_pool(name="scratch", bufs=2))

        def ln_cols(xt, xview, dst_tiles, dst_cols):
            """LN per token over feature dim (stats + apply on DVE/Pool)."""
            ps_s = pp_row.tile([1, 512], F32, tag="row_s", name="ps_s")
            ps_q = pp_row.tile([1, 512], F32, tag="row_q", name="ps_q")
            for k in range(4):
                nc.tensor.matmul(ps_s[:], onescol[:], xview(k),
                                 start=(k == 0), stop=(k == 3))
            for k in range(4):
                sq = scr.tile([128, 512], F32R, tag="lnsq", name="lnsq", bufs=3)
                eng = nc.vector if k % 2 else nc.gpsimd
                eng.tensor_mul(sq[:], xview(k), xview(k))
                nc.tensor.matmul(ps_q[:], onescol[:], sq[:],
                                 start=(k == 0), stop=(k == 3))
            mu = rows_p.tile([1, 512], F32R, tag="mu", name="lmu", bufs=2)
            e2 = rows_p.tile([1, 512], F32, tag="e2", name="le2", bufs=2)
            r = rows_p.tile([1, 512], F32R, tag="r", name="lr", bufs=2)
            nc.scalar.mul(mu[:], ps_s[:], 1.0 / D)
            with nc.allow_low_precision(reason="f32r row math"):
                nc.vector.tensor_mul(r[:], mu[:], mu[:])
                nc.vector.scalar_tensor_tensor(
                    e2[:], ps_q[:], 1.0 / D, r[:], op0=ALU.mult,
                    op1=ALU.subtract)
                # 1/sqrt(v) = exp(-0.5 ln(v + eps)): single exp/ln act table
                nc.scalar.activation(e2[:], e2[:], AF.Ln, bias=eps_c[:])
                nc.scalar.activation(r[:], e2[:], AF.Exp, scale=-0.5)
            psb_mu = pp_bc.tile([128, 512], F32, tag="bc", name="psbmu")
            psb_r = pp_bc.tile([128, 512], F32, tag="bc", name="psbr")
            nc.tensor.matmul(psb_mu[:], onesrow[:], mu[:], start=True, stop=True)
            nc.tensor.matmul(psb_r[:], onesrow[:], r[:], start=True, stop=True)
            for k in range(4):
                tmp = scr.tile([128, 512], F32, tag="lntmp", name="lntmp")
                nc.vector.tensor_tensor(tmp[:], xview(k), psb_mu[:],
                                        op=ALU.subtract)
                nc.vector.tensor_mul(dst_tiles[k][:, dst_cols], tmp[:], psb_r[:])

        def ln1_rows(x_in, nm, rcol_all):
            """Per-frame LN stats; negmu row (K=1 fold operand) + 1/sd cols."""
            for f in range(4):
                sl = slice(f * 512, (f + 1) * 512)
                ps_s = pp_row.tile([1, 512], F32, tag="row_s", name="ps_s")
                ps_q = pp_row.tile([1, 512], F32, tag="row_q", name="ps_q")
                for k in range(4):
                    nc.tensor.matmul(ps_s[:], onescol[:], x_in[k][:, sl],
                                     start=(k == 0), stop=(k == 3))
                for k in range(4):
                    sq = scr.tile([128, 512], F32R, tag="lnsq", name="lnsq", bufs=3)
                    eng = nc.vector if k % 2 else nc.gpsimd
                    eng.tensor_mul(sq[:], x_in[k][:, sl], x_in[k][:, sl])
                    nc.tensor.matmul(ps_q[:], onescol[:], sq[:],
                                     start=(k == 0), stop=(k == 3))
                e2 = rows_p.tile([1, 512], F32, tag="e2", name="le2", bufs=2)
                rr = rows_p.tile([1, 512], F32, tag="rr", name="lrr", bufs=4)
                with nc.allow_low_precision(reason="ln1 rows"):
                    nc.scalar.mul(nm[0:1, sl], ps_s[:], -1.0 / D)
                    msq = rows_p.tile([1, 512], F32, tag="rr", name="lms", bufs=4)
                    nc.scalar.activation(msq[:], ps_s[:], AF.Square,
                                         scale=1.0 / D)
                    nc.vector.scalar_tensor_tensor(
                        e2[:], ps_q[:], 1.0 / D, msq[:], op0=ALU.mult,
                        op1=ALU.subtract)
                    nc.scalar.activation(e2[:], e2[:], AF.Ln, bias=eps_c[:])
                    nc.scalar.activation(rr[:], e2[:], AF.Exp, scale=-0.5)
                for st in range(4):
                    nc.sync.dma_start(
                        rcol_all[:, f * 4 + st:f * 4 + st + 1],
                        rr[0:1, st * 128:(st + 1) * 128])

        tr_ctx = ctx.enter_context(contextlib.ExitStack())
        wp = tr_ctx.enter_context(tc.tile_pool(name="wp", bufs=1))
        wqp = tr_ctx.enter_context(tc.tile_pool(name="wqp", bufs=1))
        attn_p = tr_ctx.enter_context(tc.tile_pool(name="attn", bufs=2))
        sl_p = tr_ctx.enter_context(tc.tile_pool(name="slices", bufs=1))
        ot_p = tr_ctx.enter_context(tc.tile_pool(name="otp", bufs=1))
        otb_all = ot_p.tile([128, 4 * TOK], BF16, tag="otall", name="otall")

        def st_view(xt, k, st):
            # scattered columns {f*512 + st*128 + p} as (128, (f,p)=512)
            return xt[k][:].rearrange("p (f s) -> p f s", f=4)[:, :, st * 128:(st + 1) * 128]

        def transformer_layer(li, x_in, x_mid):
            wq = [wqp.tile([128, 3 * D], F32R, tag=f"wqkv{k}", name=f"wq{k}")
                  for k in range(4)]
            for k in range(4):
                nc.sync.dma_start(wq[k][:], wqkv_d[li, k * 128:(k + 1) * 128, :])
            ws_t = rows_p.tile([1, 3 * D], BF16, tag="wsum", name="wst", bufs=1)
            nc.sync.dma_start(ws_t[:], wsbq_d[li, 0:1, :])
            bq_t = rows_p.tile([1, 3 * D], BF16, tag="bqr", name="bqt", bufs=1)
            nc.sync.dma_start(bq_t[:], wsbq_d[li, 1:2, :])
            cols = stat_p.tile([128, 12], F32, tag="tcols", name="tcols")
            nc.sync.dma_start(cols[:], tcols_d[li])

            # q bias broadcast over the token partitions (k-bias is
            # softmax-invariant; v-bias is folded into outb on the host)
            bias_bc = attn_p.tile([128, D], BF16, tag="biasbc",
                                  name="bias_bc", bufs=1)
            psb = pp_bc.tile([128, 512], F32, tag="bc", name="psbb")
            nc.tensor.matmul(psb[:], ones_bf[:], bq_t[:, 0:512],
                             start=True, stop=True)
            nc.scalar.copy(bias_bc[:], psb[:])

            nm = attn_p.tile([1, TOK], BF16, tag="nmsd", name="nm", bufs=1)
            rcol = attn_p.tile([128, 16], F32, tag="rcol", name="rcol", bufs=1)
            ln1_rows(x_in, nm, rcol)

            for st in range(4):
                qt = attn_p.tile([128, TOK], BF16, tag="qst", name="qt", bufs=2)
                kt = attn_p.tile([128, TOK], BF16, tag="kst", name="kt", bufs=2)
                # v stored (j, d, h) so the AV multiply hits the 2x DVE mode
                vt = attn_p.tile([128, TOK], BF16, tag="vst", name="vt", bufs=2)
                for f in range(NF):
                    c0 = f * 512 + st * 128
                    for ns in range(3):
                        ps = pp_mm.tile([128, 512], F32, tag="mm", name="qps")
                        for k in range(4):
                            nc.tensor.matmul(
                                ps[:], x_in[k][:, c0:c0 + 128],
                                wq[k][:, ns * 512:(ns + 1) * 512],
                                start=(k == 0), stop=False)
                        nc.tensor.matmul(ps[:], nm[0:1, c0:c0 + 128],
                                         ws_t[:, ns * 512:(ns + 1) * 512],
                                         start=False, stop=True)
                        rc = rcol[:, f * 4 + st:f * 4 + st + 1]
                        if ns < 2:
                            nc.scalar.activation(
                                (qt if ns == 0 else kt)[:, f * 512:(f + 1) * 512],
                                ps[:], AF.Copy, scale=rc)
                        else:
                            nc.scalar.activation(
                                vt[:, f * 512:(f + 1) * 512]
                                .rearrange("p (d h) -> p h d", h=8),
                                ps[:].rearrange("p (h d) -> p h d", h=8),
                                AF.Copy, scale=rc)
                # q bias, broadcast over frames, one 2x-mode op per st
                nc.vector.tensor_tensor(
                    qt[:].rearrange("p (f c) -> p f c", f=4),
                    qt[:].rearrange("p (f c) -> p f c", f=4),
                    bias_bc[:].unsqueeze(1).broadcast_to([128, 4, 512]),
                    op=ALU.add)

                s_sc = attn_p.tile([128, 128], F32, tag="s_sc", name="s_sc",
                                   bufs=2)
                k4 = kt[:].rearrange("p (j hd) -> p j hd", j=4)
                for i in range(4):
                    pbig = attn_p.tile([128, TOK], BF16, tag="pbig",
                                       name="pbig", bufs=2)
                    qi = qt[:, i * 512:(i + 1) * 512].unsqueeze(1) \
                        .broadcast_to([128, 4, 512])
                    nc.vector.tensor_mul(
                        pbig[:].rearrange("p (j hd) -> p j hd", j=4), qi, k4)
                    st1 = attn_p.tile([128, TOK // 2], BF16, tag="qks1",
                                      name="qks1", bufs=2)
                    st2 = attn_p.tile([128, TOK // 4], BF16, tag="qks2",
                                      name="qks2", bufs=2)
                    pv = pbig[:].rearrange("p (g d) -> p g d", g=32)
                    nc.vector.tensor_tensor(
                        st1[:].rearrange("p (g d) -> p g d", g=32),
                        pv[:, :, 0:32], pv[:, :, 32:64], op=ALU.add)
                    sv = st1[:].rearrange("p (g d) -> p g d", g=32)
                    nc.vector.tensor_tensor(
                        st2[:].rearrange("p (g d) -> p g d", g=32),
                        sv[:, :, 0:16], sv[:, :, 16:32], op=ALU.add)
                    nc.vector.tensor_reduce(
                        s_sc[:, i * 32:(i + 1) * 32],
                        st2[:].rearrange("p (g d) -> p g d", g=32),
                        axis=AX.X, op=ALU.add)
                # softmax over j without max-subtraction (logits bounded)
                # S cols = (i, j, h)
                eexp = attn_p.tile([128, 128], BF16, tag="eexp", name="eexp",
                                   bufs=2)
                nc.scalar.activation(eexp[:], s_sc[:], AF.Exp)
                z = attn_p.tile([128, 32], F32, tag="z", name="zt", bufs=2)
                nc.vector.tensor_reduce(
                    z[:].rearrange("p (i h) -> p i h", i=4),
                    eexp[:].rearrange("p (i j h) -> p i h j", i=4, j=4),
                    axis=AX.X, op=ALU.add)
                zr = attn_p.tile([128, 32], F32, tag="zr", name="zr", bufs=2)
                nc.vector.reciprocal(zr[:], z[:])
                a_t = attn_p.tile([128, 128], BF16, tag="a_t", name="a_t",
                                  bufs=2)
                nc.vector.tensor_mul(
                    a_t[:].rearrange("p (i j h) -> p i j h", i=4, j=4),
                    eexp[:].rearrange("p (i j h) -> p i j h", i=4, j=4),
                    zr[:].rearrange("p (i h) -> p i h", i=4).unsqueeze(2)
                    .broadcast_to([128, 4, 4, 8]))
                for i in range(4):
                    tbig = attn_p.tile([128, TOK], BF16, tag="tbig", name="tbig", bufs=2)
                    ablk = a_t[:, i * 32:(i + 1) * 32] \
                        .rearrange("p (j h) -> p j h", j=4) \
                        .unsqueeze(2).broadcast_to([128, 4, 64, 8])
                    nc.vector.tensor_mul(
                        tbig[:].rearrange("p (j d h) -> p j d h", j=4, d=64),
                        vt[:].rearrange("p (j d h) -> p j d h", j=4, d=64),
                        ablk)
                    av01 = attn_p.tile([128, 512], BF16, tag="av01", name="av01",
                                       bufs=2)
                    av = attn_p.tile([128, 512], BF16, tag="av", name="av",
                                     bufs=2)
                    nc.vector.tensor_tensor(av01[:], tbig[:, 0:512],
                                            tbig[:, 512:1024], op=ALU.add)
                    nc.vector.tensor_tensor(av[:], tbig[:, 1024:1536],
                                            tbig[:, 1536:2048], op=ALU.add)
                    nc.gpsimd.tensor_tensor(av[:], av01[:], av[:],
                                            op=ALU.add)
                    pst = pp_bc.tile([128, 512], BF16, tag="bc", name="pst")
                    for c in range(4):
                        nc.tensor.transpose(pst[:, c * 128:(c + 1) * 128],
                                            av[:, c * 128:(c + 1) * 128],
                                            ident[:])
                    nc.scalar.copy(
                        otb_all[:].rearrange("p (c t) -> p c t", c=4)
                        [:, :, i * 512 + st * 128:i * 512 + st * 128 + 128],
                        pst[:].rearrange("p (c t) -> p c t", c=4))

            wo = [wp.tile([128, D], BF16, tag=f"wo{k}", name=f"wo{k}")
                  for k in range(4)]
            for k in range(4):
                nc.sync.dma_start(wo[k][:], wo_d[li, k * 128:(k + 1) * 128, :])
            for m in range(4):
                for ns in range(4):
                    ps = pp_mm.tile([128, 512], F32, tag="mm", name="ops")
                    for k in range(4):
                        nc.tensor.matmul(
                            ps[:], wo[k][:, m * 128:(m + 1) * 128],
                            otb_all[:, k * TOK + ns * 512:k * TOK + (ns + 1) * 512],
                            start=(k == 0), stop=(k == 3))
                    nc.vector.scalar_tensor_tensor(
                        x_mid[m][:, ns * 512:(ns + 1) * 512], ps[:],
                        cols[:, 4 + m:5 + m], x_in[m][:, ns * 512:(ns + 1) * 512],
                        op0=ALU.add, op1=ALU.add)

            w1 = [wp.tile([128, D], F32R, tag=f"w1_{k}", name=f"w1_{k}")
                  for k in range(4)]
            w2 = [wp.tile([128, D], F32R, tag=f"w2_{k}", name=f"w2_{k}")
                  for k in range(4)]
            for k in range(4):
                nc.sync.dma_start(w1[k][:], w1_d[li, k * 128:(k + 1) * 128, :])
                nc.sync.dma_start(w2[k][:], w2_d[li, k * 128:(k + 1) * 128, :])
            for ns in range(4):
                xh2 = [sl_p.tile([128, 512], F32R, tag=f"xh2_{k}", name=f"xh2_{k}")
                       for k in range(4)]
                ln_cols(x_mid,
                        lambda k: x_mid[k][:, ns * 512:(ns + 1) * 512],
                        xh2, slice(0, 512))
                hsl = [sl_p.tile([128, 512], F32R, tag=f"h_{m}", name=f"hsl{m}")
                       for m in range(4)]
                for m in range(4):
                    ps = pp_mm.tile([128, 512], F32, tag="mm", name="m1ps")
                    for k in range(4):
                        nc.tensor.matmul(
                            ps[:], w1[k][:, m * 128:(m + 1) * 128], xh2[k][:],
                            start=(k == 0), stop=(k == 3))
                    nc.scalar.activation(hsl[m][:], ps[:], AF.Gelu_apprx_tanh,
                                         bias=cols[:, m:m + 1])
                for m in range(4):
                    ps = pp_mm.tile([128, 512], F32, tag="mm", name="m2ps")
                    for k in range(4):
                        nc.tensor.matmul(
                            ps[:], w2[k][:, m * 128:(m + 1) * 128], hsl[k][:],
                            start=(k == 0), stop=(k == 3))
                    nc.vector.scalar_tensor_tensor(
                        x_mid[m][:, ns * 512:(ns + 1) * 512], ps[:],
                        cols[:, 8 + m:9 + m], x_mid[m][:, ns * 512:(ns + 1) * 512],
                        op0=ALU.add, op1=ALU.add)

        cur = xA
        _nl = int(os.environ.get("KNLAYERS", NLAYER))
        for li in range(_nl):
            transformer_layer(li, cur, cur)

        tr_ctx.close()

        # ------------------------------------------------------------------
        # heads
        # ------------------------------------------------------------------
        _skip_heads = os.environ.get("KHEADS", "1") == "0"
        if _skip_heads:
            nc.gpsimd.dma_start(d6_o[:], cur[0][0:8, :])
            nc.gpsimd.dma_start(tr_o[:], cur[1][0:8, :])
        with tc.tile_pool(name="heads", bufs=1) as hp, \
             tc.tile_pool(name="whp", bufs=1) as whp:
          if not _skip_heads:
              hc = stat_p.tile([128, 10], F32, tag="hcols", name="hc")
              nc.sync.dma_start(hc[:], hcols_d[:])
              sb3 = stat_p.tile([8, 2], F32, tag="sb3", name="sb3")
              nc.sync.dma_start(sb3[:], sb3_d[:])

              xhf = [hp.tile([128, TOK], F32R, tag=f"xhf{k}", name=f"xhf{k}")
                     for k in range(4)]
              for st in range(4):
                  ln_cols(cur, lambda k: cur[k][:, st * 512:(st + 1) * 512],
                          xhf, slice(st * 512, (st + 1) * 512))

              xp = [hp.tile([128, TOK], F32R, tag=f"xp{k}", name=f"xp{k}")
                    for k in range(4)]

              def mm_head(src_tiles, wt_dram, kdim, mdim, dst_tiles, evict):
                  KC = kdim // 128
                  MC = max(mdim // 128, 1)
                  wsb = [whp.tile([128, mdim], F32R, tag=f"wh_{kdim}_{mdim}_{k}",
                                  name=f"wh{k}") for k in range(KC)]
                  for k in range(KC):
                      nc.sync.dma_start(wsb[k][:], wt_dram[k * 128:(k + 1) * 128, :])
                  for m in range(MC):
                      for ns in range(4):
                          ps = pp_mm.tile([128, 512], F32, tag="mm", name="hps")
                          for k in range(KC):
                              nc.tensor.matmul(
                                  ps[:], wsb[k][:, m * 128:(m + 1) * 128],
                                  src_tiles[k][:, ns * 512:(ns + 1) * 512],
                                  start=(k == 0), stop=(k == KC - 1))
                          evict(ps, dst_tiles[m], m, ns)

              mm_head(xhf, projw_d, D, D, xp,
                      lambda ps, dst, m, ns: nc.scalar.activation(
                          dst[:, ns * 512:(ns + 1) * 512], ps[:], AF.Identity,
                          bias=hc[:, m:m + 1]))

              def branch(w1d, w2d, w3d, b1ofs, b2ofs, out_dram, b3col, r1, r2, pfx):
                  mm_head(xp, w1d, D, 256, r1,
                          lambda ps, dst, m, ns: nc.scalar.activation(
                              dst[:, ns * 512:(ns + 1) * 512], ps[:], AF.Relu,
                              bias=hc[:, b1ofs + m:b1ofs + m + 1]))
                  mm_head(r1, w2d, 256, 128, r2,
                          lambda ps, dst, m, ns: nc.scalar.activation(
                              dst[:, ns * 512:(ns + 1) * 512], ps[:], AF.Relu,
                              bias=hc[:, b2ofs:b2ofs + 1]))
                  w3 = whp.tile([128, 8], F32R, tag=f"w3{pfx}", name="w3")
                  nc.sync.dma_start(w3[:], w3d[:])
                  out_sb = hp.tile([8, TOK], F32, tag=f"{pfx}out", name=f"{pfx}out")
                  for ns in range(4):
                      ps = pp_mm.tile([8, 512], F32, tag="mm", name="bps")
                      nc.tensor.matmul(ps[:], w3[:], r2[0][:, ns * 512:(ns + 1) * 512],
                                       start=True, stop=True)
                      nc.scalar.activation(out_sb[:, ns * 512:(ns + 1) * 512], ps[:],
                                           AF.Identity, bias=b3col)
                      nc.sync.dma_start(out_dram[:, ns * 512:(ns + 1) * 512],
                                        out_sb[:, ns * 512:(ns + 1) * 512])

              # reuse dead transformer buffers for intermediates
              branch(rw1_d, rw2_d, rw3_d, 4, 8, d6_o, sb3[:, 0:1],
                     [cur[0], cur[1]], [xhf[0]], "r")
              branch(tw1_d, tw2_d, tw3_d, 6, 9, tr_o, sb3[:, 1:2],
                     [cur[2], cur[3]], [xhf[1]], "t")

    nc.compile()
    return nc


# ----------------------------------------------------------------------------
# host side
# ----------------------------------------------------------------------------

_CACHE = {}


def _normalize_np(v, eps=1e-12):
    return v / np.maximum(np.linalg.norm(v, axis=-1, keepdims=True), eps)


def _rot6d_np(d6):
    a1, a2 = d6[..., :3], d6[..., 3:]
    b1 = _normalize_np(a1)
    b2 = _normalize_np(a2 - np.sum(b1 * a2, -1, keepdims=True) * b1)
    b3 = np.cross(b1, b2)
    return np.stack([b1, b2, b3], axis=-2)


def _prep_weights(inp):
    f32 = np.float32
    wmap = {}
    for i, cw in enumerate(['c1w', 'c2w', 'c3w', 'c4w']):
        wmap[f'convw{i}'] = np.ascontiguousarray(inp[cw].T.astype(f32))
    for i, (g, b2) in enumerate([('bn1g', 'bn1b'), ('bn2g', 'bn2b'),
                                 ('bn3g', 'bn3b')]):
        M = CONV_DIMS[i + 1] // 128
        bn = np.concatenate([
            inp[g].reshape(M, 128).T, inp[b2].reshape(M, 128).T,
            inp[f'c{i + 1}b'].reshape(M, 128).T], axis=1)
        wmap[f'bnconst{i}'] = np.ascontiguousarray(bn.astype(f32))
    pe = _pe_table()[:NF]
    b4 = inp['c4b'][None, :].astype(f32) + pe                   # (4, 512)
    # cols: m*4 + pt ; frame index == pt
    wmap['bias4'] = np.ascontiguousarray(
        b4.reshape(NF, 4, 128).transpose(2, 1, 0).reshape(128, 16).astype(f32))

    qkvw = np.array(inp['qkvw'], f32)
    qkvb = np.array(inp['qkvb'], f32)
    qkvw[:, :, :512] /= math.sqrt(DH)
    qkvb[:, :512] /= math.sqrt(DH)
    g1 = np.array(inp['ln1g'], f32)
    b1 = np.array(inp['ln1b'], f32)
    wq_fold = g1[:, :, None] * qkvw
    bq_fold = qkvb + np.einsum('ld,ldf->lf', b1, qkvw)
    wmap['wqkv'] = np.ascontiguousarray(wq_fold.astype(f32))
    wsbq = np.concatenate([wq_fold.sum(axis=1, keepdims=True),
                           bq_fold[:, None, :]], axis=1)         # (L, 2, 3D)
    wmap['wsbq'] = np.ascontiguousarray(wsbq.astype(ml_dtypes.bfloat16))
    # attention output features are (d, h)-ordered; permute wo rows to match
    wo_ = np.array(inp['outw'], f32)                             # (L, 512, 512)
    d_idx, h_idx = np.meshgrid(np.arange(DH), np.arange(HEADS), indexing='ij')
    perm = (h_idx * DH + d_idx).reshape(512)     # perm[d*8+h] = h*64+d
    wmap['wo'] = np.ascontiguousarray(wo_[:, perm, :]
                                      .astype(ml_dtypes.bfloat16))
    g2 = np.array(inp['ln2g'], f32)
    bl2 = np.array(inp['ln2b'], f32)
    m1w = np.array(inp['m1w'], f32)
    w1_fold = g2[:, :, None] * m1w
    b1_fold = np.array(inp['m1b'], f32) + np.einsum('ld,ldf->lf', bl2, m1w)
    wmap['w1'] = np.ascontiguousarray(w1_fold.astype(f32))
    wmap['w2'] = np.ascontiguousarray(np.array(inp['m2w'], f32))
    cols = np.zeros((NLAYER, 128, 12), f32)
    cols[:, :, 0:4] = b1_fold.reshape(NLAYER, 4, 128).transpose(0, 2, 1)
    # v-bias is dropped at the v eviction; fold bv @ Wo into outb instead
    bv = bq_fold[:, 2 * 512:3 * 512]                             # (L, 512)
    outb_fold = np.array(inp['outb'], f32) + np.einsum('lk,lko->lo', bv, wo_)
    cols[:, :, 4:8] = outb_fold.reshape(NLAYER, 4, 128).transpose(0, 2, 1)
    cols[:, :, 8:12] = np.array(inp['m2b'], f32).reshape(NLAYER, 4, 128) \
        .transpose(0, 2, 1)
    wmap['tcols'] = cols

    gf_ = np.array(inp['lnfg'], f32)
    bf_ = np.array(inp['lnfb'], f32)
    projw = np.array(inp['projw'], f32)
    wmap['projw'] = np.ascontiguousarray(gf_[:, None] * projw)
    projb_fold = np.array(inp['projb'], f32) + bf_ @ projw
    wmap['rw1'] = np.ascontiguousarray(np.array(inp['rw1'], f32))
    wmap['rw2'] = np.ascontiguousarray(np.array(inp['rw2'], f32))
    rw3 = np.zeros((128, 8), f32)
    rw3[:, :6] = np.array(inp['rw3'], f32)
    wmap['rw3'] = rw3
    wmap['tw1'] = np.ascontiguousarray(np.array(inp['tw1'], f32))
    wmap['tw2'] = np.ascontiguousarray(np.array(inp['tw2'], f32))
    tw3 = np.zeros((128, 8), f32)
    tw3[:, :3] = np.array(inp['tw3'], f32)
    wmap['tw3'] = tw3
    hcols = np.zeros((128, 10), f32)
    hcols[:, 0:4] = projb_fold.reshape(4, 128).T
    hcols[:, 4:6] = np.array(inp['rb1'], f32).reshape(2, 128).T
    hcols[:, 6:8] = np.array(inp['tb1'], f32).reshape(2, 128).T
    hcols[:, 8] = np.array(inp['rb2'], f32)
    hcols[:, 9] = np.array(inp['tb2'], f32)
    wmap['hcols'] = hcols
    sb3 = np.zeros((8, 2), f32)
    sb3[0:6, 0] = np.array(inp['rb3'], f32)
    sb3[0:3, 1] = np.array(inp['tb3'], f32)
    wmap['sb3'] = sb3
    wmap['ones_c'] = np.ones((128, 128), f32)
    return wmap


def kernel(**inputs):
    inp = {k: np.asarray(v) for k, v in inputs.items()}

    idx = inp['seed_idxs'].reshape(B, -1).astype(np.int64)      # (B, N)
    sel_seed = np.take_along_axis(np.asarray(inp['fp2_features'], np.float32),
                                  idx[:, None, :], axis=2)
    sel_grasp = np.take_along_axis(np.asarray(inp['local_grasp_features'], np.float32),
                                   idx[:, None, :], axis=2)
    sel_color = np.take_along_axis(np.asarray(inp['local_color_features'], np.float32),
                                   idx[:, None, :], axis=2)
    sel_pose = np.take_along_axis(np.asarray(inp['grasp_pose_feature'], np.float32),
                                  idx[:, None, :], axis=2)
    gsf = np.asarray(inp['sa4_features'], np.float32).max(axis=-1)
    gsf = np.broadcast_to(gsf[:, :, None], (B, 256, NPTS))
    fused = sel_pose + np.concatenate([sel_grasp, sel_color, sel_seed, gsf], axis=1)
    gf = fused.reshape(BE, FRAME, 1024, NPTS)
    cond = np.broadcast_to(gf[:, :1], (BE, NF, 1024, NPTS))
    X = np.concatenate([cond, gf[:, 1:]], axis=2)               # (e, f, 2048, N)

    if 'nc' not in _CACHE:
        _CACHE['nc'] = build_kernel()
    nc = _CACHE['nc']
    wmap = _prep_weights(inp)

    in_maps = []
    for k in range(NCORES):
        xc = X[:, :, :, k * NPC:(k + 1) * NPC]                  # (e, f, c, n)
        xc = xc.transpose(2, 1, 0, 3).reshape(2048, TOK)        # (c, (f,e,n))
        m = dict(wmap)
        m['xin'] = np.ascontiguousarray(xc, dtype=np.float32)
        in_maps.append(m)

    res = run_bass_kernel_spmd(nc, in_maps, core_ids=list(range(NCORES)))

    out = np.zeros((BE * NPTS, NF, 12), np.float32)
    for k in range(NCORES):
        d6 = res.results[k]['d6'][:6]
        tr = res.results[k]['tr3'][:3]
        d6 = d6.reshape(6, NF, BE, NPC).transpose(2, 3, 1, 0)   # (e, n, f, 6)
        tr = tr.reshape(3, NF, BE, NPC).transpose(2, 3, 1, 0)
        rot = _rot6d_np(d6).reshape(BE, NPC, NF, 9)
        for e in range(BE):
            rows = slice(e * NPTS + k * NPC, e * NPTS + (k + 1) * NPC)
            out[rows, :, 0:3] = tr[e]
            out[rows, :, 3:12] = rot[e]
    return out


if __name__ == "__main__":
    build_kernel()
    print("built ok")

